# revision 1
# baseline (speedup 1.0000x reference)
"""2-layer GAT on 8 Trainium2 NeuronCores (Bass/Tile).

Strategy (dst-sharded graph parallelism):
  - Layer-0 dense part (h = feat @ W0, plus fused attention-logit columns
    el = (h*al).sum(-1), er = (h*ar).sum(-1) via host-precomputed extra weight
    columns) is computed REPLICATED on every core into local DRAM tables
    (collectives are slow, ~62 GB/s, so replicating the cheap dense compute
    beats an AllGather of h).
  - Edges are sorted by destination on the host and sharded by dst-node range
    (6250 dst nodes per core).  Edge blocks of 128 land on SBUF partitions.
  - Per-edge source rows are fetched with dma_gather (int16 indices).  Since
    int16 only addresses 32768 rows, the node table is stored twice: window A
    = conceptual rows [0, 32768) and window B = conceptual rows [17234, 50002)
    where conceptual row 0 and row 50001 are zero guard rows and node i lives
    at conceptual row i+1.  Each edge block is single-window by construction
    (host splits each dst tile's edges into lo/hi runs, padded to x128).
  - er values (indexed by dst) are first compacted into a core-local table
    erloc[local_dst] via two window-gathers + add (invalid side hits a zero
    guard row), then per-edge er comes from a dma_gather on erloc with local
    (< 6250, int16-safe) indices.
  - Per edge block: expe = exp(max(x, 0.2x)) where x = el[src]+er[dst], a 0/1
    selection matrix mask[e, j] = (dstloc[e]==j) via DVE is_equal against an
    iota row, and one fp32 matmul per block accumulates BOTH the weighted
    message sum and the softmax denominator into PSUM:
        psum[j, 0:256] += sum_e mask[e,j] * (expe[e,h] * h[src_e])
        psum[j, 256:260] += sum_e mask[e,j] * expe[e,h]
  - Finalize divides by the denominator, runs layer-1's dense projection on
    the local dst slice, AllGathers the small [N, 42] projected table
    (8.4 MB), repacks it into window tables, and repeats the edge pipeline
    for layer 1 to produce the logits slice per core (host concatenates).

The edge structure is computed at runtime from the actual inputs and padded
to a uniform shape across cores (SPMD = one program for all 8 cores).
"""

import os
import numpy as np

import concourse.bass as bass
import concourse.bacc as bacc
import concourse.mybir as mybir
import concourse.tile as tile
from concourse.bass_utils import run_bass_kernel_spmd

F32 = mybir.dt.float32
I16 = mybir.dt.int16

SLOPE = 0.2
NCORES = 8
P = 128
G = 16          # max edge blocks per gather chunk
ST = 4          # dst tiles per supertile (lo/hi run batching)
CH = 8          # node tiles per phase-A chunk
WROWS = 32768   # rows per index window
LAST_EXEC_NS = [None]
LAST_RES = [None]
LAST_SIM = [None]
LAST_BUILD = [None]


def _bcast_inner(apv, count):
    return bass.AP(tensor=apv.tensor, offset=apv.offset, ap=apv.ap + [[0, count]])


def _bcast_mid(apv, count):
    a = apv.ap
    return bass.AP(tensor=apv.tensor, offset=apv.offset, ap=[a[0], [0, count]] + a[1:])


def _fuse_w(W, al, ar):
    Fin = W.shape[0]
    H, D = al.shape
    Wr = W.reshape(Fin, H, D)
    wl = np.einsum("khd,hd->kh", Wr, al).astype(np.float32)
    wr = np.einsum("khd,hd->kh", Wr, ar).astype(np.float32)
    return np.ascontiguousarray(np.concatenate([W, wl, wr], axis=1), dtype=np.float32)


def _wrap16(idx):
    """int16 idx list (len multiple of 128) -> dma_gather SBUF layout
    [128, len/16]: idx j at [j % 16, j // 16], replicated across 8 groups."""
    w = idx.reshape(-1, 16).T.astype(np.int16)
    return np.ascontiguousarray(np.tile(w, (8, 1)))


def _prep_edges(src, dst, n_nodes, ncores, wrows):
    from types import SimpleNamespace
    plan = SimpleNamespace()
    npc = n_nodes // ncores
    tpc = (npc + P - 1) // P
    plan.npc, plan.tpc = npc, tpc
    wa_max = wrows - 2               # node i valid in A iff i+1 <= wrows-1
    plan.wb_base = n_nodes + 2 - wrows

    order = np.argsort(dst, kind="stable")
    ss = src[order].astype(np.int64)
    ds = dst[order].astype(np.int64)
    core = ds // npc
    loc = ds % npc
    tileid = loc // P
    hi = (ss > wa_max).astype(np.int64)

    counts = np.zeros((ncores, tpc, 2), np.int64)
    np.add.at(counts, (core, tileid, hi), 1)
    nblk = (counts + P - 1) // P
    bcnt = nblk.max(axis=0)
    if bcnt.sum() == 0:
        bcnt[0, 0] = 1
    plan.bcnt = bcnt

    plan.sts = [list(range(s, min(s + ST, tpc))) for s in range(0, tpc, ST)]
    plan.order_blocks = []
    for tiles in plan.sts:
        for w in (0, 1):
            for t in tiles:
                plan.order_blocks += [(t, w)] * int(bcnt[t, w])
    plan.totblk = len(plan.order_blocks)
    plan.nedge = plan.totblk * P

    slot = {}
    pos = 0
    for (t, w) in plan.order_blocks:
        if (t, w) not in slot:
            slot[(t, w)] = pos
        pos += P

    srcw = np.zeros((ncores, plan.nedge), np.int64)
    erw = np.zeros((ncores, plan.nedge), np.int64)
    dstloc = np.full((ncores, plan.nedge), 999.0, np.float32)
    for bi, (t, w) in enumerate(plan.order_blocks):
        if w == 1:
            srcw[:, bi * P:(bi + 1) * P] = wrows - 1

    # order edges by (core, tile, win) groups
    gkey = (core * tpc + tileid) * 2 + hi
    g_order = np.argsort(gkey, kind="stable")
    ss2, loc2, gkey2 = ss[g_order], loc[g_order], gkey[g_order]
    gstart = np.zeros(ncores * tpc * 2 + 1, np.int64)
    np.add.at(gstart[1:], gkey2, 1)
    gstart = np.cumsum(gstart)
    for c in range(ncores):
        for t in range(tpc):
            for w in (0, 1):
                k = (c * tpc + t) * 2 + w
                e0, e1 = int(gstart[k]), int(gstart[k + 1])
                cnt = e1 - e0
                if cnt == 0:
                    continue
                off = slot[(t, w)]
                srcs = ss2[e0:e1]
                srcw[c, off:off + cnt] = (
                    srcs + 1 if w == 0 else srcs + 1 - plan.wb_base)
                erw[c, off:off + cnt] = loc2[e0:e1]
                dstloc[c, off:off + cnt] = (loc2[e0:e1] % P).astype(np.float32)

    plan.srcw, plan.erw, plan.dstlocv = srcw, erw, dstloc
    return plan


def _edge_phase(nc, tc, pools, tabA_ap, tabB_ap, erloc_ap, row_w, er_off, nheads,
                hdim, plan, src16_sb, er16_sb, dstloc_sb, iota_sb, gw, finalize):
    """Edge pipeline for one layer.  Gathered row: [h | el | ...], gw elems
    (multiple of 64 f32).  er gathered from erloc rows (er value at er_off)."""
    d = nheads * hdim
    hg_pool, ms_pool, mask_pool, small_pool, psum_pool = pools
    # per-tile first/last block ids
    first_blk, last_blk = {}, {}
    for bi, (t, w) in enumerate(plan.order_blocks):
        if t not in first_blk:
            first_blk[t] = bi
        last_blk[t] = bi
    acc_by_tile = {}

    # chunks: maximal runs of <=G blocks within a single window
    chunks = []
    cur = None
    for bi, (t, w) in enumerate(plan.order_blocks):
        if cur is None or cur[0] != w or bi - cur[1] >= G:
            if cur is not None:
                chunks.append(cur)
            cur = [w, bi, bi + 1]
        else:
            cur[2] = bi + 1
        if cur[2] - cur[1] >= G:
            chunks.append(cur)
            cur = None
    if cur is not None:
        chunks.append(cur)

    for w, b0, b1 in chunks:
        nb = b1 - b0
        nidx = nb * P
        HG = hg_pool.tile([P, G, gw], F32, tag="hg", name="hg")
        nc.gpsimd.dma_gather(
            out_ap=HG[:, :nb, :], in_ap=(tabA_ap if w == 0 else tabB_ap),
            idxs_ap=src16_sb[:, b0 * 8:b1 * 8], num_idxs=nidx,
            num_idxs_reg=nidx, elem_size=gw, elem_step=row_w,
            single_packet=False)
        ERG = small_pool.tile([P, G, 64], F32, tag="erg", name="erg")
        nc.gpsimd.dma_gather(
            out_ap=ERG[:, :nb, :], in_ap=erloc_ap,
            idxs_ap=er16_sb[:, b0 * 8:b1 * 8], num_idxs=nidx,
            num_idxs_reg=nidx, elem_size=64, elem_step=64,
            single_packet=False)
        # expe = exp(max(x, slope*x)), x = el + er
        E4 = small_pool.tile([P, G, nheads], F32, tag="e4", name="e4")
        nc.vector.tensor_add(E4[:, :nb, :], HG[:, :nb, d:d + nheads],
                             ERG[:, :nb, er_off:er_off + nheads])
        ESC = small_pool.tile([P, G, nheads], F32, tag="esc", name="esc")
        nc.vector.tensor_scalar_mul(ESC[:, :nb, :], E4[:, :nb, :], SLOPE)
        nc.vector.tensor_tensor(out=E4[:, :nb, :], in0=E4[:, :nb, :],
                                in1=ESC[:, :nb, :], op=mybir.AluOpType.max)
        nc.scalar.activation(out=E4[:, :nb, :], in_=E4[:, :nb, :],
                             func=mybir.ActivationFunctionType.Exp)
        # mask[p, b, j] = (dstloc[p, b] == j)
        MASK = mask_pool.tile([P, G, P], F32, tag="mask", name="mask")
        nc.vector.tensor_tensor(
            out=MASK[:, :nb, :],
            in0=_bcast_inner(dstloc_sb[:, b0:b1], P),
            in1=_bcast_mid(iota_sb[:], nb),
            op=mybir.AluOpType.is_equal)
        # MS = [expe-scaled h | expe]
        msw = d + nheads
        MS = ms_pool.tile([P, G, msw], F32, tag="ms", name="ms")
        for h in range(nheads):
            nc.vector.tensor_tensor(
                out=MS[:, :nb, h * hdim:(h + 1) * hdim],
                in0=HG[:, :nb, h * hdim:(h + 1) * hdim],
                in1=_bcast_inner(E4[:, :nb, h:h + 1], hdim),
                op=mybir.AluOpType.mult)
        nc.scalar.copy(out=MS[:, :nb, d:d + nheads], in_=E4[:, :nb, :])
        for bi in range(b0, b1):
            t, _ = plan.order_blocks[bi]
            if bi == first_blk[t]:
                acc_by_tile[t] = psum_pool.tile([P, msw], F32, tag="acc",
                                                name="acc")
            acc = acc_by_tile[t]
            nc.tensor.matmul(acc[:], lhsT=MASK[:, bi - b0, :],
                             rhs=MS[:, bi - b0, :],
                             start=(bi == first_blk[t]),
                             stop=(bi == last_blk[t]))
            if bi == last_blk[t]:
                finalize(t, acc)
                del acc_by_tile[t]


def build_and_run(feat, src, dst, W0, al0, ar0, W1, al1, ar1, trace=False,
                  simulate=False):
    n_nodes = feat.shape[0]
    npc = n_nodes // NCORES
    nh0 = al0.shape[0]
    hid0 = al0.shape[1]
    d0 = nh0 * hid0                        # 256
    row0 = ((d0 + 2 * nh0 + 63) // 64) * 64  # 320 f32 = 1280B (x256B ok)
    nh1 = al1.shape[0]
    hid1 = al1.shape[1]
    d1 = nh1 * hid1                        # 40
    row1 = max(((d1 + 2 * nh1 + 63) // 64) * 64, 128)  # 128 f32 = 512B rows
    gw1 = row1
    in_dim = feat.shape[1]
    wrows = min(WROWS, n_nodes + 2)
    wb_base = n_nodes + 2 - wrows

    w0e = _fuse_w(W0, al0, ar0)            # [in_dim, d0+2nh0]
    w1e = _fuse_w(W1, al1, ar1)            # [d0, d1+2nh1]
    kchunks = d0 // P
    w1p = np.ascontiguousarray(
        w1e.reshape(kchunks, P, d1 + 2 * nh1).transpose(1, 0, 2))

    plan = _prep_edges(src, dst, n_nodes, NCORES, wrows)
    totblk = plan.totblk
    tpc = plan.tpc
    tpc_out = tpc

    # erloc build index lists (local node -> window row or zero guard)
    gidx = np.arange(npc, dtype=np.int64)
    npc_pad = ((npc + P - 1) // P) * P
    bia = np.zeros((NCORES, npc_pad), np.int64)
    bib = np.full((NCORES, npc_pad), wrows - 1, np.int64)
    for c in range(NCORES):
        g = c * npc + gidx
        a_ok = g + 1 <= wrows - 1
        bia[c, :npc] = np.where(a_ok, g + 1, 0)
        bib[c, :npc] = np.where(~a_ok, g + 1 - wb_base, wrows - 1)

    iota = np.broadcast_to(np.arange(P, dtype=np.float32), (P, P)).copy()
    ident = np.eye(P, dtype=np.float32)

    nc = bacc.Bacc(None, target_bir_lowering=False, num_devices=NCORES)
    feat_t = nc.declare_dram_parameter("feat", [n_nodes, in_dim], F32, False)
    w0e_t = nc.declare_dram_parameter("w0e", [in_dim, d0 + 2 * nh0], F32, False)
    w1e_t = nc.declare_dram_parameter("w1e", [P, kchunks, d1 + 2 * nh1], F32, False)
    iota_t = nc.declare_dram_parameter("iota", [P, P], F32, False)
    ident_t = nc.declare_dram_parameter("ident", [P, P], F32, False)
    src16_t = nc.declare_dram_parameter("src16", [P, totblk * 8], I16, False)
    er16_t = nc.declare_dram_parameter("er16", [P, totblk * 8], I16, False)
    dstloc_t = nc.declare_dram_parameter("dstloc", [P, totblk], F32, False)
    bia_t = nc.declare_dram_parameter("bia16", [P, npc_pad // 16], I16, False)
    bib_t = nc.declare_dram_parameter("bib16", [P, npc_pad // 16], I16, False)
    out_t = nc.declare_dram_parameter("out", [npc, d1], F32, True)

    tab0A = nc.dram_tensor("tab0A", [wrows, row0], F32)
    tab0B = nc.dram_tensor("tab0B", [wrows, row0], F32)
    tab1A = nc.dram_tensor("tab1A", [wrows, row1], F32)
    tab1B = nc.dram_tensor("tab1B", [wrows, row1], F32)
    erloc0 = nc.dram_tensor("erloc0", [npc_pad, 64], F32)
    erloc1 = nc.dram_tensor("erloc1", [npc_pad, 64], F32)
    h2slice = nc.dram_tensor("h2slice", [npc, d1 + 2 * nh1], F32)
    h2full = nc.dram_tensor("h2full", [NCORES, npc, d1 + 2 * nh1], F32,
                            addr_space="Shared")

    debug = os.environ.get("GAT_DEBUG", "0") == "1"
    phases = os.environ.get("GAT_PHASES", "full")
    if debug:
        dbg_t = {
            "tab0A": nc.declare_dram_parameter("dbg_tab0A", [wrows, row0], F32, True),
            "erloc0": nc.declare_dram_parameter("dbg_erloc0", [npc_pad, 64], F32, True),
            "h2s": nc.declare_dram_parameter("dbg_h2s", [npc, d1 + 2 * nh1], F32, True),
            "erloc1": nc.declare_dram_parameter("dbg_erloc1", [npc_pad, 64], F32, True),
        }

    nt_full = n_nodes // P
    rem = n_nodes - nt_full * P
    # phase-A window write ranges (node index ranges)
    wa_nodes = (0, wrows - 1)
    wb_nodes = (wb_base - 1, n_nodes)  # nodes wb_base-1 .. -> tabB rows i+1-wb_base

    with tile.TileContext(nc) as tc:
        with tc.tile_pool(name="singles", bufs=1) as singles:
            iota_sb = singles.tile([P, P], F32)
            nc.sync.dma_start(out=iota_sb[:], in_=iota_t.ap())
            ident_sb = singles.tile([P, P], F32)
            nc.sync.dma_start(out=ident_sb[:], in_=ident_t.ap())
            w0e_sb = singles.tile([P, d0 + 2 * nh0], F32)
            nc.sync.dma_start(out=w0e_sb[:], in_=w0e_t.ap())
            w1e_sb = singles.tile([P, kchunks, d1 + 2 * nh1], F32)
            nc.sync.dma_start(out=w1e_sb[:], in_=w1e_t.ap())
            src16_sb = singles.tile([P, totblk * 8], I16)
            nc.sync.dma_start(out=src16_sb[:], in_=src16_t.ap())
            er16_sb = singles.tile([P, totblk * 8], I16)
            nc.sync.dma_start(out=er16_sb[:], in_=er16_t.ap())
            dstloc_sb = singles.tile([P, totblk], F32)
            nc.sync.dma_start(out=dstloc_sb[:], in_=dstloc_t.ap())
            bia_sb = singles.tile([P, npc_pad // 16], I16)
            nc.sync.dma_start(out=bia_sb[:], in_=bia_t.ap())
            bib_sb = singles.tile([P, npc_pad // 16], I16)
            nc.sync.dma_start(out=bib_sb[:], in_=bib_t.ap())
            zrow = singles.tile([P, row0], F32)
            nc.vector.memset(zrow[:], 0.0)
            # zero guard rows
            nc.sync.dma_start(out=tab0A.ap()[0:1], in_=zrow[:1, :row0])
            nc.sync.dma_start(out=tab0B.ap()[wrows - 1:wrows], in_=zrow[:1, :row0])
            nc.sync.dma_start(out=tab1A.ap()[0:1], in_=zrow[:1, :row1])
            nc.sync.dma_start(out=tab1B.ap()[wrows - 1:wrows], in_=zrow[:1, :row1])

            # ---- Phase A: replicated dense layer 0 -> tab0A/tab0B ----
            with (tc.tile_pool(name="pa", bufs=2) as pa,
                  tc.tile_pool(name="pa_fts", bufs=3) as pa_fts,
                  tc.tile_pool(name="pa_ps", bufs=2, space="PSUM") as pa_ps,
                  tc.tile_pool(name="pa_ph", bufs=2, space="PSUM") as pa_ph):
                base = 0
                while base < n_nodes:
                    ch = min(CH, (n_nodes - base) // P)
                    partial = ch == 0
                    ch = max(ch, 1)
                    rows = rem if partial else ch * P
                    fchunk = pa.tile([P, CH, in_dim], F32, tag="fchunk",
                                     name="fchunk")
                    if partial:
                        nc.vector.memset(fchunk[:, 0, :], 0.0)
                        nc.sync.dma_start(out=fchunk[:rows, 0, :],
                                          in_=feat_t.ap()[base:base + rows])
                    else:
                        nc.sync.dma_start(
                            out=fchunk[:, :ch, :],
                            in_=feat_t.ap()[base:base + rows].rearrange(
                                "(i p) d -> p i d", p=P))
                    hstage = pa.tile([P, CH, row0], F32, tag="hstage",
                                     name="hstage")
                    if row0 > d0 + 2 * nh0:
                        nc.vector.memset(hstage[:, :, d0 + 2 * nh0:row0], 0.0)
                    for i in range(ch):
                        ftp = pa_ps.tile([P, P], F32, name="ftp")
                        nc.tensor.transpose(ftp[:], fchunk[:, i, :], ident_sb[:])
                        fts = pa_fts.tile([P, P], F32, name="fts")
                        nc.scalar.copy(out=fts[:], in_=ftp[:])
                        hps = pa_ph.tile([P, d0 + 2 * nh0], F32, name="hps")
                        nc.tensor.matmul(hps[:], lhsT=fts[:], rhs=w0e_sb[:],
                                         start=True, stop=True)
                        nc.scalar.copy(out=hstage[:, i, 0:d0 + 2 * nh0],
                                       in_=hps[:])
                    # write chunk rows [base, base+rows) to each window table
                    vw = row0
                    for (tab, lo_n, hi_n, roff) in (
                            (tab0A, wa_nodes[0], wa_nodes[1], 1),
                            (tab0B, wb_nodes[0], wb_nodes[1], 1 - wb_base)):
                        lo = max(base, lo_n)
                        hi = min(base + rows, hi_n)
                        if lo >= hi:
                            continue
                        if partial:
                            nc.sync.dma_start(
                                out=tab.ap()[lo + roff:hi + roff, 0:vw],
                                in_=hstage[lo - base:hi - base, 0, 0:vw])
                        elif lo == base and hi == base + rows:
                            nc.sync.dma_start(
                                out=tab.ap()[lo + roff:hi + roff, 0:vw].rearrange(
                                    "(i p) d -> p i d", p=P),
                                in_=hstage[:, :ch, 0:vw])
                        else:
                            for i in range(ch):
                                t0 = base + i * P
                                l2, h2 = max(lo, t0), min(hi, t0 + P)
                                if l2 >= h2:
                                    continue
                                nc.sync.dma_start(
                                    out=tab.ap()[l2 + roff:h2 + roff, 0:vw],
                                    in_=hstage[l2 - t0:h2 - t0, i, 0:vw])
                    base += rows

            # ---- shared pools for edge phases ----
            with (tc.tile_pool(name="hg", bufs=2) as hg_pool,
                  tc.tile_pool(name="ms", bufs=2) as ms_pool,
                  tc.tile_pool(name="mk", bufs=2) as mask_pool,
                  tc.tile_pool(name="sm", bufs=3) as small_pool,
                  tc.tile_pool(name="fin", bufs=2) as fin_pool,
                  tc.tile_pool(name="ps_acc", bufs=5, space="PSUM") as psum_pool,
                  tc.tile_pool(name="ps_tp", bufs=2, space="PSUM") as psum_tp,
                  tc.tile_pool(name="ps_h2", bufs=1, space="PSUM") as psum_h2):

                def build_erloc(tabA, tabB, erloc, width, col0):
                    nseg = npc_pad // P
                    with tc.tile_pool(name="ebld", bufs=1) as ebld:
                        EA = ebld.tile([P, nseg, 64], F32, tag="erga", name="ea")
                        nc.gpsimd.dma_gather(
                            out_ap=EA[:], in_ap=tabA.ap()[:, col0:col0 + 64],
                            idxs_ap=bia_sb[:], num_idxs=npc_pad,
                            num_idxs_reg=npc_pad, elem_size=64, elem_step=width,
                            single_packet=False)
                        EB = ebld.tile([P, nseg, 64], F32, tag="ergb", name="eb")
                        nc.gpsimd.dma_gather(
                            out_ap=EB[:], in_ap=tabB.ap()[:, col0:col0 + 64],
                            idxs_ap=bib_sb[:], num_idxs=npc_pad,
                            num_idxs_reg=npc_pad, elem_size=64, elem_step=width,
                            single_packet=False)
                        nc.vector.tensor_add(EA[:], EA[:], EB[:])
                        nc.sync.dma_start(
                            out=erloc.ap().rearrange("(i p) d -> p i d", p=P),
                            in_=EA[:])

                # erloc0: er at table cols [260:264] -> stored col 4+256-260...
                # gather window [row0-64, row0) covers cols 256:320; er is at
                # cols 260:264 -> offset 4 within the gathered 64
                if phases != "a":
                    build_erloc(tab0A, tab0B, erloc0, row0, row0 - 64)
                er_off0 = (d0 + nh0) - (row0 - 64)   # = 260-256 = 4

                def finalize0(t, acc):
                    rows = min(P, npc - t * P)
                    S = small_pool.tile([P, nh0], F32, tag="s0", name="s0")
                    nc.vector.tensor_scalar_max(S[:], acc[:, d0:d0 + nh0], 1e-30)
                    RC = small_pool.tile([P, nh0], F32, tag="rc0", name="rc0")
                    nc.vector.reciprocal(RC[:], S[:])
                    H1T = fin_pool.tile([P, d0], F32, tag="h1t", name="h1t")
                    nc.vector.tensor_tensor(
                        out=H1T[:].rearrange("p (h e) -> p h e", h=nh0),
                        in0=acc[:, 0:d0].rearrange("p (h e) -> p h e", h=nh0),
                        in1=_bcast_inner(RC[:], hid0),
                        op=mybir.AluOpType.mult)
                    h2ps = psum_h2.tile([P, d1 + 2 * nh1], F32, name="h2ps")
                    for k in range(kchunks):
                        tp = psum_tp.tile([P, P], F32, name="tp")
                        nc.tensor.transpose(tp[:], H1T[:, k * P:(k + 1) * P],
                                            ident_sb[:])
                        ts = fin_pool.tile([P, P], F32, tag="tsb", name="tsb")
                        nc.scalar.copy(out=ts[:], in_=tp[:])
                        nc.tensor.matmul(h2ps[:], lhsT=ts[:], rhs=w1e_sb[:, k, :],
                                         start=(k == 0), stop=(k == kchunks - 1))
                    h2sb = fin_pool.tile([P, d1 + 2 * nh1], F32, tag="h2sb",
                                         name="h2sb")
                    nc.scalar.copy(out=h2sb[:], in_=h2ps[:])
                    nc.sync.dma_start(out=h2slice.ap()[t * P:t * P + rows],
                                      in_=h2sb[:rows, :])

                if phases != "a":
                    _edge_phase(nc, tc,
                                (hg_pool, ms_pool, mask_pool, small_pool,
                                 psum_pool),
                                tab0A.ap(), tab0B.ap(), erloc0.ap(), row0,
                                er_off0, nh0, hid0, plan, src16_sb, er16_sb,
                                dstloc_sb, iota_sb, row0, finalize0)

                # ---- AllGather projected table, repack into window tables ----
                run_l1 = phases in ("full", "abc")
                if run_l1:
                    nc.gpsimd.collective_compute(
                    "AllGather", mybir.AluOpType.bypass,
                        replica_groups=[list(range(NCORES))],
                        ins=[h2slice.ap()], outs=[h2full.ap()])
                    h2flat = h2full.ap().rearrange("c n d -> (c n) d")
                    rw1 = d1 + 2 * nh1
                    na = min(wrows - 2, n_nodes - 1) + 1
                    nc.sync.dma_start(out=tab1A.ap()[1:1 + na, 0:rw1],
                                      in_=h2flat[0:na])
                    blo = max(wb_base - 1, 0)
                    nc.sync.dma_start(
                        out=tab1B.ap()[blo + 1 - wb_base:n_nodes + 1 - wb_base,
                                       0:rw1],
                        in_=h2flat[blo:n_nodes])
                    build_erloc(tab1A, tab1B, erloc1, row1, 0)
                er_off1 = d1 + nh1   # er-build window starts at col 0

                def finalize1(t, acc):  # noqa: indent-kept
                    rows = min(P, npc - t * P)
                    S = small_pool.tile([P, nh1], F32, tag="s1", name="s1")
                    nc.vector.tensor_scalar_max(S[:], acc[:, d1:d1 + nh1], 1e-30)
                    RC = small_pool.tile([P, nh1], F32, tag="rc1", name="rc1")
                    nc.vector.reciprocal(RC[:], S[:])
                    OUT = fin_pool.tile([P, d1], F32, tag="outt", name="outt")
                    nc.vector.tensor_scalar_mul(OUT[:], acc[:, 0:d1], RC[:, 0:1])
                    nc.sync.dma_start(out=out_t.ap()[t * P:t * P + rows],
                                      in_=OUT[:rows, :])

                if phases == "full":
                    _edge_phase(nc, tc,
                                (hg_pool, ms_pool, mask_pool, small_pool,
                                 psum_pool),
                                tab1A.ap(), tab1B.ap(), erloc1.ap(), row1,
                                er_off1, nh1, hid1, plan, src16_sb, er16_sb,
                                dstloc_sb, iota_sb, row1, finalize1)
                else:
                    ztile = fin_pool.tile([P, d1], F32, tag="outt", name="zout")
                    nc.vector.memset(ztile[:], 0.0)
                    for t in range(tpc_out):
                        rows = min(P, npc - t * P)
                        nc.sync.dma_start(out=out_t.ap()[t * P:t * P + rows],
                                          in_=ztile[:rows, :])

                if debug:
                    nc.sync.dma_start(out=dbg_t["tab0A"].ap(), in_=tab0A.ap())
                    nc.sync.dma_start(out=dbg_t["erloc0"].ap(), in_=erloc0.ap())
                    nc.sync.dma_start(out=dbg_t["h2s"].ap(), in_=h2slice.ap())
                    nc.sync.dma_start(out=dbg_t["erloc1"].ap(), in_=erloc1.ap())

    nc.compile()

    in_maps = []
    for c in range(NCORES):
        in_maps.append({
            "feat": np.ascontiguousarray(feat, dtype=np.float32),
            "w0e": w0e,
            "w1e": w1p,
            "iota": iota,
            "ident": ident,
            "src16": _wrap16(plan.srcw[c]),
            "er16": _wrap16(plan.erw[c]),
            "dstloc": np.ascontiguousarray(
                plan.dstlocv[c].reshape(totblk, P).T.astype(np.float32)),
            "bia16": np.ascontiguousarray(
                np.tile(bia[c].reshape(-1, 16).T.astype(np.int16), (8, 1))),
            "bib16": np.ascontiguousarray(
                np.tile(bib[c].reshape(-1, 16).T.astype(np.int16), (8, 1))),
        })
    LAST_BUILD[0] = (nc, in_maps)
    if simulate:
        from concourse import bass_interp
        sim = bass_interp.MultiCoreSim(nc, NCORES, ignore_data_errors=True)
        for c in range(NCORES):
            for k, v in in_maps[c].items():
                sim.cores[c].tensor(k)[:] = v
        sim.simulate()
        LAST_SIM[0] = sim
        out = np.concatenate(
            [np.array(sim.cores[c].tensor("out")) for c in range(NCORES)], axis=0)
        return out
    res = run_bass_kernel_spmd(nc, in_maps, list(range(NCORES)), trace=trace)
    LAST_RES[0] = res
    LAST_EXEC_NS[0] = res.exec_time_ns
    out = np.concatenate([res.results[c]["out"] for c in range(NCORES)], axis=0)
    return out


def kernel(feat, src, dst, W0, al0, ar0, W1, al1, ar1):
    trace = os.environ.get("GAT_TRACE", "0") == "1"
    out = build_and_run(np.asarray(feat), np.asarray(src), np.asarray(dst),
                        np.asarray(W0), np.asarray(al0), np.asarray(ar0),
                        np.asarray(W1), np.asarray(al1), np.asarray(ar1),
                        trace=trace)
    return out.astype(np.float32)



# revision 5
# speedup vs baseline: 1.1508x; 1.1508x over previous
"""2-layer GAT on 8 Trainium2 NeuronCores (Bass/Tile).

Strategy (dst-sharded graph parallelism):
  - Layer-0 dense part (h = feat @ W0, plus fused attention-logit columns
    el = (h*al).sum(-1), er = (h*ar).sum(-1) via host-precomputed extra weight
    columns) is computed REPLICATED on every core into local DRAM tables
    (collectives are slow, ~62 GB/s, so replicating the cheap dense compute
    beats an AllGather of h).
  - Edges are sorted by destination on the host and sharded by dst-node range
    (6250 dst nodes per core).  Edge blocks of 128 land on SBUF partitions.
  - Per-edge source rows are fetched with dma_gather (int16 indices).  Since
    int16 only addresses 32768 rows, the node table is stored twice: window A
    = conceptual rows [0, 32768) and window B = conceptual rows [17234, 50002)
    where conceptual row 0 and row 50001 are zero guard rows and node i lives
    at conceptual row i+1.  Each edge block is single-window by construction
    (host splits each dst tile's edges into lo/hi runs, padded to x128).
  - er values (indexed by dst) are first compacted into a core-local table
    erloc[local_dst] via two window-gathers + add (invalid side hits a zero
    guard row), then per-edge er comes from a dma_gather on erloc with local
    (< 6250, int16-safe) indices.
  - Per edge block: expe = exp(max(x, 0.2x)) where x = el[src]+er[dst], a 0/1
    selection matrix mask[e, j] = (dstloc[e]==j) via DVE is_equal against an
    iota row, and one fp32 matmul per block accumulates BOTH the weighted
    message sum and the softmax denominator into PSUM:
        psum[j, 0:256] += sum_e mask[e,j] * (expe[e,h] * h[src_e])
        psum[j, 256:260] += sum_e mask[e,j] * expe[e,h]
  - Finalize divides by the denominator, runs layer-1's dense projection on
    the local dst slice, AllGathers the small [N, 42] projected table
    (8.4 MB), repacks it into window tables, and repeats the edge pipeline
    for layer 1 to produce the logits slice per core (host concatenates).

The edge structure is computed at runtime from the actual inputs and padded
to a uniform shape across cores (SPMD = one program for all 8 cores).
"""

import os
import numpy as np

import concourse.bass as bass
import concourse.bacc as bacc
import concourse.mybir as mybir
import concourse.tile as tile
from concourse.bass_utils import run_bass_kernel_spmd

F32 = mybir.dt.float32
I16 = mybir.dt.int16

SLOPE = 0.2
NCORES = 8
P = 128
NQ = 4          # SWDGE queues (gather desc-gen runs on a Q7 cpu pair per queue)
G = 16          # max edge blocks per gather chunk
ST = 4          # dst tiles per supertile (lo/hi run batching)
CH = 8          # node tiles per phase-A chunk
WROWS = 32768   # rows per index window
LAST_EXEC_NS = [None]
LAST_RES = [None]
LAST_SIM = [None]
LAST_BUILD = [None]


def _bcast_inner(apv, count):
    return bass.AP(tensor=apv.tensor, offset=apv.offset, ap=apv.ap + [[0, count]])


def _bcast_mid(apv, count):
    a = apv.ap
    return bass.AP(tensor=apv.tensor, offset=apv.offset, ap=[a[0], [0, count]] + a[1:])


def _fuse_w(W, al, ar):
    Fin = W.shape[0]
    H, D = al.shape
    Wr = W.reshape(Fin, H, D)
    wl = np.einsum("khd,hd->kh", Wr, al).astype(np.float32)
    wr = np.einsum("khd,hd->kh", Wr, ar).astype(np.float32)
    return np.ascontiguousarray(np.concatenate([W, wl, wr], axis=1), dtype=np.float32)


def _wrap16(idx):
    """int16 idx list (len multiple of 128) -> dma_gather SBUF layout
    [128, len/16]: idx j at [j % 16, j // 16], replicated across 8 groups."""
    w = idx.reshape(-1, 16).T.astype(np.int16)
    return np.ascontiguousarray(np.tile(w, (8, 1)))


def _prep_edges(src, dst, n_nodes, ncores, wrows):
    from types import SimpleNamespace
    plan = SimpleNamespace()
    npc = n_nodes // ncores
    tpc = (npc + P - 1) // P
    plan.npc, plan.tpc = npc, tpc
    wa_max = wrows - 2               # node i valid in A iff i+1 <= wrows-1
    plan.wb_base = n_nodes + 2 - wrows

    order = np.argsort(dst, kind="stable")
    ss = src[order].astype(np.int64)
    ds = dst[order].astype(np.int64)
    core = ds // npc
    loc = ds % npc
    tileid = loc // P
    hi = (ss > wa_max).astype(np.int64)

    counts = np.zeros((ncores, tpc, 2), np.int64)
    np.add.at(counts, (core, tileid, hi), 1)
    nblk = (counts + P - 1) // P
    bcnt = nblk.max(axis=0)
    if bcnt.sum() == 0:
        bcnt[0, 0] = 1
    plan.bcnt = bcnt

    plan.sts = [list(range(s, min(s + ST, tpc))) for s in range(0, tpc, ST)]
    plan.order_blocks = []
    for tiles in plan.sts:
        for w in (0, 1):
            for t in tiles:
                plan.order_blocks += [(t, w)] * int(bcnt[t, w])
    plan.totblk = len(plan.order_blocks)
    plan.nedge = plan.totblk * P

    slot = {}
    pos = 0
    for (t, w) in plan.order_blocks:
        if (t, w) not in slot:
            slot[(t, w)] = pos
        pos += P

    srcw = np.zeros((ncores, plan.nedge), np.int64)
    erw = np.zeros((ncores, plan.nedge), np.int64)
    dstloc = np.full((ncores, plan.nedge), 999.0, np.float32)
    for bi, (t, w) in enumerate(plan.order_blocks):
        if w == 1:
            srcw[:, bi * P:(bi + 1) * P] = wrows - 1

    # order edges by (core, tile, win) groups
    gkey = (core * tpc + tileid) * 2 + hi
    g_order = np.argsort(gkey, kind="stable")
    ss2, loc2, gkey2 = ss[g_order], loc[g_order], gkey[g_order]
    gstart = np.zeros(ncores * tpc * 2 + 1, np.int64)
    np.add.at(gstart[1:], gkey2, 1)
    gstart = np.cumsum(gstart)
    for c in range(ncores):
        for t in range(tpc):
            for w in (0, 1):
                k = (c * tpc + t) * 2 + w
                e0, e1 = int(gstart[k]), int(gstart[k + 1])
                cnt = e1 - e0
                if cnt == 0:
                    continue
                off = slot[(t, w)]
                srcs = ss2[e0:e1]
                srcw[c, off:off + cnt] = (
                    srcs + 1 if w == 0 else srcs + 1 - plan.wb_base)
                erw[c, off:off + cnt] = loc2[e0:e1]
                dstloc[c, off:off + cnt] = (loc2[e0:e1] % P).astype(np.float32)

    plan.srcw, plan.erw, plan.dstlocv = srcw, erw, dstloc
    return plan


def _edge_phase(nc, tc, pools, tabA_ap, tabB_ap, erloc_ap, row_w, er_off, nheads,
                hdim, plan, src16_sb, er16_sb, dstloc_sb, iota_sb, gw, finalize):
    """Edge pipeline for one layer.  Gathered row: [h | el | ...], gw elems
    (multiple of 64 f32).  er gathered from erloc rows (er value at er_off)."""
    d = nheads * hdim
    hg_pool, ms_pool, mask_pool, small_pool, psum_pool = pools
    # per-tile first/last block ids
    first_blk, last_blk = {}, {}
    for bi, (t, w) in enumerate(plan.order_blocks):
        if t not in first_blk:
            first_blk[t] = bi
        last_blk[t] = bi
    acc_by_tile = {}

    # chunks: maximal runs of <=G blocks within a single window
    chunks = []
    cur = None
    for bi, (t, w) in enumerate(plan.order_blocks):
        if cur is None or cur[0] != w or bi - cur[1] >= G:
            if cur is not None:
                chunks.append(cur)
            cur = [w, bi, bi + 1]
        else:
            cur[2] = bi + 1
        if cur[2] - cur[1] >= G:
            chunks.append(cur)
            cur = None
    if cur is not None:
        chunks.append(cur)

    for ci, (w, b0, b1) in enumerate(chunks):
        nb = b1 - b0
        nidx = nb * P
        HG = hg_pool.tile([P, G, gw], F32, tag="hg", name="hg")
        nc.gpsimd.dma_gather(
            out_ap=HG[:, :nb, :], in_ap=(tabA_ap if w == 0 else tabB_ap),
            idxs_ap=src16_sb[:, b0 * 8:b1 * 8], num_idxs=nidx,
            num_idxs_reg=nidx, elem_size=gw, elem_step=row_w,
            single_packet=False, queue_num=(2 * ci) % NQ)
        ERG = small_pool.tile([P, G, 64], F32, tag="erg", name="erg")
        nc.gpsimd.dma_gather(
            out_ap=ERG[:, :nb, :], in_ap=erloc_ap,
            idxs_ap=er16_sb[:, b0 * 8:b1 * 8], num_idxs=nidx,
            num_idxs_reg=nidx, elem_size=64, elem_step=64,
            single_packet=False, queue_num=(2 * ci + 1) % NQ)
        # expe = exp(max(x, slope*x)), x = el + er
        E4 = small_pool.tile([P, G, nheads], F32, tag="e4", name="e4")
        nc.vector.tensor_add(E4[:, :nb, :], HG[:, :nb, d:d + nheads],
                             ERG[:, :nb, er_off:er_off + nheads])
        ESC = small_pool.tile([P, G, nheads], F32, tag="esc", name="esc")
        nc.vector.tensor_scalar_mul(ESC[:, :nb, :], E4[:, :nb, :], SLOPE)
        nc.vector.tensor_tensor(out=E4[:, :nb, :], in0=E4[:, :nb, :],
                                in1=ESC[:, :nb, :], op=mybir.AluOpType.max)
        nc.scalar.activation(out=E4[:, :nb, :], in_=E4[:, :nb, :],
                             func=mybir.ActivationFunctionType.Exp)
        # mask[p, b, j] = (dstloc[p, b] == j)
        MASK = mask_pool.tile([P, G, P], F32, tag="mask", name="mask")
        nc.vector.tensor_tensor(
            out=MASK[:, :nb, :],
            in0=_bcast_inner(dstloc_sb[:, b0:b1], P),
            in1=_bcast_mid(iota_sb[:], nb),
            op=mybir.AluOpType.is_equal)
        # MS = [expe-scaled h | expe]
        msw = d + nheads
        MS = ms_pool.tile([P, G, msw], F32, tag="ms", name="ms")
        for h in range(nheads):
            nc.vector.tensor_tensor(
                out=MS[:, :nb, h * hdim:(h + 1) * hdim],
                in0=HG[:, :nb, h * hdim:(h + 1) * hdim],
                in1=_bcast_inner(E4[:, :nb, h:h + 1], hdim),
                op=mybir.AluOpType.mult)
        nc.scalar.copy(out=MS[:, :nb, d:d + nheads], in_=E4[:, :nb, :])
        for bi in range(b0, b1):
            t, _ = plan.order_blocks[bi]
            if bi == first_blk[t]:
                acc_by_tile[t] = psum_pool.tile([P, msw], F32, tag="acc",
                                                name="acc")
            acc = acc_by_tile[t]
            nc.tensor.matmul(acc[:], lhsT=MASK[:, bi - b0, :],
                             rhs=MS[:, bi - b0, :],
                             start=(bi == first_blk[t]),
                             stop=(bi == last_blk[t]))
            if bi == last_blk[t]:
                finalize(t, acc)
                del acc_by_tile[t]


def build_and_run(feat, src, dst, W0, al0, ar0, W1, al1, ar1, trace=False,
                  simulate=False):
    n_nodes = feat.shape[0]
    npc = n_nodes // NCORES
    nh0 = al0.shape[0]
    hid0 = al0.shape[1]
    d0 = nh0 * hid0                        # 256
    row0 = ((d0 + 2 * nh0 + 63) // 64) * 64  # 320 f32 = 1280B (x256B ok)
    nh1 = al1.shape[0]
    hid1 = al1.shape[1]
    d1 = nh1 * hid1                        # 40
    row1 = max(((d1 + 2 * nh1 + 63) // 64) * 64, 128)  # 128 f32 = 512B rows
    gw1 = row1
    in_dim = feat.shape[1]
    wrows = min(WROWS, n_nodes + 2)
    wb_base = n_nodes + 2 - wrows

    w0e = _fuse_w(W0, al0, ar0)            # [in_dim, d0+2nh0]
    w1e = _fuse_w(W1, al1, ar1)            # [d0, d1+2nh1]
    kchunks = d0 // P
    w1p = np.ascontiguousarray(
        w1e.reshape(kchunks, P, d1 + 2 * nh1).transpose(1, 0, 2))

    plan = _prep_edges(src, dst, n_nodes, NCORES, wrows)
    totblk = plan.totblk
    tpc = plan.tpc
    tpc_out = tpc

    # erloc build index lists (local node -> window row or zero guard)
    gidx = np.arange(npc, dtype=np.int64)
    npc_pad = ((npc + P - 1) // P) * P
    bia = np.zeros((NCORES, npc_pad), np.int64)
    bib = np.full((NCORES, npc_pad), wrows - 1, np.int64)
    for c in range(NCORES):
        g = c * npc + gidx
        a_ok = g + 1 <= wrows - 1
        bia[c, :npc] = np.where(a_ok, g + 1, 0)
        bib[c, :npc] = np.where(~a_ok, g + 1 - wb_base, wrows - 1)

    iota = np.broadcast_to(np.arange(P, dtype=np.float32), (P, P)).copy()
    ident = np.eye(P, dtype=np.float32)

    nc = bacc.Bacc(None, target_bir_lowering=False, num_devices=NCORES,
                   num_swdge_queues=NQ)
    feat_t = nc.declare_dram_parameter("feat", [n_nodes, in_dim], F32, False)
    w0e_t = nc.declare_dram_parameter("w0e", [in_dim, d0 + 2 * nh0], F32, False)
    w1e_t = nc.declare_dram_parameter("w1e", [P, kchunks, d1 + 2 * nh1], F32, False)
    iota_t = nc.declare_dram_parameter("iota", [P, P], F32, False)
    ident_t = nc.declare_dram_parameter("ident", [P, P], F32, False)
    src16_t = nc.declare_dram_parameter("src16", [P, totblk * 8], I16, False)
    er16_t = nc.declare_dram_parameter("er16", [P, totblk * 8], I16, False)
    dstloc_t = nc.declare_dram_parameter("dstloc", [P, totblk], F32, False)
    bia_t = nc.declare_dram_parameter("bia16", [P, npc_pad // 16], I16, False)
    bib_t = nc.declare_dram_parameter("bib16", [P, npc_pad // 16], I16, False)
    out_t = nc.declare_dram_parameter("out", [npc, d1], F32, True)

    tab0A = nc.dram_tensor("tab0A", [wrows, row0], F32)
    tab0B = nc.dram_tensor("tab0B", [wrows, row0], F32)
    tab1A = nc.dram_tensor("tab1A", [wrows, row1], F32)
    tab1B = nc.dram_tensor("tab1B", [wrows, row1], F32)
    erloc0 = nc.dram_tensor("erloc0", [npc_pad, 64], F32)
    erloc1 = nc.dram_tensor("erloc1", [npc_pad, 64], F32)
    h2slice = nc.dram_tensor("h2slice", [npc, d1 + 2 * nh1], F32)
    h2full = nc.dram_tensor("h2full", [NCORES, npc, d1 + 2 * nh1], F32,
                            addr_space="Shared")

    debug = os.environ.get("GAT_DEBUG", "0") == "1"
    phases = os.environ.get("GAT_PHASES", "full")
    if debug:
        dbg_t = {
            "tab0A": nc.declare_dram_parameter("dbg_tab0A", [wrows, row0], F32, True),
            "erloc0": nc.declare_dram_parameter("dbg_erloc0", [npc_pad, 64], F32, True),
            "h2s": nc.declare_dram_parameter("dbg_h2s", [npc, d1 + 2 * nh1], F32, True),
            "erloc1": nc.declare_dram_parameter("dbg_erloc1", [npc_pad, 64], F32, True),
        }

    nt_full = n_nodes // P
    rem = n_nodes - nt_full * P
    # phase-A window write ranges (node index ranges)
    wa_nodes = (0, wrows - 1)
    wb_nodes = (wb_base - 1, n_nodes)  # nodes wb_base-1 .. -> tabB rows i+1-wb_base

    with tile.TileContext(nc) as tc:
        with tc.tile_pool(name="singles", bufs=1) as singles:
            iota_sb = singles.tile([P, P], F32)
            nc.sync.dma_start(out=iota_sb[:], in_=iota_t.ap())
            ident_sb = singles.tile([P, P], F32)
            nc.sync.dma_start(out=ident_sb[:], in_=ident_t.ap())
            w0e_sb = singles.tile([P, d0 + 2 * nh0], F32)
            nc.sync.dma_start(out=w0e_sb[:], in_=w0e_t.ap())
            w1e_sb = singles.tile([P, kchunks, d1 + 2 * nh1], F32)
            nc.sync.dma_start(out=w1e_sb[:], in_=w1e_t.ap())
            src16_sb = singles.tile([P, totblk * 8], I16)
            nc.sync.dma_start(out=src16_sb[:], in_=src16_t.ap())
            er16_sb = singles.tile([P, totblk * 8], I16)
            nc.sync.dma_start(out=er16_sb[:], in_=er16_t.ap())
            dstloc_sb = singles.tile([P, totblk], F32)
            nc.sync.dma_start(out=dstloc_sb[:], in_=dstloc_t.ap())
            bia_sb = singles.tile([P, npc_pad // 16], I16)
            nc.sync.dma_start(out=bia_sb[:], in_=bia_t.ap())
            bib_sb = singles.tile([P, npc_pad // 16], I16)
            nc.sync.dma_start(out=bib_sb[:], in_=bib_t.ap())
            zrow = singles.tile([P, row0], F32)
            nc.vector.memset(zrow[:], 0.0)
            # zero guard rows
            nc.sync.dma_start(out=tab0A.ap()[0:1], in_=zrow[:1, :row0])
            nc.sync.dma_start(out=tab0B.ap()[wrows - 1:wrows], in_=zrow[:1, :row0])
            nc.sync.dma_start(out=tab1A.ap()[0:1], in_=zrow[:1, :row1])
            nc.sync.dma_start(out=tab1B.ap()[wrows - 1:wrows], in_=zrow[:1, :row1])

            # ---- Phase A: replicated dense layer 0 -> tab0A/tab0B ----
            with (tc.tile_pool(name="pa", bufs=2) as pa,
                  tc.tile_pool(name="pa_fts", bufs=3) as pa_fts,
                  tc.tile_pool(name="pa_ps", bufs=2, space="PSUM") as pa_ps,
                  tc.tile_pool(name="pa_ph", bufs=2, space="PSUM") as pa_ph):
                base = 0
                while base < n_nodes:
                    ch = min(CH, (n_nodes - base) // P)
                    partial = ch == 0
                    ch = max(ch, 1)
                    rows = rem if partial else ch * P
                    fchunk = pa.tile([P, CH, in_dim], F32, tag="fchunk",
                                     name="fchunk")
                    if partial:
                        nc.vector.memset(fchunk[:, 0, :], 0.0)
                        nc.sync.dma_start(out=fchunk[:rows, 0, :],
                                          in_=feat_t.ap()[base:base + rows])
                    else:
                        nc.sync.dma_start(
                            out=fchunk[:, :ch, :],
                            in_=feat_t.ap()[base:base + rows].rearrange(
                                "(i p) d -> p i d", p=P))
                    hstage = pa.tile([P, CH, row0], F32, tag="hstage",
                                     name="hstage")
                    if row0 > d0 + 2 * nh0:
                        nc.vector.memset(hstage[:, :, d0 + 2 * nh0:row0], 0.0)
                    for i in range(ch):
                        ftp = pa_ps.tile([P, P], F32, name="ftp")
                        nc.tensor.transpose(ftp[:], fchunk[:, i, :], ident_sb[:])
                        fts = pa_fts.tile([P, P], F32, name="fts")
                        nc.scalar.copy(out=fts[:], in_=ftp[:])
                        hps = pa_ph.tile([P, d0 + 2 * nh0], F32, name="hps")
                        nc.tensor.matmul(hps[:], lhsT=fts[:], rhs=w0e_sb[:],
                                         start=True, stop=True)
                        nc.scalar.copy(out=hstage[:, i, 0:d0 + 2 * nh0],
                                       in_=hps[:])
                    # write chunk rows [base, base+rows) to each window table
                    vw = row0
                    for (tab, lo_n, hi_n, roff) in (
                            (tab0A, wa_nodes[0], wa_nodes[1], 1),
                            (tab0B, wb_nodes[0], wb_nodes[1], 1 - wb_base)):
                        lo = max(base, lo_n)
                        hi = min(base + rows, hi_n)
                        if lo >= hi:
                            continue
                        if partial:
                            nc.sync.dma_start(
                                out=tab.ap()[lo + roff:hi + roff, 0:vw],
                                in_=hstage[lo - base:hi - base, 0, 0:vw])
                        elif lo == base and hi == base + rows:
                            nc.sync.dma_start(
                                out=tab.ap()[lo + roff:hi + roff, 0:vw].rearrange(
                                    "(i p) d -> p i d", p=P),
                                in_=hstage[:, :ch, 0:vw])
                        else:
                            for i in range(ch):
                                t0 = base + i * P
                                l2, h2 = max(lo, t0), min(hi, t0 + P)
                                if l2 >= h2:
                                    continue
                                nc.sync.dma_start(
                                    out=tab.ap()[l2 + roff:h2 + roff, 0:vw],
                                    in_=hstage[l2 - t0:h2 - t0, i, 0:vw])
                    base += rows

            # ---- shared pools for edge phases ----
            with (tc.tile_pool(name="hg", bufs=2) as hg_pool,
                  tc.tile_pool(name="ms", bufs=2) as ms_pool,
                  tc.tile_pool(name="mk", bufs=2) as mask_pool,
                  tc.tile_pool(name="sm", bufs=3) as small_pool,
                  tc.tile_pool(name="fin", bufs=2) as fin_pool,
                  tc.tile_pool(name="ps_acc", bufs=5, space="PSUM") as psum_pool,
                  tc.tile_pool(name="ps_tp", bufs=2, space="PSUM") as psum_tp,
                  tc.tile_pool(name="ps_h2", bufs=1, space="PSUM") as psum_h2):

                def build_erloc(tabA, tabB, erloc, width, col0):
                    nseg = npc_pad // P
                    with tc.tile_pool(name="ebld", bufs=1) as ebld:
                        EA = ebld.tile([P, nseg, 64], F32, tag="erga", name="ea")
                        nc.gpsimd.dma_gather(
                            out_ap=EA[:], in_ap=tabA.ap()[:, col0:col0 + 64],
                            idxs_ap=bia_sb[:], num_idxs=npc_pad,
                            num_idxs_reg=npc_pad, elem_size=64, elem_step=width,
                            single_packet=False, queue_num=0)
                        EB = ebld.tile([P, nseg, 64], F32, tag="ergb", name="eb")
                        nc.gpsimd.dma_gather(
                            out_ap=EB[:], in_ap=tabB.ap()[:, col0:col0 + 64],
                            idxs_ap=bib_sb[:], num_idxs=npc_pad,
                            num_idxs_reg=npc_pad, elem_size=64, elem_step=width,
                            single_packet=False, queue_num=1)
                        nc.vector.tensor_add(EA[:], EA[:], EB[:])
                        nc.sync.dma_start(
                            out=erloc.ap().rearrange("(i p) d -> p i d", p=P),
                            in_=EA[:])

                # erloc0: er at table cols [260:264] -> stored col 4+256-260...
                # gather window [row0-64, row0) covers cols 256:320; er is at
                # cols 260:264 -> offset 4 within the gathered 64
                if phases != "a":
                    build_erloc(tab0A, tab0B, erloc0, row0, row0 - 64)
                er_off0 = (d0 + nh0) - (row0 - 64)   # = 260-256 = 4

                def finalize0(t, acc):
                    rows = min(P, npc - t * P)
                    S = small_pool.tile([P, nh0], F32, tag="s0", name="s0")
                    nc.vector.tensor_scalar_max(S[:], acc[:, d0:d0 + nh0], 1e-30)
                    RC = small_pool.tile([P, nh0], F32, tag="rc0", name="rc0")
                    nc.vector.reciprocal(RC[:], S[:])
                    H1T = fin_pool.tile([P, d0], F32, tag="h1t", name="h1t")
                    nc.vector.tensor_tensor(
                        out=H1T[:].rearrange("p (h e) -> p h e", h=nh0),
                        in0=acc[:, 0:d0].rearrange("p (h e) -> p h e", h=nh0),
                        in1=_bcast_inner(RC[:], hid0),
                        op=mybir.AluOpType.mult)
                    h2ps = psum_h2.tile([P, d1 + 2 * nh1], F32, name="h2ps")
                    for k in range(kchunks):
                        tp = psum_tp.tile([P, P], F32, name="tp")
                        nc.tensor.transpose(tp[:], H1T[:, k * P:(k + 1) * P],
                                            ident_sb[:])
                        ts = fin_pool.tile([P, P], F32, tag="tsb", name="tsb")
                        nc.scalar.copy(out=ts[:], in_=tp[:])
                        nc.tensor.matmul(h2ps[:], lhsT=ts[:], rhs=w1e_sb[:, k, :],
                                         start=(k == 0), stop=(k == kchunks - 1))
                    h2sb = fin_pool.tile([P, d1 + 2 * nh1], F32, tag="h2sb",
                                         name="h2sb")
                    nc.scalar.copy(out=h2sb[:], in_=h2ps[:])
                    nc.sync.dma_start(out=h2slice.ap()[t * P:t * P + rows],
                                      in_=h2sb[:rows, :])

                if phases != "a":
                    _edge_phase(nc, tc,
                                (hg_pool, ms_pool, mask_pool, small_pool,
                                 psum_pool),
                                tab0A.ap(), tab0B.ap(), erloc0.ap(), row0,
                                er_off0, nh0, hid0, plan, src16_sb, er16_sb,
                                dstloc_sb, iota_sb, row0, finalize0)

                # ---- AllGather projected table, repack into window tables ----
                run_l1 = phases in ("full", "abc")
                if run_l1:
                    nc.gpsimd.collective_compute(
                    "AllGather", mybir.AluOpType.bypass,
                        replica_groups=[list(range(NCORES))],
                        ins=[h2slice.ap()], outs=[h2full.ap()])
                    h2flat = h2full.ap().rearrange("c n d -> (c n) d")
                    rw1 = d1 + 2 * nh1
                    na = min(wrows - 2, n_nodes - 1) + 1
                    nc.sync.dma_start(out=tab1A.ap()[1:1 + na, 0:rw1],
                                      in_=h2flat[0:na])
                    blo = max(wb_base - 1, 0)
                    nc.sync.dma_start(
                        out=tab1B.ap()[blo + 1 - wb_base:n_nodes + 1 - wb_base,
                                       0:rw1],
                        in_=h2flat[blo:n_nodes])
                    build_erloc(tab1A, tab1B, erloc1, row1, 0)
                er_off1 = d1 + nh1   # er-build window starts at col 0

                def finalize1(t, acc):  # noqa: indent-kept
                    rows = min(P, npc - t * P)
                    S = small_pool.tile([P, nh1], F32, tag="s1", name="s1")
                    nc.vector.tensor_scalar_max(S[:], acc[:, d1:d1 + nh1], 1e-30)
                    RC = small_pool.tile([P, nh1], F32, tag="rc1", name="rc1")
                    nc.vector.reciprocal(RC[:], S[:])
                    OUT = fin_pool.tile([P, d1], F32, tag="outt", name="outt")
                    nc.vector.tensor_scalar_mul(OUT[:], acc[:, 0:d1], RC[:, 0:1])
                    nc.sync.dma_start(out=out_t.ap()[t * P:t * P + rows],
                                      in_=OUT[:rows, :])

                if phases == "full":
                    _edge_phase(nc, tc,
                                (hg_pool, ms_pool, mask_pool, small_pool,
                                 psum_pool),
                                tab1A.ap(), tab1B.ap(), erloc1.ap(), row1,
                                er_off1, nh1, hid1, plan, src16_sb, er16_sb,
                                dstloc_sb, iota_sb, row1, finalize1)
                else:
                    ztile = fin_pool.tile([P, d1], F32, tag="outt", name="zout")
                    nc.vector.memset(ztile[:], 0.0)
                    for t in range(tpc_out):
                        rows = min(P, npc - t * P)
                        nc.sync.dma_start(out=out_t.ap()[t * P:t * P + rows],
                                          in_=ztile[:rows, :])

                if debug:
                    nc.sync.dma_start(out=dbg_t["tab0A"].ap(), in_=tab0A.ap())
                    nc.sync.dma_start(out=dbg_t["erloc0"].ap(), in_=erloc0.ap())
                    nc.sync.dma_start(out=dbg_t["h2s"].ap(), in_=h2slice.ap())
                    nc.sync.dma_start(out=dbg_t["erloc1"].ap(), in_=erloc1.ap())

    nc.compile()

    in_maps = []
    for c in range(NCORES):
        in_maps.append({
            "feat": np.ascontiguousarray(feat, dtype=np.float32),
            "w0e": w0e,
            "w1e": w1p,
            "iota": iota,
            "ident": ident,
            "src16": _wrap16(plan.srcw[c]),
            "er16": _wrap16(plan.erw[c]),
            "dstloc": np.ascontiguousarray(
                plan.dstlocv[c].reshape(totblk, P).T.astype(np.float32)),
            "bia16": np.ascontiguousarray(
                np.tile(bia[c].reshape(-1, 16).T.astype(np.int16), (8, 1))),
            "bib16": np.ascontiguousarray(
                np.tile(bib[c].reshape(-1, 16).T.astype(np.int16), (8, 1))),
        })
    LAST_BUILD[0] = (nc, in_maps)
    if simulate:
        from concourse import bass_interp
        sim = bass_interp.MultiCoreSim(nc, NCORES, ignore_data_errors=True)
        for c in range(NCORES):
            for k, v in in_maps[c].items():
                sim.cores[c].tensor(k)[:] = v
        sim.simulate()
        LAST_SIM[0] = sim
        out = np.concatenate(
            [np.array(sim.cores[c].tensor("out")) for c in range(NCORES)], axis=0)
        return out
    res = run_bass_kernel_spmd(nc, in_maps, list(range(NCORES)), trace=trace)
    LAST_RES[0] = res
    LAST_EXEC_NS[0] = res.exec_time_ns
    out = np.concatenate([res.results[c]["out"] for c in range(NCORES)], axis=0)
    return out


def kernel(feat, src, dst, W0, al0, ar0, W1, al1, ar1):
    trace = os.environ.get("GAT_TRACE", "0") == "1"
    out = build_and_run(np.asarray(feat), np.asarray(src), np.asarray(dst),
                        np.asarray(W0), np.asarray(al0), np.asarray(ar0),
                        np.asarray(W1), np.asarray(al1), np.asarray(ar1),
                        trace=trace)
    return out.astype(np.float32)



# revision 18
# speedup vs baseline: 1.5460x; 1.3434x over previous
"""2-layer GAT on 8 Trainium2 NeuronCores (Bass/Tile), v2.

Strategy (dst-sharded graph parallelism, SWDGE-aware):
  - The dominant cost on this hardware is GpSimd (Q7) descriptor generation
    for dma_gather (~8ns/index, serialized per SWDGE queue).  v2 therefore:
      * uses 4 SWDGE queues with round-robin assignment (desc-gen runs on a
        different Q7 cpu pair per queue),
      * eliminates the per-edge er gathers entirely: er[dst] is broadcast to
        edges with a tiny matmul  ERE[e,h] = sum_j maskT[j,e] * er_tile[j,h]
        where maskT is the PE-transposed 0/1 dst-selection mask,
      * eliminates the erloc window-gather pairs (layer-0 er comes from a
        packed er0p table via ONE gather + select-reduce; layer-1 er is
        written column-wise into er1locT during finalize0, no gather).
  - All node tables, masks and matmul operands are bf16 (4x faster PE than
    fp32, half the DMA bytes); PSUM accumulation stays fp32.
  - feat is pre-transposed on the host so layer-0's dense projection needs
    no PE transposes and no PSUM round-trip copies.
  - Edges sorted by dst, sharded by dst range (6250 nodes/core), blocks of
    128 on SBUF partitions; src rows fetched with int16 dma_gather through
    two overlapping 32768-row windows (int16 index limit).
  - Per edge block: one fp32-accumulating bf16 matmul adds both the
    weighted message sum and the softmax denominator into PSUM.
  - Between layers: project locally, AllGather the small bf16 [N,42] table,
    repack into window tables, run the same edge pipeline for layer 1.
"""

import os
import numpy as np
import ml_dtypes

import concourse.bass as bass
import concourse.bacc as bacc
import concourse.mybir as mybir
import concourse.tile as tile
from concourse.bass_utils import run_bass_kernel_spmd

F32 = mybir.dt.float32
BF16 = mybir.dt.bfloat16
I16 = mybir.dt.int16
BFNP = ml_dtypes.bfloat16

SLOPE = 0.2
NCORES = 8
P = 128
NQ = 4          # SWDGE queues
G = 16          # max edge blocks per gather chunk
ST = 4          # dst tiles per supertile (lo/hi run batching)
CH = 8          # node tiles per phase-A chunk
WROWS = 32768   # rows per index window
LAST_EXEC_NS = [None]
LAST_RES = [None]
LAST_SIM = [None]
LAST_BUILD = [None]


def _bcast_inner(apv, count):
    return bass.AP(tensor=apv.tensor, offset=apv.offset, ap=apv.ap + [[0, count]])


def _bcast_mid(apv, count):
    a = apv.ap
    return bass.AP(tensor=apv.tensor, offset=apv.offset, ap=[a[0], [0, count]] + a[1:])


def _fuse_w(W, al, ar):
    Fin = W.shape[0]
    H, D = al.shape
    Wr = W.reshape(Fin, H, D)
    wl = np.einsum("khd,hd->kh", Wr, al).astype(np.float32)
    wr = np.einsum("khd,hd->kh", Wr, ar).astype(np.float32)
    return np.ascontiguousarray(np.concatenate([W, wl, wr], axis=1), dtype=np.float32)


def _wrap16(idx):
    """int16 idx list (len multiple of 128) -> dma_gather SBUF layout
    [128, len/16]: idx j at [j % 16, j // 16], replicated across 8 groups."""
    w = idx.reshape(-1, 16).T.astype(np.int16)
    return np.ascontiguousarray(np.tile(w, (8, 1)))


def _prep_edges(src, dst, n_nodes, ncores, wrows):
    from types import SimpleNamespace
    plan = SimpleNamespace()
    npc = n_nodes // ncores
    tpc = (npc + P - 1) // P
    plan.npc, plan.tpc = npc, tpc
    wa_max = wrows - 2               # node i valid in A iff i+1 <= wrows-1
    plan.wb_base = n_nodes + 2 - wrows

    order = np.argsort(dst, kind="stable")
    ss = src[order].astype(np.int64)
    ds = dst[order].astype(np.int64)
    core = ds // npc
    loc = ds % npc
    tileid = loc // P
    hi = (ss > wa_max).astype(np.int64)

    counts = np.zeros((ncores, tpc, 2), np.int64)
    np.add.at(counts, (core, tileid, hi), 1)
    nblk = (counts + P - 1) // P
    bcnt = nblk.max(axis=0)
    if bcnt.sum() == 0:
        bcnt[0, 0] = 1
    plan.bcnt = bcnt

    plan.sts = [list(range(s, min(s + ST, tpc))) for s in range(0, tpc, ST)]
    plan.order_blocks = []
    for tiles in plan.sts:
        for w in (0, 1):
            for t in tiles:
                plan.order_blocks += [(t, w)] * int(bcnt[t, w])
    plan.totblk = len(plan.order_blocks)
    plan.nedge = plan.totblk * P

    slot = {}
    pos = 0
    for (t, w) in plan.order_blocks:
        if (t, w) not in slot:
            slot[(t, w)] = pos
        pos += P

    srcw = np.zeros((ncores, plan.nedge), np.int64)
    dstloc = np.full((ncores, plan.nedge), 999.0, np.float32)
    for bi, (t, w) in enumerate(plan.order_blocks):
        if w == 1:
            srcw[:, bi * P:(bi + 1) * P] = wrows - 1

    # order edges by (core, tile, win) groups
    gkey = (core * tpc + tileid) * 2 + hi
    g_order = np.argsort(gkey, kind="stable")
    ss2, loc2 = ss[g_order], loc[g_order]
    gstart = np.zeros(ncores * tpc * 2 + 1, np.int64)
    np.add.at(gstart[1:], gkey[g_order], 1)
    gstart = np.cumsum(gstart)
    for c in range(ncores):
        for t in range(tpc):
            for w in (0, 1):
                k = (c * tpc + t) * 2 + w
                e0, e1 = int(gstart[k]), int(gstart[k + 1])
                cnt = e1 - e0
                if cnt == 0:
                    continue
                off = slot[(t, w)]
                srcs = ss2[e0:e1]
                srcw[c, off:off + cnt] = (
                    srcs + 1 if w == 0 else srcs + 1 - plan.wb_base)
                dstloc[c, off:off + cnt] = (loc2[e0:e1] % P).astype(np.float32)

    plan.srcw, plan.dstlocv = srcw, dstloc
    return plan


def _edge_phase(nc, tc, pools, tabA_ap, tabB_ap, er_tile, d, nheads, hdim, gw,
                plan, src16_sb, dstloc_sb, iota_sb, ident_sb, finalize):
    """Edge pipeline for one layer.  Gathered bf16 row: [h(d) | el(nheads) |
    pad], gw elems.  er comes from er_tile [P, tpc, nheads] (bf16 SBUF) via
    maskT matmul broadcast."""
    (hg_pool, ms_pool, mask_pool, mt_pool, small_pool, eb_pool,
     psum_acc, psum_tp) = pools
    first_blk, last_blk = {}, {}
    for bi, (t, w) in enumerate(plan.order_blocks):
        if t not in first_blk:
            first_blk[t] = bi
        last_blk[t] = bi
    acc_by_tile = {}

    # chunks: maximal runs of <=G blocks within a single window
    chunks = []
    cur = None
    for bi, (t, w) in enumerate(plan.order_blocks):
        if cur is None or cur[0] != w or bi - cur[1] >= G:
            if cur is not None:
                chunks.append(cur)
            cur = [w, bi, bi + 1]
        else:
            cur[2] = bi + 1
        if cur[2] - cur[1] >= G:
            chunks.append(cur)
            cur = None
    if cur is not None:
        chunks.append(cur)

    for ci, (w, b0, b1) in enumerate(chunks):
        nb = b1 - b0
        nidx = nb * P
        HG = hg_pool.tile([P, G, gw], BF16, tag="hg", name="hg")
        nc.gpsimd.dma_gather(
            out_ap=HG[:, :nb, :], in_ap=(tabA_ap if w == 0 else tabB_ap),
            idxs_ap=src16_sb[:, b0 * 8:b1 * 8], num_idxs=nidx,
            num_idxs_reg=nidx, elem_size=gw, elem_step=gw,
            single_packet=False, queue_num=ci % NQ)
        # mask[e, b, j] = (dstloc[e, b] == j)   (bf16 0/1)
        MASK = mask_pool.tile([P, G, P], BF16, tag="mask", name="mask")
        nc.vector.tensor_tensor(
            out=MASK[:, :nb, :],
            in0=_bcast_inner(dstloc_sb[:, b0:b1], P),
            in1=_bcast_mid(iota_sb[:], nb),
            op=mybir.AluOpType.is_equal)
        # per-block: maskT (PE transpose), er broadcast matmul, E4 = el + er
        E4 = small_pool.tile([P, G, nheads], F32, tag="e4", name="e4")
        for bi in range(b0, b1):
            t, _ = plan.order_blocks[bi]
            TP = psum_tp.tile([P, P], BF16, tag="tp", name="tp")
            nc.tensor.transpose(TP[:], MASK[:, bi - b0, :], ident_sb[:])
            MT = mt_pool.tile([P, P], BF16, tag="mt", name="mt")
            nc.scalar.copy(out=MT[:], in_=TP[:])
            ERE = psum_tp.tile([P, nheads], F32, tag="tp", name="ere")
            nc.tensor.matmul(ERE[:], lhsT=MT[:], rhs=er_tile[:, t, :],
                             start=True, stop=True)
            nc.vector.tensor_add(E4[:, bi - b0, :], HG[:, bi - b0, d:d + nheads],
                                 ERE[:])
        # expe = exp(lrelu(E4))  (lrelu via DVE mul+max; Exp on scalar), bf16
        ESC = small_pool.tile([P, G, nheads], F32, tag="esc", name="esc")
        nc.vector.tensor_scalar_mul(ESC[:, :nb, :], E4[:, :nb, :], SLOPE)
        nc.vector.tensor_tensor(out=E4[:, :nb, :], in0=E4[:, :nb, :],
                                in1=ESC[:, :nb, :], op=mybir.AluOpType.max)
        EB = eb_pool.tile([P, G, nheads], BF16, tag="eb", name="eb")
        nc.scalar.activation(out=EB[:, :nb, :], in_=E4[:, :nb, :],
                             func=mybir.ActivationFunctionType.Exp)
        # MS = [expe-scaled h | expe]  (bf16)
        msw = d + nheads
        MS = ms_pool.tile([P, G, msw], BF16, tag="ms", name="ms")
        for h in range(nheads):
            nc.vector.tensor_tensor(
                out=MS[:, :nb, h * hdim:(h + 1) * hdim],
                in0=HG[:, :nb, h * hdim:(h + 1) * hdim],
                in1=_bcast_inner(EB[:, :nb, h:h + 1], hdim),
                op=mybir.AluOpType.mult)
        nc.scalar.copy(out=MS[:, :nb, d:d + nheads], in_=EB[:, :nb, :])
        for bi in range(b0, b1):
            t, _ = plan.order_blocks[bi]
            if bi == first_blk[t]:
                acc_by_tile[t] = psum_acc.tile([P, msw], F32, tag="acc",
                                               name="acc")
            acc = acc_by_tile[t]
            nc.tensor.matmul(acc[:], lhsT=MASK[:, bi - b0, :],
                             rhs=MS[:, bi - b0, :],
                             start=(bi == first_blk[t]),
                             stop=(bi == last_blk[t]))
            if bi == last_blk[t]:
                finalize(t, acc)
                del acc_by_tile[t]


def build_and_run(feat, src, dst, W0, al0, ar0, W1, al1, ar1, trace=False,
                  simulate=False):
    n_nodes = feat.shape[0]
    npc = n_nodes // NCORES
    nh0 = al0.shape[0]
    hid0 = al0.shape[1]
    d0 = nh0 * hid0                        # 256
    row0 = ((d0 + nh0 + 127) // 128) * 128  # 384 bf16 = 768B rows
    nh1 = al1.shape[0]
    hid1 = al1.shape[1]
    d1 = nh1 * hid1                        # 40
    row1 = ((d1 + 2 * nh1 + 127) // 128) * 128  # 128 bf16 = 256B rows
    in_dim = feat.shape[1]
    assert in_dim == P
    wrows = min(WROWS, n_nodes + 2)
    wb_base = n_nodes + 2 - wrows

    w0e = _fuse_w(W0, al0, ar0)            # [in_dim, d0+2nh0]
    w1e = _fuse_w(W1, al1, ar1)            # [d0, d1+2nh1]
    kchunks = d0 // P
    w1p = np.ascontiguousarray(
        w1e.reshape(kchunks, P, d1 + 2 * nh1).transpose(1, 0, 2)).astype(BFNP)
    featT = np.ascontiguousarray(feat.T).astype(BFNP)    # [128, N]

    plan = _prep_edges(src, dst, n_nodes, NCORES, wrows)
    totblk = plan.totblk
    tpc = plan.tpc
    npc_pad = tpc * P

    # layer-0 er gather: one idx per (tile t, partition j) -> er0p row
    # (16 nodes per 256B row); selection mask W picks the right 4 floats.
    er0p_rows = (n_nodes + 15) // 16 + 1
    eri = np.zeros((NCORES, npc_pad), np.int64)
    ersel = np.zeros((NCORES, P, 64), np.float32)
    for c in range(NCORES):
        g = c * npc + np.arange(npc_pad, dtype=np.int64)
        g = np.minimum(g, n_nodes - 1)
        eri[c] = g // 16
        sub = (c * npc + np.arange(P, dtype=np.int64)) % 16
        for j in range(P):
            ersel[c, j, 4 * sub[j]:4 * sub[j] + 4] = 1.0

    iota = np.broadcast_to(np.arange(P, dtype=np.float32), (P, P)).astype(BFNP)
    ident = np.eye(P, dtype=np.float32).astype(BFNP)

    nc = bacc.Bacc(None, target_bir_lowering=False, num_devices=NCORES,
                   num_swdge_queues=NQ)
    featT_t = nc.declare_dram_parameter("featT", [P, n_nodes], BF16, False)
    w0e_t = nc.declare_dram_parameter("w0e", [P, d0 + 2 * nh0], BF16, False)
    w1e_t = nc.declare_dram_parameter("w1e", [P, kchunks, d1 + 2 * nh1], BF16,
                                      False)
    iota_t = nc.declare_dram_parameter("iota", [P, P], BF16, False)
    ident_t = nc.declare_dram_parameter("ident", [P, P], BF16, False)
    src16_t = nc.declare_dram_parameter("src16", [P, totblk * 8], I16, False)
    dstloc_t = nc.declare_dram_parameter("dstloc", [P, totblk], BF16, False)
    eri16_t = nc.declare_dram_parameter("eri16", [P, npc_pad // 16], I16, False)
    ersel_t = nc.declare_dram_parameter("ersel", [P, 64], F32, False)
    out_t = nc.declare_dram_parameter("out", [npc, d1], F32, True)

    tab0A = nc.dram_tensor("tab0A", [wrows, row0], BF16)
    tab0B = nc.dram_tensor("tab0B", [wrows, row0], BF16)
    tab1A = nc.dram_tensor("tab1A", [wrows, row1], BF16)
    tab1B = nc.dram_tensor("tab1B", [wrows, row1], BF16)
    er0p = nc.dram_tensor("er0p", [er0p_rows, 64], F32)
    er1locT = nc.dram_tensor("er1locT", [P, tpc], BF16)
    h2slice = nc.dram_tensor("h2slice", [npc, d1 + 2 * nh1], BF16)
    h2full = nc.dram_tensor("h2full", [NCORES, npc, d1 + 2 * nh1], BF16,
                            addr_space="Shared")

    nt_full = n_nodes // P
    rem = n_nodes - nt_full * P
    wa_nodes = (0, wrows - 1)
    wb_nodes = (wb_base - 1, n_nodes)

    with tile.TileContext(nc) as tc:
        with tc.tile_pool(name="singles", bufs=1) as singles:
            iota_sb = singles.tile([P, P], BF16)
            nc.sync.dma_start(out=iota_sb[:], in_=iota_t.ap())
            ident_sb = singles.tile([P, P], BF16)
            nc.sync.dma_start(out=ident_sb[:], in_=ident_t.ap())
            w0e_sb = singles.tile([P, d0 + 2 * nh0], BF16)
            nc.sync.dma_start(out=w0e_sb[:], in_=w0e_t.ap())
            w1e_sb = singles.tile([P, kchunks, d1 + 2 * nh1], BF16)
            nc.sync.dma_start(out=w1e_sb[:], in_=w1e_t.ap())
            src16_sb = singles.tile([P, totblk * 8], I16)
            nc.sync.dma_start(out=src16_sb[:], in_=src16_t.ap())
            dstloc_sb = singles.tile([P, totblk], BF16)
            nc.sync.dma_start(out=dstloc_sb[:], in_=dstloc_t.ap())
            eri16_sb = singles.tile([P, npc_pad // 16], I16)
            nc.sync.dma_start(out=eri16_sb[:], in_=eri16_t.ap())
            ersel_sb = singles.tile([P, 64], F32)
            nc.sync.dma_start(out=ersel_sb[:], in_=ersel_t.ap())
            er0_tile = singles.tile([P, tpc, nh0], BF16)
            er1_tile = singles.tile([P, tpc, nh1], BF16)
            eps0 = singles.tile([P, nh0], F32)
            nc.vector.memset(eps0[:], 1e-30)
            eps1 = singles.tile([P, nh1], F32)
            nc.vector.memset(eps1[:], 1e-30)
            zrow = singles.tile([P, row0], BF16)
            nc.vector.memset(zrow[:], 0.0)
            # zero guard rows
            nc.sync.dma_start(out=tab0A.ap()[0:1], in_=zrow[:1, :row0])
            nc.sync.dma_start(out=tab0B.ap()[wrows - 1:wrows], in_=zrow[:1, :row0])
            nc.sync.dma_start(out=tab1A.ap()[0:1], in_=zrow[:1, :row1])
            nc.sync.dma_start(out=tab1B.ap()[wrows - 1:wrows], in_=zrow[:1, :row1])

            # ---- Phase A: replicated dense layer 0 -> tab0A/B + er0p ----
            with (tc.tile_pool(name="pa", bufs=3) as pa,
                  tc.tile_pool(name="pa_ph", bufs=3, space="PSUM") as pa_ph):
                base = 0
                chunk_i = 0
                while base < n_nodes:
                    ch = min(CH, (n_nodes - base) // P)
                    partial = ch == 0
                    ch = max(ch, 1)
                    rows = rem if partial else ch * P
                    ftc = pa.tile([P, CH * P], BF16, tag="ftc", name="ftc")
                    nc.sync.dma_start(out=ftc[:, :rows],
                                      in_=featT_t.ap()[:, base:base + rows])
                    hstage = pa.tile([P, CH, row0], BF16, tag="hstage",
                                     name="hstage")
                    if chunk_i < 3:  # pool bufs: pad cols stay zero on reuse
                        nc.vector.memset(hstage[:, :, d0 + nh0:row0], 0.0)
                    chunk_i += 1
                    erst = pa.tile([P, CH, nh0], F32, tag="erst", name="erst")
                    for i in range(ch):
                        m = rows - i * P if partial else P
                        hps = pa_ph.tile([P, d0 + 2 * nh0], F32, name="hps")
                        nc.tensor.matmul(hps[:m, :], lhsT=ftc[:, i * P:i * P + m],
                                         rhs=w0e_sb[:], start=True, stop=True)
                        nc.scalar.copy(out=hstage[:m, i, 0:d0 + nh0],
                                       in_=hps[:m, 0:d0 + nh0])
                        nc.scalar.copy(out=erst[:m, i, :],
                                       in_=hps[:m, d0 + nh0:d0 + 2 * nh0])
                    # write chunk rows to each window table
                    for (tab, lo_n, hi_n, roff) in (
                            (tab0A, wa_nodes[0], wa_nodes[1], 1),
                            (tab0B, wb_nodes[0], wb_nodes[1], 1 - wb_base)):
                        lo = max(base, lo_n)
                        hi = min(base + rows, hi_n)
                        if lo >= hi:
                            continue
                        if partial:
                            nc.sync.dma_start(
                                out=tab.ap()[lo + roff:hi + roff, :],
                                in_=hstage[lo - base:hi - base, 0, :])
                        elif lo == base and hi == base + rows:
                            nc.sync.dma_start(
                                out=tab.ap()[lo + roff:hi + roff, :].rearrange(
                                    "(i p) d -> p i d", p=P),
                                in_=hstage[:, :ch, :])
                        else:
                            for i in range(ch):
                                t0 = base + i * P
                                l2, h2 = max(lo, t0), min(hi, t0 + P)
                                if l2 >= h2:
                                    continue
                                nc.sync.dma_start(
                                    out=tab.ap()[l2 + roff:h2 + roff, :],
                                    in_=hstage[l2 - t0:h2 - t0, i, :])
                    # write er columns to packed er0p (node-major fp32)
                    r0 = base // 16
                    if partial:
                        nc.sync.dma_start(
                            out=er0p.ap()[r0:r0 + rows // 16, :].rearrange(
                                "a (p d) -> (a p) d", p=16),
                            in_=erst[:rows, 0, :])
                    else:
                        nc.sync.dma_start(
                            out=er0p.ap()[r0:r0 + rows // 16, :].rearrange(
                                "(i r8) (p16 d) -> (r8 p16) i d",
                                i=ch, p16=16),
                            in_=erst[:, :ch, :])
                    base += rows

            # ---- er0_tile: one gather + select-reduce ----
            with tc.tile_pool(name="ebld", bufs=1) as ebld:
                ERAW = ebld.tile([P, tpc, 64], F32, tag="eraw", name="eraw")
                nc.gpsimd.dma_gather(
                    out_ap=ERAW[:], in_ap=er0p.ap(),
                    idxs_ap=eri16_sb[:], num_idxs=npc_pad,
                    num_idxs_reg=npc_pad, elem_size=64, elem_step=64,
                    single_packet=False, queue_num=1)
                EMUL = ebld.tile([P, tpc, 64], F32, tag="emul", name="emul")
                nc.vector.tensor_tensor(out=EMUL[:], in0=ERAW[:],
                                        in1=_bcast_mid(ersel_sb[:], tpc),
                                        op=mybir.AluOpType.mult)
                ERED = ebld.tile([P, tpc, nh0], F32, tag="ered", name="ered")
                nc.vector.tensor_reduce(
                    out=ERED[:],
                    in_=EMUL[:].rearrange("p t (s h) -> p t h s", h=nh0),
                    axis=mybir.AxisListType.X, op=mybir.AluOpType.add)
                nc.scalar.copy(out=er0_tile[:], in_=ERED[:])

            # ---- shared pools for edge phases ----
            with (tc.tile_pool(name="hg", bufs=4) as hg_pool,
                  tc.tile_pool(name="ms", bufs=3) as ms_pool,
                  tc.tile_pool(name="mk", bufs=3) as mask_pool,
                  tc.tile_pool(name="mt", bufs=4) as mt_pool,
                  tc.tile_pool(name="sm", bufs=3) as small_pool,
                  tc.tile_pool(name="eb", bufs=3) as eb_pool,
                  tc.tile_pool(name="fin", bufs=2) as fin_pool,
                  tc.tile_pool(name="ps_acc", bufs=5, space="PSUM") as psum_acc,
                  tc.tile_pool(name="ps_tp", bufs=2, space="PSUM") as psum_tp,
                  tc.tile_pool(name="ps_h2", bufs=1, space="PSUM") as psum_h2):

                def finalize0(t, acc):
                    rows = min(P, npc - t * P)
                    S = small_pool.tile([P, nh0], F32, tag="s0", name="s0")
                    nc.vector.tensor_tensor(out=S[:], in0=acc[:, d0:d0 + nh0],
                                            in1=eps0[:],
                                            op=mybir.AluOpType.max)
                    RC = small_pool.tile([P, nh0], F32, tag="rc0", name="rc0")
                    nc.vector.reciprocal(RC[:], S[:])
                    H1T = fin_pool.tile([P, d0], BF16, tag="h1t", name="h1t")
                    nc.vector.tensor_tensor(
                        out=H1T[:].rearrange("p (h e) -> p h e", h=nh0),
                        in0=acc[:, 0:d0].rearrange("p (h e) -> p h e", h=nh0),
                        in1=_bcast_inner(RC[:], hid0),
                        op=mybir.AluOpType.mult)
                    h2ps = psum_h2.tile([P, d1 + 2 * nh1], F32, name="h2ps")
                    for k in range(kchunks):
                        tp = psum_tp.tile([P, P], BF16, tag="tp", name="ftp")
                        nc.tensor.transpose(tp[:], H1T[:, k * P:(k + 1) * P],
                                            ident_sb[:])
                        ts = fin_pool.tile([P, P], BF16, tag="tsb", name="tsb")
                        nc.scalar.copy(out=ts[:], in_=tp[:])
                        nc.tensor.matmul(h2ps[:], lhsT=ts[:], rhs=w1e_sb[:, k, :],
                                         start=(k == 0), stop=(k == kchunks - 1))
                    h2sb = fin_pool.tile([P, d1 + 2 * nh1], BF16, tag="h2sb",
                                         name="h2sb")
                    nc.scalar.copy(out=h2sb[:], in_=h2ps[:])
                    nc.sync.dma_start(out=h2slice.ap()[t * P:t * P + rows],
                                      in_=h2sb[:rows, :])
                    nc.sync.dma_start(out=er1locT.ap()[:, t:t + 1],
                                      in_=h2sb[:, d1 + nh1:d1 + 2 * nh1])

                _edge_phase(nc, tc,
                            (hg_pool, ms_pool, mask_pool, mt_pool, small_pool,
                             eb_pool, psum_acc, psum_tp),
                            tab0A.ap(), tab0B.ap(), er0_tile, d0, nh0, hid0,
                            row0, plan, src16_sb, dstloc_sb, iota_sb, ident_sb,
                            finalize0)

                # ---- AllGather projected table, repack into window tables ----
                nc.gpsimd.collective_compute(
                    "AllGather", mybir.AluOpType.bypass,
                    replica_groups=[list(range(NCORES))],
                    ins=[h2slice.ap()], outs=[h2full.ap()])
                h2flat = h2full.ap().rearrange("c n d -> (c n) d")
                rw1 = d1 + 2 * nh1
                na = min(wrows - 2, n_nodes - 1) + 1
                nc.sync.dma_start(out=tab1A.ap()[1:1 + na, 0:rw1],
                                  in_=h2flat[0:na])
                blo = max(wb_base - 1, 0)
                nc.sync.dma_start(
                    out=tab1B.ap()[blo + 1 - wb_base:n_nodes + 1 - wb_base,
                                   0:rw1],
                    in_=h2flat[blo:n_nodes])
                nc.sync.dma_start(
                    out=er1_tile[:, :, 0],
                    in_=er1locT.ap())

                def finalize1(t, acc):
                    rows = min(P, npc - t * P)
                    S = small_pool.tile([P, nh1], F32, tag="s1", name="s1")
                    nc.vector.tensor_tensor(out=S[:], in0=acc[:, d1:d1 + nh1],
                                            in1=eps1[:],
                                            op=mybir.AluOpType.max)
                    RC = small_pool.tile([P, nh1], F32, tag="rc1", name="rc1")
                    nc.vector.reciprocal(RC[:], S[:])
                    OUT = fin_pool.tile([P, d1], F32, tag="outt", name="outt")
                    nc.vector.tensor_tensor(out=OUT[:], in0=acc[:, 0:d1],
                                            in1=_bcast_inner(RC[:], d1),
                                            op=mybir.AluOpType.mult)
                    nc.sync.dma_start(out=out_t.ap()[t * P:t * P + rows],
                                      in_=OUT[:rows, :])

                _edge_phase(nc, tc,
                            (hg_pool, ms_pool, mask_pool, mt_pool, small_pool,
                             eb_pool, psum_acc, psum_tp),
                            tab1A.ap(), tab1B.ap(), er1_tile, d1, nh1, hid1,
                            row1, plan, src16_sb, dstloc_sb, iota_sb, ident_sb,
                            finalize1)

    nc.compile()
    if os.environ.get("GAT_COMPILE_ONLY", "0") == "1":
        LAST_BUILD[0] = (nc, None)
        return np.zeros((n_nodes, d1), np.float32)

    in_maps = []
    for c in range(NCORES):
        in_maps.append({
            "featT": featT,
            "w0e": np.ascontiguousarray(w0e).astype(BFNP),
            "w1e": w1p,
            "iota": np.ascontiguousarray(iota),
            "ident": np.ascontiguousarray(ident),
            "src16": _wrap16(plan.srcw[c]),
            "dstloc": np.ascontiguousarray(
                plan.dstlocv[c].reshape(totblk, P).T).astype(BFNP),
            "eri16": _wrap16(eri[c]),
            "ersel": np.ascontiguousarray(ersel[c]),
        })
    LAST_BUILD[0] = (nc, in_maps)
    if simulate:
        from concourse import bass_interp
        sim = bass_interp.MultiCoreSim(nc, NCORES, ignore_data_errors=True)
        for c in range(NCORES):
            for k, v in in_maps[c].items():
                sim.cores[c].tensor(k)[:] = v
        sim.simulate()
        LAST_SIM[0] = sim
        out = np.concatenate(
            [np.array(sim.cores[c].tensor("out")) for c in range(NCORES)], axis=0)
        return out
    res = run_bass_kernel_spmd(nc, in_maps, list(range(NCORES)), trace=trace)
    LAST_RES[0] = res
    LAST_EXEC_NS[0] = res.exec_time_ns
    out = np.concatenate([res.results[c]["out"] for c in range(NCORES)], axis=0)
    return out


def kernel(feat, src, dst, W0, al0, ar0, W1, al1, ar1):
    trace = os.environ.get("GAT_TRACE", "0") == "1"
    out = build_and_run(np.asarray(feat), np.asarray(src), np.asarray(dst),
                        np.asarray(W0), np.asarray(al0), np.asarray(ar0),
                        np.asarray(W1), np.asarray(al1), np.asarray(ar1),
                        trace=trace)
    return out.astype(np.float32)


# revision 30
# speedup vs baseline: 2.0045x; 1.2966x over previous
"""2-layer GAT on 8 Trainium2 NeuronCores (Bass/Tile), v2.

Strategy (dst-sharded graph parallelism, SWDGE-aware):
  - The dominant cost on this hardware is GpSimd (Q7) descriptor generation
    for dma_gather (~8ns/index, serialized per SWDGE queue).  v2 therefore:
      * uses 4 SWDGE queues with round-robin assignment (desc-gen runs on a
        different Q7 cpu pair per queue),
      * eliminates the per-edge er gathers entirely: er[dst] is broadcast to
        edges with a tiny matmul  ERE[e,h] = sum_j maskT[j,e] * er_tile[j,h]
        where maskT is the PE-transposed 0/1 dst-selection mask,
      * eliminates the erloc window-gather pairs (layer-0 er comes from a
        packed er0p table via ONE gather + select-reduce; layer-1 er is
        written column-wise into er1locT during finalize0, no gather).
  - All node tables, masks and matmul operands are bf16 (4x faster PE than
    fp32, half the DMA bytes); PSUM accumulation stays fp32.
  - feat is pre-transposed on the host so layer-0's dense projection needs
    no PE transposes and no PSUM round-trip copies.
  - Edges sorted by dst, sharded by dst range (6250 nodes/core), blocks of
    128 on SBUF partitions; src rows fetched with int16 dma_gather through
    two overlapping 32768-row windows (int16 index limit).
  - Per edge block: one fp32-accumulating bf16 matmul adds both the
    weighted message sum and the softmax denominator into PSUM.
  - Between layers: project locally, AllGather the small bf16 [N,42] table,
    repack into window tables, run the same edge pipeline for layer 1.
"""

import os
import numpy as np
import ml_dtypes

import concourse.bass as bass
import concourse.bacc as bacc
import concourse.mybir as mybir
import concourse.tile as tile
from concourse.bass_utils import run_bass_kernel_spmd

F32 = mybir.dt.float32
BF16 = mybir.dt.bfloat16
I16 = mybir.dt.int16
BFNP = ml_dtypes.bfloat16

SLOPE = 0.2
NCORES = 8
P = 128
NQ = 4          # SWDGE queues
G = 16          # max edge blocks per gather chunk
ST = 4          # dst tiles per supertile (lo/hi run batching)
CH = 8          # node tiles per phase-A chunk
WROWS = 32768   # rows per index window
LAST_EXEC_NS = [None]
LAST_RES = [None]
LAST_SIM = [None]
LAST_BUILD = [None]


def _bcast_inner(apv, count):
    return bass.AP(tensor=apv.tensor, offset=apv.offset, ap=apv.ap + [[0, count]])


def _bcast_mid(apv, count):
    a = apv.ap
    return bass.AP(tensor=apv.tensor, offset=apv.offset, ap=[a[0], [0, count]] + a[1:])


def _fuse_w(W, al, ar):
    Fin = W.shape[0]
    H, D = al.shape
    Wr = W.reshape(Fin, H, D)
    wl = np.einsum("khd,hd->kh", Wr, al).astype(np.float32)
    wr = np.einsum("khd,hd->kh", Wr, ar).astype(np.float32)
    return np.ascontiguousarray(np.concatenate([W, wl, wr], axis=1), dtype=np.float32)


def _wrap16(idx):
    """int16 idx list (len multiple of 128) -> dma_gather SBUF layout
    [128, len/16]: idx j at [j % 16, j // 16], replicated across 8 groups."""
    w = idx.reshape(-1, 16).T.astype(np.int16)
    return np.ascontiguousarray(np.tile(w, (8, 1)))


def _prep_edges(src, dst, n_nodes, ncores, wrows):
    from types import SimpleNamespace
    plan = SimpleNamespace()
    npc = n_nodes // ncores
    tpc = (npc + P - 1) // P
    plan.npc, plan.tpc = npc, tpc
    wa_max = wrows - 2               # node i valid in A iff i+1 <= wrows-1
    plan.wb_base = n_nodes + 2 - wrows

    order = np.argsort(dst, kind="stable")
    ss = src[order].astype(np.int64)
    ds = dst[order].astype(np.int64)
    core = ds // npc
    loc = ds % npc
    tileid = loc // P
    hi = (ss > wa_max).astype(np.int64)

    counts = np.zeros((ncores, tpc, 2), np.int64)
    np.add.at(counts, (core, tileid, hi), 1)
    nblk = (counts + P - 1) // P
    bcnt = nblk.max(axis=0)
    if bcnt.sum() == 0:
        bcnt[0, 0] = 1
    plan.bcnt = bcnt

    plan.sts = [list(range(s, min(s + ST, tpc))) for s in range(0, tpc, ST)]
    plan.order_blocks = []
    for tiles in plan.sts:
        for w in (0, 1):
            for t in tiles:
                plan.order_blocks += [(t, w)] * int(bcnt[t, w])
    plan.totblk = len(plan.order_blocks)
    plan.nedge = plan.totblk * P

    slot = {}
    pos = 0
    for (t, w) in plan.order_blocks:
        if (t, w) not in slot:
            slot[(t, w)] = pos
        pos += P

    srcw = np.zeros((ncores, plan.nedge), np.int64)
    dstloc = np.full((ncores, plan.nedge), 999.0, np.float32)
    for bi, (t, w) in enumerate(plan.order_blocks):
        if w == 1:
            srcw[:, bi * P:(bi + 1) * P] = wrows - 1

    # order edges by (core, tile, win) groups
    gkey = (core * tpc + tileid) * 2 + hi
    g_order = np.argsort(gkey, kind="stable")
    ss2, loc2 = ss[g_order], loc[g_order]
    gstart = np.zeros(ncores * tpc * 2 + 1, np.int64)
    np.add.at(gstart[1:], gkey[g_order], 1)
    gstart = np.cumsum(gstart)
    for c in range(ncores):
        for t in range(tpc):
            for w in (0, 1):
                k = (c * tpc + t) * 2 + w
                e0, e1 = int(gstart[k]), int(gstart[k + 1])
                cnt = e1 - e0
                if cnt == 0:
                    continue
                off = slot[(t, w)]
                srcs = ss2[e0:e1]
                srcw[c, off:off + cnt] = (
                    srcs + 1 if w == 0 else srcs + 1 - plan.wb_base)
                dstloc[c, off:off + cnt] = (loc2[e0:e1] % P).astype(np.float32)

    plan.srcw, plan.dstlocv = srcw, dstloc
    return plan


def _edge_phase(nc, tc, pools, tabA_ap, tabB_ap, er_tile, d, nheads, hdim, gw,
                plan, src16_sb, dstloc_sb, iota_sb, ident_sb, slope_sb,
                finalize, tile_done=None):
    """Edge pipeline for one layer.  Gathered bf16 row: [h(d) | el(nheads) |
    pad], gw elems.  er comes from er_tile [P, tpc, nheads] (bf16 SBUF) via
    maskT matmul broadcast.  tile_done(t) is called after finalize(t)."""
    (hg_pool, ms_pool, mask_pool, mt_pool, small_pool, eb_pool,
     psum_acc, psum_tp, psum_er) = pools
    first_blk, last_blk = {}, {}
    for bi, (t, w) in enumerate(plan.order_blocks):
        if t not in first_blk:
            first_blk[t] = bi
        last_blk[t] = bi
    acc_by_tile = {}

    # chunks: maximal runs of <=G blocks within a single window
    chunks = []
    cur = None
    for bi, (t, w) in enumerate(plan.order_blocks):
        if cur is None or cur[0] != w or bi - cur[1] >= G:
            if cur is not None:
                chunks.append(cur)
            cur = [w, bi, bi + 1]
        else:
            cur[2] = bi + 1
        if cur[2] - cur[1] >= G:
            chunks.append(cur)
            cur = None
    if cur is not None:
        chunks.append(cur)

    for ci, (w, b0, b1) in enumerate(chunks):
        nb = b1 - b0
        nidx = nb * P
        HG = hg_pool.tile([P, G, gw], BF16, tag="hg", name="hg")
        nc.gpsimd.dma_gather(
            out_ap=HG[:, :nb, :], in_ap=(tabA_ap if w == 0 else tabB_ap),
            idxs_ap=src16_sb[:, b0 * 8:b1 * 8], num_idxs=nidx,
            num_idxs_reg=nidx, elem_size=gw, elem_step=gw,
            single_packet=False, queue_num=ci % NQ)
        # mask[e, b, j] = (dstloc[e, b] == j)   (bf16 0/1)
        MASK = mask_pool.tile([P, G, P], BF16, tag="mask", name="mask")
        nc.vector.tensor_tensor(
            out=MASK[:, :nb, :],
            in0=_bcast_inner(dstloc_sb[:, b0:b1], P),
            in1=_bcast_mid(iota_sb[:], nb),
            op=mybir.AluOpType.is_equal)
        # per-block: maskT (PE transpose) + er broadcast matmul into one
        # chunk-wide PSUM strip; then a single E4 = el + er add.
        EREC = psum_er.tile([P, G, nheads], F32, tag="erec", name="erec")
        for bi in range(b0, b1):
            t, _ = plan.order_blocks[bi]
            TP = psum_tp.tile([P, P], BF16, tag="tp", name="tp")
            nc.tensor.transpose(TP[:], MASK[:, bi - b0, :], ident_sb[:])
            MT = mt_pool.tile([P, P], BF16, tag="mt", name="mt")
            nc.scalar.copy(out=MT[:], in_=TP[:])
            nc.tensor.matmul(EREC[:, bi - b0, :], lhsT=MT[:],
                             rhs=er_tile[:, t, :], start=True, stop=True)
        E4 = small_pool.tile([P, G, nheads], F32, tag="e4", name="e4")
        nc.vector.tensor_add(E4[:, :nb, :], HG[:, :nb, d:d + nheads],
                             EREC[:, :nb, :])
        # expe = exp(lrelu(E4))  (lrelu via DVE mul+max; Exp on scalar), bf16
        ESC = small_pool.tile([P, G, nheads], F32, tag="esc", name="esc")
        nc.vector.tensor_tensor(out=ESC[:, :nb, :], in0=E4[:, :nb, :],
                                in1=_bcast_mid(slope_sb[:, 0:nheads], nb),
                                op=mybir.AluOpType.mult)
        nc.vector.tensor_tensor(out=E4[:, :nb, :], in0=E4[:, :nb, :],
                                in1=ESC[:, :nb, :], op=mybir.AluOpType.max)
        EB = eb_pool.tile([P, G, nheads], BF16, tag="eb", name="eb")
        nc.scalar.activation(out=EB[:, :nb, :], in_=E4[:, :nb, :],
                             func=mybir.ActivationFunctionType.Exp)
        # MS = [expe-scaled h | expe]  (bf16)
        msw = d + nheads
        MS = ms_pool.tile([P, G, msw], BF16, tag="ms", name="ms")
        for h in range(nheads):
            nc.vector.tensor_tensor(
                out=MS[:, :nb, h * hdim:(h + 1) * hdim],
                in0=HG[:, :nb, h * hdim:(h + 1) * hdim],
                in1=_bcast_inner(EB[:, :nb, h:h + 1], hdim),
                op=mybir.AluOpType.mult)
        nc.scalar.copy(out=MS[:, :nb, d:d + nheads], in_=EB[:, :nb, :])
        for bi in range(b0, b1):
            t, _ = plan.order_blocks[bi]
            if bi == first_blk[t]:
                acc_by_tile[t] = psum_acc.tile([P, msw], F32, tag="acc",
                                               name="acc")
            acc = acc_by_tile[t]
            nc.tensor.matmul(acc[:], lhsT=MASK[:, bi - b0, :],
                             rhs=MS[:, bi - b0, :],
                             start=(bi == first_blk[t]),
                             stop=(bi == last_blk[t]))
            if bi == last_blk[t]:
                finalize(t, acc)
                del acc_by_tile[t]
                if tile_done is not None:
                    tile_done(t)


def build_and_run(feat, src, dst, W0, al0, ar0, W1, al1, ar1, trace=False,
                  simulate=False):
    n_nodes = feat.shape[0]
    npc = n_nodes // NCORES
    nh0 = al0.shape[0]
    hid0 = al0.shape[1]
    d0 = nh0 * hid0                        # 256
    row0 = ((d0 + nh0 + 127) // 128) * 128  # 384 bf16 = 768B rows
    nh1 = al1.shape[0]
    hid1 = al1.shape[1]
    d1 = nh1 * hid1                        # 40
    row1 = ((d1 + 2 * nh1 + 127) // 128) * 128  # 128 bf16 = 256B rows
    in_dim = feat.shape[1]
    assert in_dim == P
    wrows = min(WROWS, n_nodes + 2)
    wb_base = n_nodes + 2 - wrows

    w0e = _fuse_w(W0, al0, ar0)            # [in_dim, d0+2nh0]
    w1e = _fuse_w(W1, al1, ar1)            # [d0, d1+2nh1]
    kchunks = d0 // P
    w1p = np.ascontiguousarray(
        w1e.reshape(kchunks, P, d1 + 2 * nh1).transpose(1, 0, 2)).astype(BFNP)
    featT = np.ascontiguousarray(feat.T).astype(BFNP)    # [128, N]

    plan = _prep_edges(src, dst, n_nodes, NCORES, wrows)
    totblk = plan.totblk
    tpc = plan.tpc
    npc_pad = tpc * P

    # layer-0 er gather: one idx per (tile t, partition j) -> er0p row
    # (16 nodes per 256B row); selection mask W picks the right 4 floats.
    er0p_rows = (n_nodes + 15) // 16 + 1
    eri = np.zeros((NCORES, npc_pad), np.int64)
    ersel = np.zeros((NCORES, P, 64), np.float32)
    for c in range(NCORES):
        g = c * npc + np.arange(npc_pad, dtype=np.int64)
        g = np.minimum(g, n_nodes - 1)
        eri[c] = g // 16
        sub = (c * npc + np.arange(P, dtype=np.int64)) % 16
        for j in range(P):
            ersel[c, j, 4 * sub[j]:4 * sub[j] + 4] = 1.0

    iota = np.broadcast_to(np.arange(P, dtype=np.float32), (P, P)).astype(BFNP)
    ident = np.eye(P, dtype=np.float32).astype(BFNP)

    nc = bacc.Bacc(None, target_bir_lowering=False, num_devices=NCORES,
                   num_swdge_queues=NQ)
    featT_t = nc.declare_dram_parameter("featT", [P, n_nodes], BF16, False)
    w0e_t = nc.declare_dram_parameter("w0e", [P, d0 + 2 * nh0], BF16, False)
    w1e_t = nc.declare_dram_parameter("w1e", [P, kchunks, d1 + 2 * nh1], BF16,
                                      False)
    iota_t = nc.declare_dram_parameter("iota", [P, P], BF16, False)
    ident_t = nc.declare_dram_parameter("ident", [P, P], BF16, False)
    src16_t = nc.declare_dram_parameter("src16", [P, totblk * 8], I16, False)
    dstloc_t = nc.declare_dram_parameter("dstloc", [P, totblk], BF16, False)
    eri16_t = nc.declare_dram_parameter("eri16", [P, npc_pad // 16], I16, False)
    ersel_t = nc.declare_dram_parameter("ersel", [P, 64], F32, False)
    out_t = nc.declare_dram_parameter("out", [npc, d1], F32, True)

    # single tables; window A = rows [0, wrows), window B = rows
    # [wb_base, wb_base+wrows) of the same tensor (node i lives at row i+1)
    tab0 = nc.dram_tensor("tab0", [n_nodes + 2, row0], BF16)
    tab1 = nc.dram_tensor("tab1", [n_nodes + 2, row1], BF16)
    er0p = nc.dram_tensor("er0p", [er0p_rows, 64], F32)
    er1locT = nc.dram_tensor("er1locT", [P, tpc], BF16)
    h2slice = nc.dram_tensor("h2slice", [npc, d1 + 2 * nh1], BF16)
    NGRP = 4
    gsz = (tpc + NGRP - 1) // NGRP
    grp_bounds = []
    for g in range(NGRP):
        tlo, thi = g * gsz, min((g + 1) * gsz, tpc)
        if tlo < thi:
            grp_bounds.append((tlo * P, min(thi * P, npc), thi - 1))
    h2fullg = [
        nc.dram_tensor(f"h2full{g}", [NCORES, hi - lo, d1 + 2 * nh1], BF16,
                       addr_space="Shared")
        for g, (lo, hi, _) in enumerate(grp_bounds)]

    nt_full = n_nodes // P
    rem = n_nodes - nt_full * P
    wa_nodes = (0, wrows - 1)
    wb_nodes = (wb_base - 1, n_nodes)

    with tile.TileContext(nc) as tc:
        with tc.tile_pool(name="singles", bufs=1) as singles:
            iota_sb = singles.tile([P, P], BF16)
            nc.sync.dma_start(out=iota_sb[:], in_=iota_t.ap())
            ident_sb = singles.tile([P, P], BF16)
            nc.sync.dma_start(out=ident_sb[:], in_=ident_t.ap())
            w0e_sb = singles.tile([P, d0 + 2 * nh0], BF16)
            nc.sync.dma_start(out=w0e_sb[:], in_=w0e_t.ap())
            w1e_sb = singles.tile([P, kchunks, d1 + 2 * nh1], BF16)
            nc.sync.dma_start(out=w1e_sb[:], in_=w1e_t.ap())
            src16_sb = singles.tile([P, totblk * 8], I16)
            nc.sync.dma_start(out=src16_sb[:], in_=src16_t.ap())
            dstloc_sb = singles.tile([P, totblk], BF16)
            nc.sync.dma_start(out=dstloc_sb[:], in_=dstloc_t.ap())
            eri16_sb = singles.tile([P, npc_pad // 16], I16)
            nc.sync.dma_start(out=eri16_sb[:], in_=eri16_t.ap())
            ersel_sb = singles.tile([P, 64], F32)
            nc.sync.dma_start(out=ersel_sb[:], in_=ersel_t.ap())
            er0_tile = singles.tile([P, tpc, nh0], BF16)
            er1_tile = singles.tile([P, tpc, nh1], BF16)
            eps0 = singles.tile([P, nh0], F32)
            nc.vector.memset(eps0[:], 1e-30)
            eps1 = singles.tile([P, nh1], F32)
            nc.vector.memset(eps1[:], 1e-30)
            slope_sb = singles.tile([P, nh0], F32)
            nc.vector.memset(slope_sb[:], SLOPE)
            zrow = singles.tile([P, row0], BF16)
            nc.vector.memset(zrow[:], 0.0)
            # zero guard rows (row 0 and row n_nodes+1 of each table)
            nc.sync.dma_start(out=tab0.ap()[0:1], in_=zrow[:1, :row0])
            nc.sync.dma_start(out=tab0.ap()[n_nodes + 1:n_nodes + 2],
                              in_=zrow[:1, :row0])
            nc.sync.dma_start(out=tab1.ap()[0:1], in_=zrow[:1, :row1])
            nc.sync.dma_start(out=tab1.ap()[n_nodes + 1:n_nodes + 2],
                              in_=zrow[:1, :row1])

            # ---- Phase A: replicated dense layer 0 -> tab0A/B + er0p ----
            with (tc.tile_pool(name="pa", bufs=3) as pa,
                  tc.tile_pool(name="pa_ph", bufs=3, space="PSUM") as pa_ph):
                base = 0
                chunk_i = 0
                while base < n_nodes:
                    ch = min(CH, (n_nodes - base) // P)
                    partial = ch == 0
                    ch = max(ch, 1)
                    rows = rem if partial else ch * P
                    ftc = pa.tile([P, CH * P], BF16, tag="ftc", name="ftc")
                    nc.sync.dma_start(out=ftc[:, :rows],
                                      in_=featT_t.ap()[:, base:base + rows])
                    hstage = pa.tile([P, CH, row0], BF16, tag="hstage",
                                     name="hstage")
                    if chunk_i < 3:  # pool bufs: pad cols stay zero on reuse
                        nc.vector.memset(hstage[:, :, d0 + nh0:row0], 0.0)
                    chunk_i += 1
                    erst = pa.tile([P, CH, nh0], F32, tag="erst", name="erst")
                    for i in range(ch):
                        m = rows - i * P if partial else P
                        hps = pa_ph.tile([P, d0 + 2 * nh0], F32, name="hps")
                        nc.tensor.matmul(hps[:m, :], lhsT=ftc[:, i * P:i * P + m],
                                         rhs=w0e_sb[:], start=True, stop=True)
                        nc.scalar.copy(out=hstage[:m, i, 0:d0 + nh0],
                                       in_=hps[:m, 0:d0 + nh0])
                        nc.scalar.copy(out=erst[:m, i, :],
                                       in_=hps[:m, d0 + nh0:d0 + 2 * nh0])
                    # write chunk rows once into the single table
                    weng = nc.sync if (chunk_i % 2 == 0) else nc.scalar
                    if partial:
                        weng.dma_start(
                            out=tab0.ap()[base + 1:base + rows + 1, :],
                            in_=hstage[:rows, 0, :])
                    else:
                        weng.dma_start(
                            out=tab0.ap()[base + 1:base + rows + 1, :].rearrange(
                                "(i p) d -> p i d", p=P),
                            in_=hstage[:, :ch, :])
                    # write er columns to packed er0p (node-major fp32)
                    r0 = base // 16
                    if partial:
                        nc.sync.dma_start(
                            out=er0p.ap()[r0:r0 + rows // 16, :].rearrange(
                                "a (p d) -> (a p) d", p=16),
                            in_=erst[:rows, 0, :])
                    else:
                        nc.sync.dma_start(
                            out=er0p.ap()[r0:r0 + rows // 16, :].rearrange(
                                "(i r8) (p16 d) -> (r8 p16) i d",
                                i=ch, p16=16),
                            in_=erst[:, :ch, :])
                    base += rows

            # ---- er0_tile: one gather + select-reduce ----
            with tc.tile_pool(name="ebld", bufs=1) as ebld:
                ERAW = ebld.tile([P, tpc, 64], F32, tag="eraw", name="eraw")
                nc.gpsimd.dma_gather(
                    out_ap=ERAW[:], in_ap=er0p.ap(),
                    idxs_ap=eri16_sb[:], num_idxs=npc_pad,
                    num_idxs_reg=npc_pad, elem_size=64, elem_step=64,
                    single_packet=False, queue_num=1)
                EMUL = ebld.tile([P, tpc, 64], F32, tag="emul", name="emul")
                nc.vector.tensor_tensor(out=EMUL[:], in0=ERAW[:],
                                        in1=_bcast_mid(ersel_sb[:], tpc),
                                        op=mybir.AluOpType.mult)
                ERED = ebld.tile([P, tpc, nh0], F32, tag="ered", name="ered")
                nc.vector.tensor_reduce(
                    out=ERED[:],
                    in_=EMUL[:].rearrange("p t (s h) -> p t h s", h=nh0),
                    axis=mybir.AxisListType.X, op=mybir.AluOpType.add)
                nc.scalar.copy(out=er0_tile[:], in_=ERED[:])

            # ---- shared pools for edge phases ----
            with (tc.tile_pool(name="hg", bufs=4) as hg_pool,
                  tc.tile_pool(name="ms", bufs=3) as ms_pool,
                  tc.tile_pool(name="mk", bufs=3) as mask_pool,
                  tc.tile_pool(name="mt", bufs=4) as mt_pool,
                  tc.tile_pool(name="sm", bufs=3) as small_pool,
                  tc.tile_pool(name="eb", bufs=3) as eb_pool,
                  tc.tile_pool(name="fin", bufs=2) as fin_pool,
                  tc.tile_pool(name="ps_acc", bufs=5, space="PSUM") as psum_acc,
                  tc.tile_pool(name="ps_tp", bufs=1, space="PSUM") as psum_tp,
                  tc.tile_pool(name="ps_er", bufs=1, space="PSUM") as psum_er,
                  tc.tile_pool(name="ps_h2", bufs=1, space="PSUM") as psum_h2):

                def finalize0(t, acc):
                    rows = min(P, npc - t * P)
                    S = small_pool.tile([P, nh0], F32, tag="s0", name="s0")
                    nc.vector.tensor_tensor(out=S[:], in0=acc[:, d0:d0 + nh0],
                                            in1=eps0[:],
                                            op=mybir.AluOpType.max)
                    RC = small_pool.tile([P, nh0], F32, tag="rc0", name="rc0")
                    nc.vector.reciprocal(RC[:], S[:])
                    H1T = fin_pool.tile([P, d0], BF16, tag="h1t", name="h1t")
                    nc.vector.tensor_tensor(
                        out=H1T[:].rearrange("p (h e) -> p h e", h=nh0),
                        in0=acc[:, 0:d0].rearrange("p (h e) -> p h e", h=nh0),
                        in1=_bcast_inner(RC[:], hid0),
                        op=mybir.AluOpType.mult)
                    h2ps = psum_h2.tile([P, d1 + 2 * nh1], F32, name="h2ps")
                    for k in range(kchunks):
                        tp = psum_tp.tile([P, P], BF16, tag="tp", name="ftp")
                        nc.tensor.transpose(tp[:], H1T[:, k * P:(k + 1) * P],
                                            ident_sb[:])
                        ts = fin_pool.tile([P, P], BF16, tag="tsb", name="tsb")
                        nc.scalar.copy(out=ts[:], in_=tp[:])
                        nc.tensor.matmul(h2ps[:], lhsT=ts[:], rhs=w1e_sb[:, k, :],
                                         start=(k == 0), stop=(k == kchunks - 1))
                    h2sb = fin_pool.tile([P, d1 + 2 * nh1], BF16, tag="h2sb",
                                         name="h2sb")
                    nc.scalar.copy(out=h2sb[:], in_=h2ps[:])
                    nc.sync.dma_start(out=h2slice.ap()[t * P:t * P + rows],
                                      in_=h2sb[:rows, :])
                    nc.sync.dma_start(out=er1locT.ap()[:, t:t + 1],
                                      in_=h2sb[:, d1 + nh1:d1 + 2 * nh1])

                # chunked AllGather: after the last tile of each tile-group
                # finishes, gather that row range and repack it into tab1,
                # overlapping with the tail of the layer-0 edge phase.
                rw1 = d1 + 2 * nh1
                grp_last = {last_t: g for g, (_, _, last_t) in
                            enumerate(grp_bounds)}
                tab1_t = tab1.ap().tensor

                def tile_done0(t):
                    if t not in grp_last:
                        return
                    g = grp_last[t]
                    lo, hi, _ = grp_bounds[g]
                    nc.gpsimd.collective_compute(
                        "AllGather", mybir.AluOpType.bypass,
                        replica_groups=[list(range(NCORES))],
                        ins=[h2slice.ap()[lo:hi]],
                        outs=[h2fullg[g].ap()])
                    # tab1 row for node (c, l) is 1 + c*npc + l
                    out_ap = bass.AP(
                        tensor=tab1_t, offset=(1 + lo) * row1,
                        ap=[[npc * row1, NCORES], [row1, hi - lo], [1, rw1]])
                    nc.sync.dma_start(out=out_ap, in_=h2fullg[g].ap())

                _edge_phase(nc, tc,
                            (hg_pool, ms_pool, mask_pool, mt_pool, small_pool,
                             eb_pool, psum_acc, psum_tp, psum_er),
                            tab0.ap()[0:wrows], tab0.ap()[wb_base:wb_base + wrows],
                            er0_tile, d0, nh0, hid0,
                            row0, plan, src16_sb, dstloc_sb, iota_sb, ident_sb,
                            slope_sb, finalize0, tile_done=tile_done0)

                nc.sync.dma_start(
                    out=er1_tile[:, :, 0],
                    in_=er1locT.ap())

                def finalize1(t, acc):
                    rows = min(P, npc - t * P)
                    S = small_pool.tile([P, nh1], F32, tag="s1", name="s1")
                    nc.vector.tensor_tensor(out=S[:], in0=acc[:, d1:d1 + nh1],
                                            in1=eps1[:],
                                            op=mybir.AluOpType.max)
                    RC = small_pool.tile([P, nh1], F32, tag="rc1", name="rc1")
                    nc.vector.reciprocal(RC[:], S[:])
                    OUT = fin_pool.tile([P, d1], F32, tag="outt", name="outt")
                    nc.vector.tensor_tensor(out=OUT[:], in0=acc[:, 0:d1],
                                            in1=_bcast_inner(RC[:], d1),
                                            op=mybir.AluOpType.mult)
                    nc.sync.dma_start(out=out_t.ap()[t * P:t * P + rows],
                                      in_=OUT[:rows, :])

                _edge_phase(nc, tc,
                            (hg_pool, ms_pool, mask_pool, mt_pool, small_pool,
                             eb_pool, psum_acc, psum_tp, psum_er),
                            tab1.ap()[0:wrows], tab1.ap()[wb_base:wb_base + wrows],
                            er1_tile, d1, nh1, hid1,
                            row1, plan, src16_sb, dstloc_sb, iota_sb, ident_sb,
                            slope_sb, finalize1)

    nc.compile()
    if os.environ.get("GAT_COMPILE_ONLY", "0") == "1":
        LAST_BUILD[0] = (nc, None)
        return np.zeros((n_nodes, d1), np.float32)

    in_maps = []
    for c in range(NCORES):
        in_maps.append({
            "featT": featT,
            "w0e": np.ascontiguousarray(w0e).astype(BFNP),
            "w1e": w1p,
            "iota": np.ascontiguousarray(iota),
            "ident": np.ascontiguousarray(ident),
            "src16": _wrap16(plan.srcw[c]),
            "dstloc": np.ascontiguousarray(
                plan.dstlocv[c].reshape(totblk, P).T).astype(BFNP),
            "eri16": _wrap16(eri[c]),
            "ersel": np.ascontiguousarray(ersel[c]),
        })
    LAST_BUILD[0] = (nc, in_maps)
    if simulate:
        from concourse import bass_interp
        sim = bass_interp.MultiCoreSim(nc, NCORES, ignore_data_errors=True)
        for c in range(NCORES):
            for k, v in in_maps[c].items():
                sim.cores[c].tensor(k)[:] = v
        sim.simulate()
        LAST_SIM[0] = sim
        out = np.concatenate(
            [np.array(sim.cores[c].tensor("out")) for c in range(NCORES)], axis=0)
        return out
    res = run_bass_kernel_spmd(nc, in_maps, list(range(NCORES)), trace=trace)
    LAST_RES[0] = res
    LAST_EXEC_NS[0] = res.exec_time_ns
    out = np.concatenate([res.results[c]["out"] for c in range(NCORES)], axis=0)
    return out


def kernel(feat, src, dst, W0, al0, ar0, W1, al1, ar1):
    trace = os.environ.get("GAT_TRACE", "0") == "1"
    out = build_and_run(np.asarray(feat), np.asarray(src), np.asarray(dst),
                        np.asarray(W0), np.asarray(al0), np.asarray(ar0),
                        np.asarray(W1), np.asarray(al1), np.asarray(ar1),
                        trace=trace)
    return out.astype(np.float32)


# revision 34
# speedup vs baseline: 2.0168x; 1.0061x over previous
"""2-layer GAT on 8 Trainium2 NeuronCores (Bass/Tile), v2.

Strategy (dst-sharded graph parallelism, SWDGE-aware):
  - The dominant cost on this hardware is GpSimd (Q7) descriptor generation
    for dma_gather (~8ns/index, serialized per SWDGE queue).  v2 therefore:
      * uses 4 SWDGE queues with round-robin assignment (desc-gen runs on a
        different Q7 cpu pair per queue),
      * eliminates the per-edge er gathers entirely: er[dst] is broadcast to
        edges with a tiny matmul  ERE[e,h] = sum_j maskT[j,e] * er_tile[j,h]
        where maskT is the PE-transposed 0/1 dst-selection mask,
      * eliminates the erloc window-gather pairs (layer-0 er comes from a
        packed er0p table via ONE gather + select-reduce; layer-1 er is
        written column-wise into er1locT during finalize0, no gather).
  - All node tables, masks and matmul operands are bf16 (4x faster PE than
    fp32, half the DMA bytes); PSUM accumulation stays fp32.
  - feat is pre-transposed on the host so layer-0's dense projection needs
    no PE transposes and no PSUM round-trip copies.
  - Edges sorted by dst, sharded by dst range (6250 nodes/core), blocks of
    128 on SBUF partitions; src rows fetched with int16 dma_gather through
    two overlapping 32768-row windows (int16 index limit).
  - Per edge block: one fp32-accumulating bf16 matmul adds both the
    weighted message sum and the softmax denominator into PSUM.
  - Between layers: project locally, AllGather the small bf16 [N,42] table,
    repack into window tables, run the same edge pipeline for layer 1.
"""

import os
import numpy as np
import ml_dtypes

import concourse.bass as bass
import concourse.bacc as bacc
import concourse.mybir as mybir
import concourse.tile as tile
from concourse.bass_utils import run_bass_kernel_spmd

F32 = mybir.dt.float32
BF16 = mybir.dt.bfloat16
I16 = mybir.dt.int16
BFNP = ml_dtypes.bfloat16

SLOPE = 0.2
NCORES = 8
P = 128
NQ = 4          # SWDGE queues
G = 16          # max edge blocks per gather chunk
ST = 4          # dst tiles per supertile (lo/hi run batching)
CH = 8          # node tiles per phase-A chunk
WROWS = 32768   # rows per index window
LAST_EXEC_NS = [None]
LAST_RES = [None]
LAST_SIM = [None]
LAST_BUILD = [None]


def _bcast_inner(apv, count):
    return bass.AP(tensor=apv.tensor, offset=apv.offset, ap=apv.ap + [[0, count]])


def _bcast_mid(apv, count):
    a = apv.ap
    return bass.AP(tensor=apv.tensor, offset=apv.offset, ap=[a[0], [0, count]] + a[1:])


def _fuse_w(W, al, ar):
    Fin = W.shape[0]
    H, D = al.shape
    Wr = W.reshape(Fin, H, D)
    wl = np.einsum("khd,hd->kh", Wr, al).astype(np.float32)
    wr = np.einsum("khd,hd->kh", Wr, ar).astype(np.float32)
    return np.ascontiguousarray(np.concatenate([W, wl, wr], axis=1), dtype=np.float32)


def _wrap16(idx):
    """int16 idx list (len multiple of 128) -> dma_gather SBUF layout
    [128, len/16]: idx j at [j % 16, j // 16], replicated across 8 groups."""
    w = idx.reshape(-1, 16).T.astype(np.int16)
    return np.ascontiguousarray(np.tile(w, (8, 1)))


def _prep_edges(src, dst, n_nodes, ncores, wrows):
    from types import SimpleNamespace
    plan = SimpleNamespace()
    npc = n_nodes // ncores
    tpc = (npc + P - 1) // P
    plan.npc, plan.tpc = npc, tpc
    wa_max = wrows - 2               # node i valid in A iff i+1 <= wrows-1
    plan.wb_base = n_nodes + 2 - wrows

    order = np.argsort(dst, kind="stable")
    ss = src[order].astype(np.int64)
    ds = dst[order].astype(np.int64)
    core = ds // npc
    loc = ds % npc
    tileid = loc // P
    hi = (ss > wa_max).astype(np.int64)

    counts = np.zeros((ncores, tpc, 2), np.int64)
    np.add.at(counts, (core, tileid, hi), 1)
    nblk = (counts + P - 1) // P
    bcnt = nblk.max(axis=0)
    if bcnt.sum() == 0:
        bcnt[0, 0] = 1
    plan.bcnt = bcnt

    plan.sts = [list(range(s, min(s + ST, tpc))) for s in range(0, tpc, ST)]
    plan.order_blocks = []
    for tiles in plan.sts:
        for w in (0, 1):
            for t in tiles:
                plan.order_blocks += [(t, w)] * int(bcnt[t, w])
    plan.totblk = len(plan.order_blocks)
    plan.nedge = plan.totblk * P

    slot = {}
    pos = 0
    for (t, w) in plan.order_blocks:
        if (t, w) not in slot:
            slot[(t, w)] = pos
        pos += P

    srcw = np.zeros((ncores, plan.nedge), np.int64)
    dstloc = np.full((ncores, plan.nedge), 999.0, np.float32)
    for bi, (t, w) in enumerate(plan.order_blocks):
        if w == 1:
            srcw[:, bi * P:(bi + 1) * P] = wrows - 1

    # order edges by (core, tile, win) groups
    gkey = (core * tpc + tileid) * 2 + hi
    g_order = np.argsort(gkey, kind="stable")
    ss2, loc2 = ss[g_order], loc[g_order]
    gstart = np.zeros(ncores * tpc * 2 + 1, np.int64)
    np.add.at(gstart[1:], gkey[g_order], 1)
    gstart = np.cumsum(gstart)
    for c in range(ncores):
        for t in range(tpc):
            for w in (0, 1):
                k = (c * tpc + t) * 2 + w
                e0, e1 = int(gstart[k]), int(gstart[k + 1])
                cnt = e1 - e0
                if cnt == 0:
                    continue
                off = slot[(t, w)]
                srcs = ss2[e0:e1]
                srcw[c, off:off + cnt] = (
                    srcs + 1 if w == 0 else srcs + 1 - plan.wb_base)
                dstloc[c, off:off + cnt] = (loc2[e0:e1] % P).astype(np.float32)

    plan.srcw, plan.dstlocv = srcw, dstloc
    return plan


def _edge_phase(nc, tc, pools, tabA_ap, tabB_ap, er_tile, d, nheads, hdim, gw,
                plan, src16_sb, dstloc_sb, iota_sb, ident_sb, slope_sb,
                finalize, tile_done=None):
    """Edge pipeline for one layer.  Gathered bf16 row: [h(d) | el(nheads) |
    pad], gw elems.  er comes from er_tile [P, tpc, nheads] (bf16 SBUF) via
    maskT matmul broadcast.  tile_done(t) is called after finalize(t)."""
    (hg_pool, ms_pool, mask_pool, mt_pool, small_pool, eb_pool,
     psum_acc, psum_tp, psum_er) = pools
    first_blk, last_blk = {}, {}
    for bi, (t, w) in enumerate(plan.order_blocks):
        if t not in first_blk:
            first_blk[t] = bi
        last_blk[t] = bi
    acc_by_tile = {}

    # chunks: maximal runs of <=G blocks within a single window
    chunks = []
    cur = None
    for bi, (t, w) in enumerate(plan.order_blocks):
        if cur is None or cur[0] != w or bi - cur[1] >= G:
            if cur is not None:
                chunks.append(cur)
            cur = [w, bi, bi + 1]
        else:
            cur[2] = bi + 1
        if cur[2] - cur[1] >= G:
            chunks.append(cur)
            cur = None
    if cur is not None:
        chunks.append(cur)

    for ci, (w, b0, b1) in enumerate(chunks):
        nb = b1 - b0
        nidx = nb * P
        HG = hg_pool.tile([P, G, gw], BF16, tag="hg", name="hg")
        nc.gpsimd.dma_gather(
            out_ap=HG[:, :nb, :], in_ap=(tabA_ap if w == 0 else tabB_ap),
            idxs_ap=src16_sb[:, b0 * 8:b1 * 8], num_idxs=nidx,
            num_idxs_reg=nidx, elem_size=gw, elem_step=gw,
            single_packet=False, queue_num=ci % NQ)
        # mask[e, b, j] = (dstloc[e, b] == j)   (bf16 0/1)
        MASK = mask_pool.tile([P, G, P], BF16, tag="mask", name="mask")
        nc.vector.tensor_tensor(
            out=MASK[:, :nb, :],
            in0=_bcast_inner(dstloc_sb[:, b0:b1], P),
            in1=_bcast_mid(iota_sb[:], nb),
            op=mybir.AluOpType.is_equal)
        # per-block: maskT (PE transpose) + er broadcast matmul into one
        # chunk-wide PSUM strip; then a single E4 = el + er add.
        EREC = psum_er.tile([P, G, nheads], F32, tag="erec", name="erec")
        for bi in range(b0, b1):
            t, _ = plan.order_blocks[bi]
            TP = psum_tp.tile([P, P], BF16, tag="tp", name="tp")
            nc.tensor.transpose(TP[:], MASK[:, bi - b0, :], ident_sb[:])
            MT = mt_pool.tile([P, P], BF16, tag="mt", name="mt")
            nc.scalar.copy(out=MT[:], in_=TP[:])
            nc.tensor.matmul(EREC[:, bi - b0, :], lhsT=MT[:],
                             rhs=er_tile[:, t, :], start=True, stop=True)
        E4 = small_pool.tile([P, G, nheads], F32, tag="e4", name="e4")
        nc.vector.tensor_add(E4[:, :nb, :], HG[:, :nb, d:d + nheads],
                             EREC[:, :nb, :])
        # expe = exp(lrelu(E4))  (lrelu via DVE mul+max; Exp on scalar), bf16
        ESC = small_pool.tile([P, G, nheads], F32, tag="esc", name="esc")
        nc.vector.tensor_tensor(out=ESC[:, :nb, :], in0=E4[:, :nb, :],
                                in1=_bcast_mid(slope_sb[:, 0:nheads], nb),
                                op=mybir.AluOpType.mult)
        nc.vector.tensor_tensor(out=E4[:, :nb, :], in0=E4[:, :nb, :],
                                in1=ESC[:, :nb, :], op=mybir.AluOpType.max)
        EB = eb_pool.tile([P, G, nheads], BF16, tag="eb", name="eb")
        nc.scalar.activation(out=EB[:, :nb, :], in_=E4[:, :nb, :],
                             func=mybir.ActivationFunctionType.Exp)
        # MS = [expe-scaled h | expe]  (bf16)
        msw = d + nheads
        MS = ms_pool.tile([P, G, msw], BF16, tag="ms", name="ms")
        for h in range(nheads):
            nc.vector.tensor_tensor(
                out=MS[:, :nb, h * hdim:(h + 1) * hdim],
                in0=HG[:, :nb, h * hdim:(h + 1) * hdim],
                in1=_bcast_inner(EB[:, :nb, h:h + 1], hdim),
                op=mybir.AluOpType.mult)
        nc.scalar.copy(out=MS[:, :nb, d:d + nheads], in_=EB[:, :nb, :])
        for bi in range(b0, b1):
            t, _ = plan.order_blocks[bi]
            if bi == first_blk[t]:
                acc_by_tile[t] = psum_acc.tile([P, msw], F32, tag="acc",
                                               name="acc")
            acc = acc_by_tile[t]
            nc.tensor.matmul(acc[:], lhsT=MASK[:, bi - b0, :],
                             rhs=MS[:, bi - b0, :],
                             start=(bi == first_blk[t]),
                             stop=(bi == last_blk[t]))
            if bi == last_blk[t]:
                finalize(t, acc)
                del acc_by_tile[t]
                if tile_done is not None:
                    tile_done(t)


def build_and_run(feat, src, dst, W0, al0, ar0, W1, al1, ar1, trace=False,
                  simulate=False):
    n_nodes = feat.shape[0]
    npc = n_nodes // NCORES
    nh0 = al0.shape[0]
    hid0 = al0.shape[1]
    d0 = nh0 * hid0                        # 256
    row0 = ((d0 + nh0 + 127) // 128) * 128  # 384 bf16 = 768B rows
    nh1 = al1.shape[0]
    hid1 = al1.shape[1]
    d1 = nh1 * hid1                        # 40
    row1 = ((d1 + 2 * nh1 + 127) // 128) * 128  # 128 bf16 = 256B rows
    in_dim = feat.shape[1]
    assert in_dim == P
    wrows = min(WROWS, n_nodes + 2)
    wb_base = n_nodes + 2 - wrows

    w0e = _fuse_w(W0, al0, ar0)            # [in_dim, d0+2nh0]
    w1e = _fuse_w(W1, al1, ar1)            # [d0, d1+2nh1]
    kchunks = d0 // P
    w1p = np.ascontiguousarray(
        w1e.reshape(kchunks, P, d1 + 2 * nh1).transpose(1, 0, 2)).astype(BFNP)
    featT = np.ascontiguousarray(feat.T).astype(BFNP)    # [128, N]

    plan = _prep_edges(src, dst, n_nodes, NCORES, wrows)
    totblk = plan.totblk
    tpc = plan.tpc
    npc_pad = tpc * P

    # layer-0 er gather: one idx per (tile t, partition j) -> er0p row
    # (16 nodes per 256B row); selection mask W picks the right 4 floats.
    er0p_rows = (n_nodes + 15) // 16 + 1
    eri = np.zeros((NCORES, npc_pad), np.int64)
    ersel = np.zeros((NCORES, P, 64), np.float32)
    for c in range(NCORES):
        g = c * npc + np.arange(npc_pad, dtype=np.int64)
        g = np.minimum(g, n_nodes - 1)
        eri[c] = g // 16
        sub = (c * npc + np.arange(P, dtype=np.int64)) % 16
        for j in range(P):
            ersel[c, j, 4 * sub[j]:4 * sub[j] + 4] = 1.0

    iota = np.broadcast_to(np.arange(P, dtype=np.float32), (P, P)).astype(BFNP)
    ident = np.eye(P, dtype=np.float32).astype(BFNP)

    nc = bacc.Bacc(None, target_bir_lowering=False, num_devices=NCORES,
                   num_swdge_queues=NQ)
    featT_t = nc.declare_dram_parameter("featT", [P, n_nodes], BF16, False)
    w0e_t = nc.declare_dram_parameter("w0e", [P, d0 + 2 * nh0], BF16, False)
    w1e_t = nc.declare_dram_parameter("w1e", [P, kchunks, d1 + 2 * nh1], BF16,
                                      False)
    iota_t = nc.declare_dram_parameter("iota", [P, P], BF16, False)
    ident_t = nc.declare_dram_parameter("ident", [P, P], BF16, False)
    src16_t = nc.declare_dram_parameter("src16", [P, totblk * 8], I16, False)
    dstloc_t = nc.declare_dram_parameter("dstloc", [P, totblk], BF16, False)
    eri16_t = nc.declare_dram_parameter("eri16", [P, npc_pad // 16], I16, False)
    ersel_t = nc.declare_dram_parameter("ersel", [P, 64], F32, False)
    out_t = nc.declare_dram_parameter("out", [npc, d1], F32, True)

    # single tables; window A = rows [0, wrows), window B = rows
    # [wb_base, wb_base+wrows) of the same tensor (node i lives at row i+1)
    tab0 = nc.dram_tensor("tab0", [n_nodes + 2, row0], BF16)
    tab1 = nc.dram_tensor("tab1", [n_nodes + 2, row1], BF16)
    er0p = nc.dram_tensor("er0p", [er0p_rows, 64], F32)
    er1locT = nc.dram_tensor("er1locT", [P, tpc], BF16)
    h2slice = nc.dram_tensor("h2slice", [npc, d1 + 2 * nh1], BF16)
    NGRP = 4
    gsz = (tpc + NGRP - 1) // NGRP
    grp_bounds = []
    for g in range(NGRP):
        tlo, thi = g * gsz, min((g + 1) * gsz, tpc)
        if tlo < thi:
            grp_bounds.append((tlo * P, min(thi * P, npc), thi - 1))
    h2fullg = [
        nc.dram_tensor(f"h2full{g}", [NCORES, hi - lo, d1 + 2 * nh1], BF16,
                       addr_space="Shared")
        for g, (lo, hi, _) in enumerate(grp_bounds)]

    nt_full = n_nodes // P
    rem = n_nodes - nt_full * P
    wa_nodes = (0, wrows - 1)
    wb_nodes = (wb_base - 1, n_nodes)

    with tile.TileContext(nc) as tc:
        with tc.tile_pool(name="singles", bufs=1) as singles:
            iota_sb = singles.tile([P, P], BF16)
            nc.sync.dma_start(out=iota_sb[:], in_=iota_t.ap())
            ident_sb = singles.tile([P, P], BF16)
            nc.sync.dma_start(out=ident_sb[:], in_=ident_t.ap())
            w0e_sb = singles.tile([P, d0 + 2 * nh0], BF16)
            nc.sync.dma_start(out=w0e_sb[:], in_=w0e_t.ap())
            w1e_sb = singles.tile([P, kchunks, d1 + 2 * nh1], BF16)
            nc.sync.dma_start(out=w1e_sb[:], in_=w1e_t.ap())
            src16_sb = singles.tile([P, totblk * 8], I16)
            nc.sync.dma_start(out=src16_sb[:], in_=src16_t.ap())
            dstloc_sb = singles.tile([P, totblk], BF16)
            nc.sync.dma_start(out=dstloc_sb[:], in_=dstloc_t.ap())
            eri16_sb = singles.tile([P, npc_pad // 16], I16)
            nc.sync.dma_start(out=eri16_sb[:], in_=eri16_t.ap())
            ersel_sb = singles.tile([P, 64], F32)
            nc.sync.dma_start(out=ersel_sb[:], in_=ersel_t.ap())
            er0_tile = singles.tile([P, tpc, nh0], BF16)
            er1_tile = singles.tile([P, tpc, nh1], BF16)
            eps0 = singles.tile([P, nh0], F32)
            nc.vector.memset(eps0[:], 1e-30)
            eps1 = singles.tile([P, nh1], F32)
            nc.vector.memset(eps1[:], 1e-30)
            slope_sb = singles.tile([P, nh0], F32)
            nc.vector.memset(slope_sb[:], SLOPE)
            zrow = singles.tile([P, row0], BF16)
            nc.vector.memset(zrow[:], 0.0)
            # zero guard rows (row 0 and row n_nodes+1 of each table)
            nc.sync.dma_start(out=tab0.ap()[0:1], in_=zrow[:1, :row0])
            nc.sync.dma_start(out=tab0.ap()[n_nodes + 1:n_nodes + 2],
                              in_=zrow[:1, :row0])
            nc.sync.dma_start(out=tab1.ap()[0:1], in_=zrow[:1, :row1])
            nc.sync.dma_start(out=tab1.ap()[n_nodes + 1:n_nodes + 2],
                              in_=zrow[:1, :row1])

            # ---- Phase A0: er-only pre-pass -> er0p (so the er0_tile gather
            # can run concurrently with the main dense pass) ----
            with (tc.tile_pool(name="pe0", bufs=3) as pe0,
                  tc.tile_pool(name="pe0_ps", bufs=2, space="PSUM") as pe0_ps):
                base = 0
                while base < n_nodes:
                    ch = min(CH, (n_nodes - base) // P)
                    partial = ch == 0
                    ch = max(ch, 1)
                    rows = rem if partial else ch * P
                    ftc = pe0.tile([P, CH * P], BF16, tag="ftc", name="ftc")
                    nc.sync.dma_start(out=ftc[:, :rows],
                                      in_=featT_t.ap()[:, base:base + rows])
                    eps_ = pe0_ps.tile([P, CH, nh0], F32, name="erps")
                    for i in range(ch):
                        m = rows - i * P if partial else P
                        nc.tensor.matmul(eps_[:m, i, :],
                                         lhsT=ftc[:, i * P:i * P + m],
                                         rhs=w0e_sb[:, d0 + nh0:d0 + 2 * nh0],
                                         start=True, stop=True)
                    erst = pe0.tile([P, CH, nh0], F32, tag="erst", name="erst")
                    nc.scalar.copy(out=erst[:, :ch, :], in_=eps_[:, :ch, :])
                    r0 = base // 16
                    if partial:
                        nc.sync.dma_start(
                            out=er0p.ap()[r0:r0 + rows // 16, :].rearrange(
                                "a (p d) -> (a p) d", p=16),
                            in_=erst[:rows, 0, :])
                    else:
                        nc.sync.dma_start(
                            out=er0p.ap()[r0:r0 + rows // 16, :].rearrange(
                                "(i r8) (p16 d) -> (r8 p16) i d",
                                i=ch, p16=16),
                            in_=erst[:, :ch, :])
                    base += rows

            # ---- er0_tile gather (overlaps phase A below) ----
            ebld_cm = tc.tile_pool(name="ebld", bufs=1)
            ebld = ebld_cm.__enter__()
            ERAW = ebld.tile([P, tpc, 64], F32, tag="eraw", name="eraw")
            nc.gpsimd.dma_gather(
                out_ap=ERAW[:], in_ap=er0p.ap(),
                idxs_ap=eri16_sb[:], num_idxs=npc_pad,
                num_idxs_reg=npc_pad, elem_size=64, elem_step=64,
                single_packet=False, queue_num=1)

            # ---- Phase A: replicated dense layer 0 -> tab0 ----
            with (tc.tile_pool(name="pa", bufs=3) as pa,
                  tc.tile_pool(name="pa_ph", bufs=3, space="PSUM") as pa_ph):
                base = 0
                chunk_i = 0
                while base < n_nodes:
                    ch = min(CH, (n_nodes - base) // P)
                    partial = ch == 0
                    ch = max(ch, 1)
                    rows = rem if partial else ch * P
                    ftc = pa.tile([P, CH * P], BF16, tag="ftc", name="ftc")
                    nc.sync.dma_start(out=ftc[:, :rows],
                                      in_=featT_t.ap()[:, base:base + rows])
                    hstage = pa.tile([P, CH, row0], BF16, tag="hstage",
                                     name="hstage")
                    if chunk_i < 3:  # pool bufs: pad cols stay zero on reuse
                        nc.vector.memset(hstage[:, :, d0 + nh0:row0], 0.0)
                    chunk_i += 1
                    for i in range(ch):
                        m = rows - i * P if partial else P
                        hps = pa_ph.tile([P, d0 + nh0], F32, name="hps")
                        nc.tensor.matmul(hps[:m, :], lhsT=ftc[:, i * P:i * P + m],
                                         rhs=w0e_sb[:, 0:d0 + nh0],
                                         start=True, stop=True)
                        nc.scalar.copy(out=hstage[:m, i, 0:d0 + nh0],
                                       in_=hps[:m, :])
                    # write chunk rows once into the single table
                    weng = nc.sync if (chunk_i % 2 == 0) else nc.scalar
                    if partial:
                        weng.dma_start(
                            out=tab0.ap()[base + 1:base + rows + 1, :],
                            in_=hstage[:rows, 0, :])
                    else:
                        weng.dma_start(
                            out=tab0.ap()[base + 1:base + rows + 1, :].rearrange(
                                "(i p) d -> p i d", p=P),
                            in_=hstage[:, :ch, :])
                    base += rows

            # ---- er0_tile select-reduce ----
            EMUL = ebld.tile([P, tpc, 64], F32, tag="emul", name="emul")
            nc.vector.tensor_tensor(out=EMUL[:], in0=ERAW[:],
                                    in1=_bcast_mid(ersel_sb[:], tpc),
                                    op=mybir.AluOpType.mult)
            ERED = ebld.tile([P, tpc, nh0], F32, tag="ered", name="ered")
            nc.vector.tensor_reduce(
                out=ERED[:],
                in_=EMUL[:].rearrange("p t (s h) -> p t h s", h=nh0),
                axis=mybir.AxisListType.X, op=mybir.AluOpType.add)
            nc.scalar.copy(out=er0_tile[:], in_=ERED[:])
            ebld_cm.__exit__(None, None, None)

            # ---- shared pools for edge phases ----
            with (tc.tile_pool(name="hg", bufs=6) as hg_pool,
                  tc.tile_pool(name="ms", bufs=4) as ms_pool,
                  tc.tile_pool(name="mk", bufs=4) as mask_pool,
                  tc.tile_pool(name="mt", bufs=6) as mt_pool,
                  tc.tile_pool(name="sm", bufs=3) as small_pool,
                  tc.tile_pool(name="eb", bufs=3) as eb_pool,
                  tc.tile_pool(name="fin", bufs=2) as fin_pool,
                  tc.tile_pool(name="ps_acc", bufs=5, space="PSUM") as psum_acc,
                  tc.tile_pool(name="ps_tp", bufs=1, space="PSUM") as psum_tp,
                  tc.tile_pool(name="ps_er", bufs=1, space="PSUM") as psum_er,
                  tc.tile_pool(name="ps_h2", bufs=1, space="PSUM") as psum_h2):

                def finalize0(t, acc):
                    rows = min(P, npc - t * P)
                    S = small_pool.tile([P, nh0], F32, tag="s0", name="s0")
                    nc.vector.tensor_tensor(out=S[:], in0=acc[:, d0:d0 + nh0],
                                            in1=eps0[:],
                                            op=mybir.AluOpType.max)
                    RC = small_pool.tile([P, nh0], F32, tag="rc0", name="rc0")
                    nc.vector.reciprocal(RC[:], S[:])
                    H1T = fin_pool.tile([P, d0], BF16, tag="h1t", name="h1t")
                    nc.vector.tensor_tensor(
                        out=H1T[:].rearrange("p (h e) -> p h e", h=nh0),
                        in0=acc[:, 0:d0].rearrange("p (h e) -> p h e", h=nh0),
                        in1=_bcast_inner(RC[:], hid0),
                        op=mybir.AluOpType.mult)
                    h2ps = psum_h2.tile([P, d1 + 2 * nh1], F32, name="h2ps")
                    for k in range(kchunks):
                        tp = psum_tp.tile([P, P], BF16, tag="tp", name="ftp")
                        nc.tensor.transpose(tp[:], H1T[:, k * P:(k + 1) * P],
                                            ident_sb[:])
                        ts = fin_pool.tile([P, P], BF16, tag="tsb", name="tsb")
                        nc.scalar.copy(out=ts[:], in_=tp[:])
                        nc.tensor.matmul(h2ps[:], lhsT=ts[:], rhs=w1e_sb[:, k, :],
                                         start=(k == 0), stop=(k == kchunks - 1))
                    h2sb = fin_pool.tile([P, d1 + 2 * nh1], BF16, tag="h2sb",
                                         name="h2sb")
                    nc.scalar.copy(out=h2sb[:], in_=h2ps[:])
                    nc.sync.dma_start(out=h2slice.ap()[t * P:t * P + rows],
                                      in_=h2sb[:rows, :])
                    nc.sync.dma_start(out=er1locT.ap()[:, t:t + 1],
                                      in_=h2sb[:, d1 + nh1:d1 + 2 * nh1])

                # chunked AllGather: after the last tile of each tile-group
                # finishes, gather that row range and repack it into tab1,
                # overlapping with the tail of the layer-0 edge phase.
                rw1 = d1 + 2 * nh1
                grp_last = {last_t: g for g, (_, _, last_t) in
                            enumerate(grp_bounds)}
                tab1_t = tab1.ap().tensor

                def tile_done0(t):
                    if t not in grp_last:
                        return
                    g = grp_last[t]
                    lo, hi, _ = grp_bounds[g]
                    nc.gpsimd.collective_compute(
                        "AllGather", mybir.AluOpType.bypass,
                        replica_groups=[list(range(NCORES))],
                        ins=[h2slice.ap()[lo:hi]],
                        outs=[h2fullg[g].ap()])
                    # tab1 row for node (c, l) is 1 + c*npc + l
                    out_ap = bass.AP(
                        tensor=tab1_t, offset=(1 + lo) * row1,
                        ap=[[npc * row1, NCORES], [row1, hi - lo], [1, rw1]])
                    nc.sync.dma_start(out=out_ap, in_=h2fullg[g].ap())

                _edge_phase(nc, tc,
                            (hg_pool, ms_pool, mask_pool, mt_pool, small_pool,
                             eb_pool, psum_acc, psum_tp, psum_er),
                            tab0.ap()[0:wrows], tab0.ap()[wb_base:wb_base + wrows],
                            er0_tile, d0, nh0, hid0,
                            row0, plan, src16_sb, dstloc_sb, iota_sb, ident_sb,
                            slope_sb, finalize0, tile_done=tile_done0)

                nc.sync.dma_start(
                    out=er1_tile[:, :, 0],
                    in_=er1locT.ap())

                def finalize1(t, acc):
                    rows = min(P, npc - t * P)
                    S = small_pool.tile([P, nh1], F32, tag="s1", name="s1")
                    nc.vector.tensor_tensor(out=S[:], in0=acc[:, d1:d1 + nh1],
                                            in1=eps1[:],
                                            op=mybir.AluOpType.max)
                    RC = small_pool.tile([P, nh1], F32, tag="rc1", name="rc1")
                    nc.vector.reciprocal(RC[:], S[:])
                    OUT = fin_pool.tile([P, d1], F32, tag="outt", name="outt")
                    nc.vector.tensor_tensor(out=OUT[:], in0=acc[:, 0:d1],
                                            in1=_bcast_inner(RC[:], d1),
                                            op=mybir.AluOpType.mult)
                    nc.sync.dma_start(out=out_t.ap()[t * P:t * P + rows],
                                      in_=OUT[:rows, :])

                _edge_phase(nc, tc,
                            (hg_pool, ms_pool, mask_pool, mt_pool, small_pool,
                             eb_pool, psum_acc, psum_tp, psum_er),
                            tab1.ap()[0:wrows], tab1.ap()[wb_base:wb_base + wrows],
                            er1_tile, d1, nh1, hid1,
                            row1, plan, src16_sb, dstloc_sb, iota_sb, ident_sb,
                            slope_sb, finalize1)

    nc.compile()
    if os.environ.get("GAT_COMPILE_ONLY", "0") == "1":
        LAST_BUILD[0] = (nc, None)
        return np.zeros((n_nodes, d1), np.float32)

    in_maps = []
    for c in range(NCORES):
        in_maps.append({
            "featT": featT,
            "w0e": np.ascontiguousarray(w0e).astype(BFNP),
            "w1e": w1p,
            "iota": np.ascontiguousarray(iota),
            "ident": np.ascontiguousarray(ident),
            "src16": _wrap16(plan.srcw[c]),
            "dstloc": np.ascontiguousarray(
                plan.dstlocv[c].reshape(totblk, P).T).astype(BFNP),
            "eri16": _wrap16(eri[c]),
            "ersel": np.ascontiguousarray(ersel[c]),
        })
    LAST_BUILD[0] = (nc, in_maps)
    if simulate:
        from concourse import bass_interp
        sim = bass_interp.MultiCoreSim(nc, NCORES, ignore_data_errors=True)
        for c in range(NCORES):
            for k, v in in_maps[c].items():
                sim.cores[c].tensor(k)[:] = v
        sim.simulate()
        LAST_SIM[0] = sim
        out = np.concatenate(
            [np.array(sim.cores[c].tensor("out")) for c in range(NCORES)], axis=0)
        return out
    res = run_bass_kernel_spmd(nc, in_maps, list(range(NCORES)), trace=trace)
    LAST_RES[0] = res
    LAST_EXEC_NS[0] = res.exec_time_ns
    out = np.concatenate([res.results[c]["out"] for c in range(NCORES)], axis=0)
    return out


def kernel(feat, src, dst, W0, al0, ar0, W1, al1, ar1):
    trace = os.environ.get("GAT_TRACE", "0") == "1"
    out = build_and_run(np.asarray(feat), np.asarray(src), np.asarray(dst),
                        np.asarray(W0), np.asarray(al0), np.asarray(ar0),
                        np.asarray(W1), np.asarray(al1), np.asarray(ar1),
                        trace=trace)
    return out.astype(np.float32)


# revision 43
# speedup vs baseline: 2.2848x; 1.1329x over previous
"""2-layer GAT on 8 Trainium2 NeuronCores (Bass/Tile), v2.

Strategy (dst-sharded graph parallelism, SWDGE-aware):
  - The dominant cost on this hardware is GpSimd (Q7) descriptor generation
    for dma_gather (~8ns/index, serialized per SWDGE queue).  v2 therefore:
      * uses 4 SWDGE queues with round-robin assignment (desc-gen runs on a
        different Q7 cpu pair per queue),
      * eliminates the per-edge er gathers entirely: er[dst] is broadcast to
        edges with a tiny matmul  ERE[e,h] = sum_j maskT[j,e] * er_tile[j,h]
        where maskT is the PE-transposed 0/1 dst-selection mask,
      * eliminates the erloc window-gather pairs (layer-0 er comes from a
        packed er0p table via ONE gather + select-reduce; layer-1 er is
        written column-wise into er1locT during finalize0, no gather).
  - All node tables, masks and matmul operands are bf16 (4x faster PE than
    fp32, half the DMA bytes); PSUM accumulation stays fp32.
  - feat is pre-transposed on the host so layer-0's dense projection needs
    no PE transposes and no PSUM round-trip copies.
  - Edges sorted by dst, sharded by dst range (6250 nodes/core), blocks of
    128 on SBUF partitions; src rows fetched with int16 dma_gather through
    two overlapping 32768-row windows (int16 index limit).
  - Per edge block: one fp32-accumulating bf16 matmul adds both the
    weighted message sum and the softmax denominator into PSUM.
  - Between layers: project locally, AllGather the small bf16 [N,42] table,
    repack into window tables, run the same edge pipeline for layer 1.
"""

import os
import numpy as np
import ml_dtypes

import concourse.bass as bass
import concourse.bacc as bacc
import concourse.mybir as mybir
import concourse.tile as tile
from concourse.bass_utils import run_bass_kernel_spmd

F32 = mybir.dt.float32
BF16 = mybir.dt.bfloat16
I16 = mybir.dt.int16
BFNP = ml_dtypes.bfloat16

SLOPE = 0.2
NCORES = 8
P = 128
NQ = 4          # SWDGE queues
G = 16          # max edge blocks per gather chunk
ST = 4          # dst tiles per supertile (lo/hi run batching)
CH = 8          # node tiles per phase-A chunk
WROWS = 32768   # rows per index window
LAST_EXEC_NS = [None]
LAST_RES = [None]
LAST_SIM = [None]
LAST_BUILD = [None]


def _bcast_inner(apv, count):
    return bass.AP(tensor=apv.tensor, offset=apv.offset, ap=apv.ap + [[0, count]])


def _bcast_mid(apv, count):
    a = apv.ap
    return bass.AP(tensor=apv.tensor, offset=apv.offset, ap=[a[0], [0, count]] + a[1:])


def _fuse_w(W, al, ar):
    Fin = W.shape[0]
    H, D = al.shape
    Wr = W.reshape(Fin, H, D)
    wl = np.einsum("khd,hd->kh", Wr, al).astype(np.float32)
    wr = np.einsum("khd,hd->kh", Wr, ar).astype(np.float32)
    return np.ascontiguousarray(np.concatenate([W, wl, wr], axis=1), dtype=np.float32)


def _wrap16(idx):
    """int16 idx list (len multiple of 128) -> dma_gather SBUF layout
    [128, len/16]: idx j at [j % 16, j // 16], replicated across 8 groups."""
    w = idx.reshape(-1, 16).T.astype(np.int16)
    return np.ascontiguousarray(np.tile(w, (8, 1)))


def _prep_edges(src, dst, n_nodes, ncores, wrows):
    from types import SimpleNamespace
    plan = SimpleNamespace()
    npc = n_nodes // ncores
    tpc = (npc + P - 1) // P
    plan.npc, plan.tpc = npc, tpc
    wa_max = wrows - 2               # node i valid in A iff i+1 <= wrows-1
    plan.wb_base = n_nodes + 2 - wrows

    order = np.argsort(dst, kind="stable")
    ss = src[order].astype(np.int64)
    ds = dst[order].astype(np.int64)
    core = ds // npc
    loc = ds % npc
    tileid = loc // P
    hi = (ss > wa_max).astype(np.int64)

    counts = np.zeros((ncores, tpc, 2), np.int64)
    np.add.at(counts, (core, tileid, hi), 1)
    nblk = (counts + P - 1) // P
    bcnt = nblk.max(axis=0)
    if bcnt.sum() == 0:
        bcnt[0, 0] = 1
    plan.bcnt = bcnt

    plan.sts = [list(range(s, min(s + ST, tpc))) for s in range(0, tpc, ST)]
    plan.order_blocks = []
    for tiles in plan.sts:
        for w in (0, 1):
            for t in tiles:
                plan.order_blocks += [(t, w)] * int(bcnt[t, w])
    plan.totblk = len(plan.order_blocks)
    plan.nedge = plan.totblk * P

    slot = {}
    pos = 0
    for (t, w) in plan.order_blocks:
        if (t, w) not in slot:
            slot[(t, w)] = pos
        pos += P

    srcw = np.zeros((ncores, plan.nedge), np.int64)
    dstloc = np.full((ncores, plan.nedge), 999.0, np.float32)
    for bi, (t, w) in enumerate(plan.order_blocks):
        if w == 1:
            srcw[:, bi * P:(bi + 1) * P] = wrows - 1

    # order edges by (core, tile, win) groups
    gkey = (core * tpc + tileid) * 2 + hi
    g_order = np.argsort(gkey, kind="stable")
    ss2, loc2 = ss[g_order], loc[g_order]
    gstart = np.zeros(ncores * tpc * 2 + 1, np.int64)
    np.add.at(gstart[1:], gkey[g_order], 1)
    gstart = np.cumsum(gstart)
    for c in range(ncores):
        for t in range(tpc):
            for w in (0, 1):
                k = (c * tpc + t) * 2 + w
                e0, e1 = int(gstart[k]), int(gstart[k + 1])
                cnt = e1 - e0
                if cnt == 0:
                    continue
                off = slot[(t, w)]
                srcs = ss2[e0:e1]
                srcw[c, off:off + cnt] = (
                    srcs + 1 if w == 0 else srcs + 1 - plan.wb_base)
                dstloc[c, off:off + cnt] = (loc2[e0:e1] % P).astype(np.float32)

    plan.srcw, plan.dstlocv = srcw, dstloc
    return plan


def _edge_phase(nc, tc, pools, tabA_ap, tabB_ap, er_tile, d, nheads, hdim, gw,
                plan, src16_sb, dstloc_sb, iota_sb, ident_sb, slope_sb,
                finalize, tile_done=None):
    """Edge pipeline for one layer.  Gathered bf16 row: [h(d) | el(nheads) |
    pad], gw elems.  er comes from er_tile [P, tpc, nheads] (bf16 SBUF) via
    maskT matmul broadcast.  tile_done(t) is called after finalize(t)."""
    (hg_pool, ms_pool, mask_pool, mt_pool, small_pool, eb_pool,
     psum_acc, psum_tp, psum_er) = pools
    first_blk, last_blk = {}, {}
    for bi, (t, w) in enumerate(plan.order_blocks):
        if t not in first_blk:
            first_blk[t] = bi
        last_blk[t] = bi
    acc_by_tile = {}

    # chunks: maximal runs of <=G blocks within a single window
    chunks = []
    cur = None
    for bi, (t, w) in enumerate(plan.order_blocks):
        if cur is None or cur[0] != w or bi - cur[1] >= G:
            if cur is not None:
                chunks.append(cur)
            cur = [w, bi, bi + 1]
        else:
            cur[2] = bi + 1
        if cur[2] - cur[1] >= G:
            chunks.append(cur)
            cur = None
    if cur is not None:
        chunks.append(cur)

    for ci, (w, b0, b1) in enumerate(chunks):
        nb = b1 - b0
        nidx = nb * P
        HG = hg_pool.tile([P, G, gw], BF16, tag="hg", name="hg")
        nc.gpsimd.dma_gather(
            out_ap=HG[:, :nb, :], in_ap=(tabA_ap if w == 0 else tabB_ap),
            idxs_ap=src16_sb[:, b0 * 8:b1 * 8], num_idxs=nidx,
            num_idxs_reg=nidx, elem_size=gw, elem_step=gw,
            single_packet=False, queue_num=ci % NQ)
        # mask[e, b, j] = (dstloc[e, b] == j)   (bf16 0/1)
        MASK = mask_pool.tile([P, G, P], BF16, tag="mask", name="mask")
        nc.vector.tensor_tensor(
            out=MASK[:, :nb, :],
            in0=_bcast_inner(dstloc_sb[:, b0:b1], P),
            in1=_bcast_mid(iota_sb[:], nb),
            op=mybir.AluOpType.is_equal)
        # per-block: maskT (PE transpose) + er broadcast matmul into one
        # chunk-wide PSUM strip; then a single E4 = el + er add.
        EREC = psum_er.tile([P, G, nheads], F32, tag="erec", name="erec")
        for bi in range(b0, b1):
            t, _ = plan.order_blocks[bi]
            TP = psum_tp.tile([P, P], BF16, tag="tp", name="tp")
            nc.tensor.transpose(TP[:], MASK[:, bi - b0, :], ident_sb[:])
            MT = mt_pool.tile([P, P], BF16, tag="mt", name="mt")
            nc.vector.tensor_copy(MT[:], TP[:])
            nc.tensor.matmul(EREC[:, bi - b0, :], lhsT=MT[:],
                             rhs=er_tile[:, t, :], start=True, stop=True)
        E4 = small_pool.tile([P, G, nheads], F32, tag="e4", name="e4")
        nc.vector.tensor_add(E4[:, :nb, :], HG[:, :nb, d:d + nheads],
                             EREC[:, :nb, :])
        # expe = exp(lrelu(E4))  (lrelu via DVE mul+max; Exp on scalar), bf16
        ESC = small_pool.tile([P, G, nheads], F32, tag="esc", name="esc")
        nc.vector.tensor_tensor(out=ESC[:, :nb, :], in0=E4[:, :nb, :],
                                in1=_bcast_mid(slope_sb[:, 0:nheads], nb),
                                op=mybir.AluOpType.mult)
        nc.vector.tensor_tensor(out=E4[:, :nb, :], in0=E4[:, :nb, :],
                                in1=ESC[:, :nb, :], op=mybir.AluOpType.max)
        EB = eb_pool.tile([P, G, nheads], BF16, tag="eb", name="eb")
        nc.scalar.activation(out=EB[:, :nb, :], in_=E4[:, :nb, :],
                             func=mybir.ActivationFunctionType.Exp)
        # MS = [expe-scaled h | expe]  (bf16)
        msw = d + nheads
        MS = ms_pool.tile([P, G, msw], BF16, tag="ms", name="ms")
        for h in range(nheads):
            nc.vector.tensor_tensor(
                out=MS[:, :nb, h * hdim:(h + 1) * hdim],
                in0=HG[:, :nb, h * hdim:(h + 1) * hdim],
                in1=_bcast_inner(EB[:, :nb, h:h + 1], hdim),
                op=mybir.AluOpType.mult)
        nc.scalar.copy(out=MS[:, :nb, d:d + nheads], in_=EB[:, :nb, :])
        for bi in range(b0, b1):
            t, _ = plan.order_blocks[bi]
            if bi == first_blk[t]:
                acc_by_tile[t] = psum_acc.tile([P, msw], F32, tag="acc",
                                               name="acc")
            acc = acc_by_tile[t]
            nc.tensor.matmul(acc[:], lhsT=MASK[:, bi - b0, :],
                             rhs=MS[:, bi - b0, :],
                             start=(bi == first_blk[t]),
                             stop=(bi == last_blk[t]))
            if bi == last_blk[t]:
                finalize(t, acc)
                del acc_by_tile[t]
                if tile_done is not None:
                    tile_done(t)


def build_and_run(feat, src, dst, W0, al0, ar0, W1, al1, ar1, trace=False,
                  simulate=False):
    n_nodes = feat.shape[0]
    npc = n_nodes // NCORES
    nh0 = al0.shape[0]
    hid0 = al0.shape[1]
    d0 = nh0 * hid0                        # 256
    row0 = ((d0 + nh0 + 127) // 128) * 128  # 384 bf16 = 768B rows
    nh1 = al1.shape[0]
    hid1 = al1.shape[1]
    d1 = nh1 * hid1                        # 40
    row1 = ((d1 + 2 * nh1 + 127) // 128) * 128  # 128 bf16 = 256B rows
    in_dim = feat.shape[1]
    assert in_dim == P
    wrows = min(WROWS, n_nodes + 2)
    wb_base = n_nodes + 2 - wrows

    w0e = _fuse_w(W0, al0, ar0)            # [in_dim, d0+2nh0]
    w1e = _fuse_w(W1, al1, ar1)            # [d0, d1+2nh1]
    kchunks = d0 // P
    w1p = np.ascontiguousarray(
        w1e.reshape(kchunks, P, d1 + 2 * nh1).transpose(1, 0, 2)).astype(BFNP)
    featT = np.ascontiguousarray(feat.T).astype(BFNP)    # [128, N]

    plan = _prep_edges(src, dst, n_nodes, NCORES, wrows)
    totblk = plan.totblk
    tpc = plan.tpc
    npc_pad = tpc * P

    # layer-0 er gather: one idx per (tile t, partition j) -> er0p row
    # (32 bf16-nodes per 256B row); selection mask W picks the right 4 vals.
    er0p_rows = (n_nodes + 31) // 32 + 1
    eri = np.zeros((NCORES, npc_pad), np.int64)
    ersel = np.zeros((NCORES, P, 128), np.float32)
    for c in range(NCORES):
        g = c * npc + np.arange(npc_pad, dtype=np.int64)
        g = np.minimum(g, n_nodes - 1)
        eri[c] = g // 32
        sub = (c * npc + np.arange(P, dtype=np.int64)) % 32
        for j in range(P):
            ersel[c, j, 4 * sub[j]:4 * sub[j] + 4] = 1.0

    iota = np.broadcast_to(np.arange(P, dtype=np.float32), (P, P)).astype(BFNP)
    ident = np.eye(P, dtype=np.float32).astype(BFNP)

    nc = bacc.Bacc(None, target_bir_lowering=False, num_devices=NCORES,
                   num_swdge_queues=NQ)
    featT_t = nc.declare_dram_parameter("featT", [P, n_nodes], BF16, False)
    w0e_t = nc.declare_dram_parameter("w0e", [P, d0 + 2 * nh0], BF16, False)
    w1e_t = nc.declare_dram_parameter("w1e", [P, kchunks, d1 + 2 * nh1], BF16,
                                      False)
    iota_t = nc.declare_dram_parameter("iota", [P, P], BF16, False)
    ident_t = nc.declare_dram_parameter("ident", [P, P], BF16, False)
    src16_t = nc.declare_dram_parameter("src16", [P, totblk * 8], I16, False)
    dstloc_t = nc.declare_dram_parameter("dstloc", [P, totblk], BF16, False)
    eri16_t = nc.declare_dram_parameter("eri16", [P, npc_pad // 16], I16, False)
    ersel_t = nc.declare_dram_parameter("ersel", [P, 128], F32, False)
    out_t = nc.declare_dram_parameter("out", [npc, d1], F32, True)

    # single tables; window A = rows [0, wrows), window B = rows
    # [wb_base, wb_base+wrows) of the same tensor (node i lives at row i+1)
    tab0 = nc.dram_tensor("tab0", [n_nodes + 2, row0], BF16)
    tab1 = nc.dram_tensor("tab1", [n_nodes + 2, row1], BF16)
    er0p = nc.dram_tensor("er0p", [er0p_rows, 128], BF16)
    er1locT = nc.dram_tensor("er1locT", [P, tpc], BF16)
    h2slice = nc.dram_tensor("h2slice", [npc, d1 + 2 * nh1], BF16)
    NGRP = 4
    gsz = (tpc + NGRP - 1) // NGRP
    grp_bounds = []
    for g in range(NGRP):
        tlo, thi = g * gsz, min((g + 1) * gsz, tpc)
        if tlo < thi:
            grp_bounds.append((tlo * P, min(thi * P, npc), thi - 1))
    h2fullg = [
        nc.dram_tensor(f"h2full{g}", [NCORES, hi - lo, d1 + 2 * nh1], BF16,
                       addr_space="Shared")
        for g, (lo, hi, _) in enumerate(grp_bounds)]

    nt_full = n_nodes // P
    rem = n_nodes - nt_full * P
    wa_nodes = (0, wrows - 1)
    wb_nodes = (wb_base - 1, n_nodes)

    with tile.TileContext(nc) as tc:
        with tc.tile_pool(name="singles", bufs=1) as singles:
            iota_sb = singles.tile([P, P], BF16)
            nc.sync.dma_start(out=iota_sb[:], in_=iota_t.ap())
            ident_sb = singles.tile([P, P], BF16)
            nc.sync.dma_start(out=ident_sb[:], in_=ident_t.ap())
            w0e_sb = singles.tile([P, d0 + 2 * nh0], BF16)
            nc.sync.dma_start(out=w0e_sb[:], in_=w0e_t.ap())
            w1e_sb = singles.tile([P, kchunks, d1 + 2 * nh1], BF16)
            nc.sync.dma_start(out=w1e_sb[:], in_=w1e_t.ap())
            src16_sb = singles.tile([P, totblk * 8], I16)
            nc.sync.dma_start(out=src16_sb[:], in_=src16_t.ap())
            dstloc_sb = singles.tile([P, totblk], BF16)
            nc.sync.dma_start(out=dstloc_sb[:], in_=dstloc_t.ap())
            eri16_sb = singles.tile([P, npc_pad // 16], I16)
            nc.sync.dma_start(out=eri16_sb[:], in_=eri16_t.ap())
            ersel_sb = singles.tile([P, 128], F32)
            nc.sync.dma_start(out=ersel_sb[:], in_=ersel_t.ap())
            er0_tile = singles.tile([P, tpc, nh0], BF16)
            er1_tile = singles.tile([P, tpc, nh1], BF16)
            eps0 = singles.tile([P, nh0], F32)
            nc.vector.memset(eps0[:], 1e-30)
            eps1 = singles.tile([P, nh1], F32)
            nc.vector.memset(eps1[:], 1e-30)
            slope_sb = singles.tile([P, nh0], F32)
            nc.vector.memset(slope_sb[:], SLOPE)
            zrow = singles.tile([P, row0], BF16)
            nc.vector.memset(zrow[:], 0.0)
            # zero guard rows (row 0 and row n_nodes+1 of each table)
            nc.sync.dma_start(out=tab0.ap()[0:1], in_=zrow[:1, :row0])
            nc.sync.dma_start(out=tab0.ap()[n_nodes + 1:n_nodes + 2],
                              in_=zrow[:1, :row0])
            nc.sync.dma_start(out=tab1.ap()[0:1], in_=zrow[:1, :row1])
            nc.sync.dma_start(out=tab1.ap()[n_nodes + 1:n_nodes + 2],
                              in_=zrow[:1, :row1])

            # ---- Phase A: replicated dense layer 0 -> tab0 + er0p ----
            with (tc.tile_pool(name="pa", bufs=3) as pa,
                  tc.tile_pool(name="pa_ph", bufs=3, space="PSUM") as pa_ph):
                base = 0
                chunk_i = 0
                while base < n_nodes:
                    ch = min(CH, (n_nodes - base) // P)
                    partial = ch == 0
                    ch = max(ch, 1)
                    rows = rem if partial else ch * P
                    ftc = pa.tile([P, CH * P], BF16, tag="ftc", name="ftc")
                    nc.sync.dma_start(out=ftc[:, :rows],
                                      in_=featT_t.ap()[:, base:base + rows])
                    hstage = pa.tile([P, CH, row0], BF16, tag="hstage",
                                     name="hstage")
                    if chunk_i < 3:  # pool bufs: pad cols stay zero on reuse
                        nc.vector.memset(hstage[:, :, d0 + 2 * nh0:row0], 0.0)
                    chunk_i += 1
                    for i in range(ch):
                        m = rows - i * P if partial else P
                        hps = pa_ph.tile([P, d0 + 2 * nh0], F32, name="hps")
                        nc.tensor.matmul(hps[:m, :], lhsT=ftc[:, i * P:i * P + m],
                                         rhs=w0e_sb[:], start=True, stop=True)
                        nc.scalar.copy(out=hstage[:m, i, 0:d0 + 2 * nh0],
                                       in_=hps[:m, :])
                    # write chunk rows once into the single table
                    weng = nc.sync if (chunk_i % 2 == 0) else nc.scalar
                    if partial:
                        weng.dma_start(
                            out=tab0.ap()[base + 1:base + rows + 1, :],
                            in_=hstage[:rows, 0, :])
                    else:
                        weng.dma_start(
                            out=tab0.ap()[base + 1:base + rows + 1, :].rearrange(
                                "(i p) d -> p i d", p=P),
                            in_=hstage[:, :ch, :])
                    # er columns -> packed er0p (node-major bf16, 4 per node)
                    er0p_t = er0p.ap().tensor
                    if partial:
                        nc.sync.dma_start(
                            out=bass.AP(tensor=er0p_t, offset=base * 4,
                                        ap=[[4, rows], [1, 4]]),
                            in_=hstage[:rows, 0, d0 + nh0:d0 + 2 * nh0])
                    else:
                        nc.sync.dma_start(
                            out=bass.AP(tensor=er0p_t, offset=base * 4,
                                        ap=[[4, P], [4 * P, ch], [1, 4]]),
                            in_=hstage[:, :ch, d0 + nh0:d0 + 2 * nh0])
                    base += rows

            # ---- er0_tile: one gather + select-reduce ----
            with tc.tile_pool(name="ebld", bufs=1) as ebld:
                ERAW = ebld.tile([P, tpc, 128], BF16, tag="eraw", name="eraw")
                nc.gpsimd.dma_gather(
                    out_ap=ERAW[:], in_ap=er0p.ap(),
                    idxs_ap=eri16_sb[:], num_idxs=npc_pad,
                    num_idxs_reg=npc_pad, elem_size=128, elem_step=128,
                    single_packet=False, queue_num=1)
                EMUL = ebld.tile([P, tpc, 128], F32, tag="emul", name="emul")
                nc.vector.tensor_tensor(out=EMUL[:], in0=ERAW[:],
                                        in1=_bcast_mid(ersel_sb[:], tpc),
                                        op=mybir.AluOpType.mult)
                ERED = ebld.tile([P, tpc, nh0], F32, tag="ered", name="ered")
                nc.vector.tensor_reduce(
                    out=ERED[:],
                    in_=EMUL[:].rearrange("p t (s h) -> p t h s", h=nh0),
                    axis=mybir.AxisListType.X, op=mybir.AluOpType.add)
                nc.scalar.copy(out=er0_tile[:], in_=ERED[:])

            # ---- shared pools for edge phases ----
            with (tc.tile_pool(name="hg", bufs=6) as hg_pool,
                  tc.tile_pool(name="ms", bufs=4) as ms_pool,
                  tc.tile_pool(name="mk", bufs=4) as mask_pool,
                  tc.tile_pool(name="mt", bufs=6) as mt_pool,
                  tc.tile_pool(name="sm", bufs=3) as small_pool,
                  tc.tile_pool(name="eb", bufs=3) as eb_pool,
                  tc.tile_pool(name="fin", bufs=2) as fin_pool,
                  tc.tile_pool(name="ps_acc", bufs=4, space="PSUM") as psum_acc,
                  tc.tile_pool(name="ps_tp", bufs=2, space="PSUM") as psum_tp,
                  tc.tile_pool(name="ps_er", bufs=1, space="PSUM") as psum_er,
                  tc.tile_pool(name="ps_h2", bufs=1, space="PSUM") as psum_h2):

                def finalize0(t, acc):
                    rows = min(P, npc - t * P)
                    S = small_pool.tile([P, nh0], F32, tag="s0", name="s0")
                    nc.vector.tensor_tensor(out=S[:], in0=acc[:, d0:d0 + nh0],
                                            in1=eps0[:],
                                            op=mybir.AluOpType.max)
                    RC = small_pool.tile([P, nh0], F32, tag="rc0", name="rc0")
                    nc.vector.reciprocal(RC[:], S[:])
                    H1T = fin_pool.tile([P, d0], BF16, tag="h1t", name="h1t")
                    nc.vector.tensor_tensor(
                        out=H1T[:].rearrange("p (h e) -> p h e", h=nh0),
                        in0=acc[:, 0:d0].rearrange("p (h e) -> p h e", h=nh0),
                        in1=_bcast_inner(RC[:], hid0),
                        op=mybir.AluOpType.mult)
                    h2ps = psum_h2.tile([P, d1 + 2 * nh1], F32, name="h2ps")
                    for k in range(kchunks):
                        tp = psum_tp.tile([P, P], BF16, tag="tp", name="ftp")
                        nc.tensor.transpose(tp[:], H1T[:, k * P:(k + 1) * P],
                                            ident_sb[:])
                        ts = fin_pool.tile([P, P], BF16, tag="tsb", name="tsb")
                        nc.scalar.copy(out=ts[:], in_=tp[:])
                        nc.tensor.matmul(h2ps[:], lhsT=ts[:], rhs=w1e_sb[:, k, :],
                                         start=(k == 0), stop=(k == kchunks - 1))
                    h2sb = fin_pool.tile([P, d1 + 2 * nh1], BF16, tag="h2sb",
                                         name="h2sb")
                    nc.scalar.copy(out=h2sb[:], in_=h2ps[:])
                    nc.sync.dma_start(out=h2slice.ap()[t * P:t * P + rows],
                                      in_=h2sb[:rows, :])
                    nc.sync.dma_start(out=er1locT.ap()[:, t:t + 1],
                                      in_=h2sb[:, d1 + nh1:d1 + 2 * nh1])

                # chunked AllGather: after the last tile of each tile-group
                # finishes, gather that row range and repack it into tab1,
                # overlapping with the tail of the layer-0 edge phase.
                rw1 = d1 + 2 * nh1
                grp_last = {last_t: g for g, (_, _, last_t) in
                            enumerate(grp_bounds)}
                tab1_t = tab1.ap().tensor

                def tile_done0(t):
                    if t not in grp_last:
                        return
                    g = grp_last[t]
                    lo, hi, _ = grp_bounds[g]
                    nc.gpsimd.collective_compute(
                        "AllGather", mybir.AluOpType.bypass,
                        replica_groups=[list(range(NCORES))],
                        ins=[h2slice.ap()[lo:hi]],
                        outs=[h2fullg[g].ap()])
                    # tab1 row for node (c, l) is 1 + c*npc + l
                    out_ap = bass.AP(
                        tensor=tab1_t, offset=(1 + lo) * row1,
                        ap=[[npc * row1, NCORES], [row1, hi - lo], [1, rw1]])
                    nc.sync.dma_start(out=out_ap, in_=h2fullg[g].ap())

                _edge_phase(nc, tc,
                            (hg_pool, ms_pool, mask_pool, mt_pool, small_pool,
                             eb_pool, psum_acc, psum_tp, psum_er),
                            tab0.ap()[0:wrows], tab0.ap()[wb_base:wb_base + wrows],
                            er0_tile, d0, nh0, hid0,
                            row0, plan, src16_sb, dstloc_sb, iota_sb, ident_sb,
                            slope_sb, finalize0, tile_done=tile_done0)

                nc.sync.dma_start(
                    out=er1_tile[:, :, 0],
                    in_=er1locT.ap())

                def finalize1(t, acc):
                    rows = min(P, npc - t * P)
                    S = small_pool.tile([P, nh1], F32, tag="s1", name="s1")
                    nc.vector.tensor_tensor(out=S[:], in0=acc[:, d1:d1 + nh1],
                                            in1=eps1[:],
                                            op=mybir.AluOpType.max)
                    RC = small_pool.tile([P, nh1], F32, tag="rc1", name="rc1")
                    nc.vector.reciprocal(RC[:], S[:])
                    OUT = fin_pool.tile([P, d1], F32, tag="outt", name="outt")
                    nc.vector.tensor_tensor(out=OUT[:], in0=acc[:, 0:d1],
                                            in1=_bcast_inner(RC[:], d1),
                                            op=mybir.AluOpType.mult)
                    nc.sync.dma_start(out=out_t.ap()[t * P:t * P + rows],
                                      in_=OUT[:rows, :])

                _edge_phase(nc, tc,
                            (hg_pool, ms_pool, mask_pool, mt_pool, small_pool,
                             eb_pool, psum_acc, psum_tp, psum_er),
                            tab1.ap()[0:wrows], tab1.ap()[wb_base:wb_base + wrows],
                            er1_tile, d1, nh1, hid1,
                            row1, plan, src16_sb, dstloc_sb, iota_sb, ident_sb,
                            slope_sb, finalize1)

    nc.compile()
    if os.environ.get("GAT_COMPILE_ONLY", "0") == "1":
        LAST_BUILD[0] = (nc, None)
        return np.zeros((n_nodes, d1), np.float32)

    in_maps = []
    for c in range(NCORES):
        in_maps.append({
            "featT": featT,
            "w0e": np.ascontiguousarray(w0e).astype(BFNP),
            "w1e": w1p,
            "iota": np.ascontiguousarray(iota),
            "ident": np.ascontiguousarray(ident),
            "src16": _wrap16(plan.srcw[c]),
            "dstloc": np.ascontiguousarray(
                plan.dstlocv[c].reshape(totblk, P).T).astype(BFNP),
            "eri16": _wrap16(eri[c]),
            "ersel": np.ascontiguousarray(ersel[c]),
        })
    LAST_BUILD[0] = (nc, in_maps)
    if simulate:
        from concourse import bass_interp
        sim = bass_interp.MultiCoreSim(nc, NCORES, ignore_data_errors=True)
        for c in range(NCORES):
            for k, v in in_maps[c].items():
                sim.cores[c].tensor(k)[:] = v
        sim.simulate()
        LAST_SIM[0] = sim
        out = np.concatenate(
            [np.array(sim.cores[c].tensor("out")) for c in range(NCORES)], axis=0)
        return out
    res = run_bass_kernel_spmd(nc, in_maps, list(range(NCORES)), trace=trace)
    LAST_RES[0] = res
    LAST_EXEC_NS[0] = res.exec_time_ns
    out = np.concatenate([res.results[c]["out"] for c in range(NCORES)], axis=0)
    return out


def kernel(feat, src, dst, W0, al0, ar0, W1, al1, ar1):
    trace = os.environ.get("GAT_TRACE", "0") == "1"
    out = build_and_run(np.asarray(feat), np.asarray(src), np.asarray(dst),
                        np.asarray(W0), np.asarray(al0), np.asarray(ar0),
                        np.asarray(W1), np.asarray(al1), np.asarray(ar1),
                        trace=trace)
    return out.astype(np.float32)


# revision 46
# speedup vs baseline: 2.3815x; 1.0424x over previous
"""2-layer GAT on 8 Trainium2 NeuronCores (Bass/Tile), v2.

Strategy (dst-sharded graph parallelism, SWDGE-aware):
  - The dominant cost on this hardware is GpSimd (Q7) descriptor generation
    for dma_gather (~8ns/index, serialized per SWDGE queue).  v2 therefore:
      * uses 4 SWDGE queues with round-robin assignment (desc-gen runs on a
        different Q7 cpu pair per queue),
      * eliminates the per-edge er gathers entirely: er[dst] is broadcast to
        edges with a tiny matmul  ERE[e,h] = sum_j maskT[j,e] * er_tile[j,h]
        where maskT is the PE-transposed 0/1 dst-selection mask,
      * eliminates the erloc window-gather pairs (layer-0 er comes from a
        packed er0p table via ONE gather + select-reduce; layer-1 er is
        written column-wise into er1locT during finalize0, no gather).
  - All node tables, masks and matmul operands are bf16 (4x faster PE than
    fp32, half the DMA bytes); PSUM accumulation stays fp32.
  - feat is pre-transposed on the host so layer-0's dense projection needs
    no PE transposes and no PSUM round-trip copies.
  - Edges sorted by dst, sharded by dst range (6250 nodes/core), blocks of
    128 on SBUF partitions; src rows fetched with int16 dma_gather through
    two overlapping 32768-row windows (int16 index limit).
  - Per edge block: one fp32-accumulating bf16 matmul adds both the
    weighted message sum and the softmax denominator into PSUM.
  - Between layers: project locally, AllGather the small bf16 [N,42] table,
    repack into window tables, run the same edge pipeline for layer 1.
"""

import os
import numpy as np
import ml_dtypes

import concourse.bass as bass
import concourse.bacc as bacc
import concourse.mybir as mybir
import concourse.tile as tile
from concourse.bass_utils import run_bass_kernel_spmd

F32 = mybir.dt.float32
BF16 = mybir.dt.bfloat16
I16 = mybir.dt.int16
BFNP = ml_dtypes.bfloat16

SLOPE = 0.2
NCORES = 8
P = 128
NQ = 4          # SWDGE queues
G = 16          # max edge blocks per gather chunk
ST = 4          # dst tiles per supertile (lo/hi run batching)
CH = 16         # node tiles per phase-A chunk
WROWS = 32768   # rows per index window
LAST_EXEC_NS = [None]
LAST_RES = [None]
LAST_SIM = [None]
LAST_BUILD = [None]


def _bcast_inner(apv, count):
    return bass.AP(tensor=apv.tensor, offset=apv.offset, ap=apv.ap + [[0, count]])


def _bcast_mid(apv, count):
    a = apv.ap
    return bass.AP(tensor=apv.tensor, offset=apv.offset, ap=[a[0], [0, count]] + a[1:])


def _fuse_w(W, al, ar):
    Fin = W.shape[0]
    H, D = al.shape
    Wr = W.reshape(Fin, H, D)
    wl = np.einsum("khd,hd->kh", Wr, al).astype(np.float32)
    wr = np.einsum("khd,hd->kh", Wr, ar).astype(np.float32)
    return np.ascontiguousarray(np.concatenate([W, wl, wr], axis=1), dtype=np.float32)


def _wrap16(idx):
    """int16 idx list (len multiple of 128) -> dma_gather SBUF layout
    [128, len/16]: idx j at [j % 16, j // 16], replicated across 8 groups."""
    w = idx.reshape(-1, 16).T.astype(np.int16)
    return np.ascontiguousarray(np.tile(w, (8, 1)))


def _prep_edges(src, dst, n_nodes, ncores, wrows):
    from types import SimpleNamespace
    plan = SimpleNamespace()
    npc = n_nodes // ncores
    tpc = (npc + P - 1) // P
    plan.npc, plan.tpc = npc, tpc
    wa_max = wrows - 2               # node i valid in A iff i+1 <= wrows-1
    plan.wb_base = n_nodes + 2 - wrows

    order = np.argsort(dst, kind="stable")
    ss = src[order].astype(np.int64)
    ds = dst[order].astype(np.int64)
    core = ds // npc
    loc = ds % npc
    tileid = loc // P
    hi = (ss > wa_max).astype(np.int64)

    counts = np.zeros((ncores, tpc, 2), np.int64)
    np.add.at(counts, (core, tileid, hi), 1)
    nblk = (counts + P - 1) // P
    bcnt = nblk.max(axis=0)
    if bcnt.sum() == 0:
        bcnt[0, 0] = 1
    plan.bcnt = bcnt

    plan.sts = [list(range(s, min(s + ST, tpc))) for s in range(0, tpc, ST)]
    plan.order_blocks = []
    for tiles in plan.sts:
        for w in (0, 1):
            for t in tiles:
                plan.order_blocks += [(t, w)] * int(bcnt[t, w])
    plan.totblk = len(plan.order_blocks)
    plan.nedge = plan.totblk * P

    slot = {}
    pos = 0
    for (t, w) in plan.order_blocks:
        if (t, w) not in slot:
            slot[(t, w)] = pos
        pos += P

    srcw = np.zeros((ncores, plan.nedge), np.int64)
    dstloc = np.full((ncores, plan.nedge), 999.0, np.float32)
    for bi, (t, w) in enumerate(plan.order_blocks):
        if w == 1:
            srcw[:, bi * P:(bi + 1) * P] = wrows - 1

    # order edges by (core, tile, win) groups
    gkey = (core * tpc + tileid) * 2 + hi
    g_order = np.argsort(gkey, kind="stable")
    ss2, loc2 = ss[g_order], loc[g_order]
    gstart = np.zeros(ncores * tpc * 2 + 1, np.int64)
    np.add.at(gstart[1:], gkey[g_order], 1)
    gstart = np.cumsum(gstart)
    for c in range(ncores):
        for t in range(tpc):
            for w in (0, 1):
                k = (c * tpc + t) * 2 + w
                e0, e1 = int(gstart[k]), int(gstart[k + 1])
                cnt = e1 - e0
                if cnt == 0:
                    continue
                off = slot[(t, w)]
                srcs = ss2[e0:e1]
                srcw[c, off:off + cnt] = (
                    srcs + 1 if w == 0 else srcs + 1 - plan.wb_base)
                dstloc[c, off:off + cnt] = (loc2[e0:e1] % P).astype(np.float32)

    plan.srcw, plan.dstlocv = srcw, dstloc
    return plan


def _edge_phase(nc, tc, pools, tabA_ap, tabB_ap, er_tile, d, nheads, hdim, gw,
                plan, src16_sb, dstloc_sb, iota_sb, ident_sb, slope_sb,
                finalize, tile_done=None):
    """Edge pipeline for one layer.  Gathered bf16 row: [h(d) | el(nheads) |
    pad], gw elems.  er comes from er_tile [P, tpc, nheads] (bf16 SBUF) via
    maskT matmul broadcast.  tile_done(t) is called after finalize(t)."""
    (hg_pool, ms_pool, mask_pool, mt_pool, small_pool, eb_pool,
     psum_acc, psum_tp, psum_er) = pools
    first_blk, last_blk = {}, {}
    for bi, (t, w) in enumerate(plan.order_blocks):
        if t not in first_blk:
            first_blk[t] = bi
        last_blk[t] = bi
    acc_by_tile = {}

    # chunks: maximal runs of <=G blocks within a single window
    chunks = []
    cur = None
    for bi, (t, w) in enumerate(plan.order_blocks):
        if cur is None or cur[0] != w or bi - cur[1] >= G:
            if cur is not None:
                chunks.append(cur)
            cur = [w, bi, bi + 1]
        else:
            cur[2] = bi + 1
        if cur[2] - cur[1] >= G:
            chunks.append(cur)
            cur = None
    if cur is not None:
        chunks.append(cur)

    for ci, (w, b0, b1) in enumerate(chunks):
        nb = b1 - b0
        nidx = nb * P
        HG = hg_pool.tile([P, G, gw], BF16, tag="hg", name="hg")
        nc.gpsimd.dma_gather(
            out_ap=HG[:, :nb, :], in_ap=(tabA_ap if w == 0 else tabB_ap),
            idxs_ap=src16_sb[:, b0 * 8:b1 * 8], num_idxs=nidx,
            num_idxs_reg=nidx, elem_size=gw, elem_step=gw,
            single_packet=False, queue_num=ci % NQ)
        # mask[e, b, j] = (dstloc[e, b] == j)   (bf16 0/1)
        MASK = mask_pool.tile([P, G, P], BF16, tag="mask", name="mask")
        nc.vector.tensor_tensor(
            out=MASK[:, :nb, :],
            in0=_bcast_inner(dstloc_sb[:, b0:b1], P),
            in1=_bcast_mid(iota_sb[:], nb),
            op=mybir.AluOpType.is_equal)
        # per-block: maskT (PE transpose) + er broadcast matmul into one
        # chunk-wide PSUM strip; then a single E4 = el + er add.
        EREC = psum_er.tile([P, G, nheads], F32, tag="erec", name="erec")
        for bi in range(b0, b1):
            t, _ = plan.order_blocks[bi]
            TP = psum_tp.tile([P, P], BF16, tag="tp", name="tp")
            nc.tensor.transpose(TP[:], MASK[:, bi - b0, :], ident_sb[:])
            MT = mt_pool.tile([P, P], BF16, tag="mt", name="mt")
            nc.vector.tensor_copy(MT[:], TP[:])
            nc.tensor.matmul(EREC[:, bi - b0, :], lhsT=MT[:],
                             rhs=er_tile[:, t, :], start=True, stop=True)
        E4 = small_pool.tile([P, G, nheads], F32, tag="e4", name="e4")
        nc.vector.tensor_add(E4[:, :nb, :], HG[:, :nb, d:d + nheads],
                             EREC[:, :nb, :])
        # expe = exp(lrelu(E4))  (lrelu via DVE mul+max; Exp on scalar), bf16
        ESC = small_pool.tile([P, G, nheads], F32, tag="esc", name="esc")
        nc.vector.tensor_tensor(out=ESC[:, :nb, :], in0=E4[:, :nb, :],
                                in1=_bcast_mid(slope_sb[:, 0:nheads], nb),
                                op=mybir.AluOpType.mult)
        nc.vector.tensor_tensor(out=E4[:, :nb, :], in0=E4[:, :nb, :],
                                in1=ESC[:, :nb, :], op=mybir.AluOpType.max)
        EB = eb_pool.tile([P, G, nheads], BF16, tag="eb", name="eb")
        nc.scalar.activation(out=EB[:, :nb, :], in_=E4[:, :nb, :],
                             func=mybir.ActivationFunctionType.Exp)
        # MS = [expe-scaled h | expe]  (bf16)
        msw = d + nheads
        MS = ms_pool.tile([P, G, msw], BF16, tag="ms", name="ms")
        for h in range(nheads):
            nc.vector.tensor_tensor(
                out=MS[:, :nb, h * hdim:(h + 1) * hdim],
                in0=HG[:, :nb, h * hdim:(h + 1) * hdim],
                in1=_bcast_inner(EB[:, :nb, h:h + 1], hdim),
                op=mybir.AluOpType.mult)
        nc.scalar.copy(out=MS[:, :nb, d:d + nheads], in_=EB[:, :nb, :])
        for bi in range(b0, b1):
            t, _ = plan.order_blocks[bi]
            if bi == first_blk[t]:
                acc_by_tile[t] = psum_acc.tile([P, msw], F32, tag="acc",
                                               name="acc")
            acc = acc_by_tile[t]
            nc.tensor.matmul(acc[:], lhsT=MASK[:, bi - b0, :],
                             rhs=MS[:, bi - b0, :],
                             start=(bi == first_blk[t]),
                             stop=(bi == last_blk[t]))
            if bi == last_blk[t]:
                finalize(t, acc)
                del acc_by_tile[t]
                if tile_done is not None:
                    tile_done(t)


def build_and_run(feat, src, dst, W0, al0, ar0, W1, al1, ar1, trace=False,
                  simulate=False):
    n_nodes = feat.shape[0]
    npc = n_nodes // NCORES
    nh0 = al0.shape[0]
    hid0 = al0.shape[1]
    d0 = nh0 * hid0                        # 256
    row0 = ((d0 + nh0 + 127) // 128) * 128  # 384 bf16 = 768B rows
    nh1 = al1.shape[0]
    hid1 = al1.shape[1]
    d1 = nh1 * hid1                        # 40
    row1 = ((d1 + 2 * nh1 + 127) // 128) * 128  # 128 bf16 = 256B rows
    in_dim = feat.shape[1]
    assert in_dim == P
    wrows = min(WROWS, n_nodes + 2)
    wb_base = n_nodes + 2 - wrows

    w0e = _fuse_w(W0, al0, ar0)            # [in_dim, d0+2nh0]
    w1e = _fuse_w(W1, al1, ar1)            # [d0, d1+2nh1]
    kchunks = d0 // P
    w1p = np.ascontiguousarray(
        w1e.reshape(kchunks, P, d1 + 2 * nh1).transpose(1, 0, 2)).astype(BFNP)
    featT = np.ascontiguousarray(feat.T).astype(BFNP)    # [128, N]

    plan = _prep_edges(src, dst, n_nodes, NCORES, wrows)
    totblk = plan.totblk
    tpc = plan.tpc
    npc_pad = tpc * P

    # layer-0 er gather: one idx per (tile t, partition j) -> er0p row
    # (32 bf16-nodes per 256B row); selection mask W picks the right 4 vals.
    er0p_rows = (n_nodes + 31) // 32 + 1
    eri = np.zeros((NCORES, npc_pad), np.int64)
    ersel = np.zeros((NCORES, P, 128), np.float32)
    for c in range(NCORES):
        g = c * npc + np.arange(npc_pad, dtype=np.int64)
        g = np.minimum(g, n_nodes - 1)
        eri[c] = g // 32
        sub = (c * npc + np.arange(P, dtype=np.int64)) % 32
        for j in range(P):
            ersel[c, j, 4 * sub[j]:4 * sub[j] + 4] = 1.0

    iota = np.broadcast_to(np.arange(P, dtype=np.float32), (P, P)).astype(BFNP)
    ident = np.eye(P, dtype=np.float32).astype(BFNP)

    nc = bacc.Bacc(None, target_bir_lowering=False, num_devices=NCORES,
                   num_swdge_queues=NQ)
    featT_t = nc.declare_dram_parameter("featT", [P, n_nodes], BF16, False)
    w0e_t = nc.declare_dram_parameter("w0e", [P, d0 + 2 * nh0], BF16, False)
    w1e_t = nc.declare_dram_parameter("w1e", [P, kchunks, d1 + 2 * nh1], BF16,
                                      False)
    iota_t = nc.declare_dram_parameter("iota", [P, P], BF16, False)
    ident_t = nc.declare_dram_parameter("ident", [P, P], BF16, False)
    src16_t = nc.declare_dram_parameter("src16", [P, totblk * 8], I16, False)
    dstloc_t = nc.declare_dram_parameter("dstloc", [P, totblk], BF16, False)
    eri16_t = nc.declare_dram_parameter("eri16", [P, npc_pad // 16], I16, False)
    ersel_t = nc.declare_dram_parameter("ersel", [P, 128], F32, False)
    out_t = nc.declare_dram_parameter("out", [npc, d1], F32, True)

    # single tables; window A = rows [0, wrows), window B = rows
    # [wb_base, wb_base+wrows) of the same tensor (node i lives at row i+1)
    tab0 = nc.dram_tensor("tab0", [n_nodes + 2, row0], BF16)
    tab1 = nc.dram_tensor("tab1", [n_nodes + 2, row1], BF16)
    er0p = nc.dram_tensor("er0p", [er0p_rows, 128], BF16)
    er1locT = nc.dram_tensor("er1locT", [P, tpc], BF16)
    h2slice = nc.dram_tensor("h2slice", [npc, d1 + 2 * nh1], BF16)
    NGRP = 4
    gsz = (tpc + NGRP - 1) // NGRP
    grp_bounds = []
    for g in range(NGRP):
        tlo, thi = g * gsz, min((g + 1) * gsz, tpc)
        if tlo < thi:
            grp_bounds.append((tlo * P, min(thi * P, npc), thi - 1))
    h2fullg = [
        nc.dram_tensor(f"h2full{g}", [NCORES, hi - lo, d1 + 2 * nh1], BF16,
                       addr_space="Shared")
        for g, (lo, hi, _) in enumerate(grp_bounds)]

    nt_full = n_nodes // P
    rem = n_nodes - nt_full * P
    wa_nodes = (0, wrows - 1)
    wb_nodes = (wb_base - 1, n_nodes)

    with tile.TileContext(nc) as tc:
        with tc.tile_pool(name="singles", bufs=1) as singles:
            iota_sb = singles.tile([P, P], BF16)
            nc.sync.dma_start(out=iota_sb[:], in_=iota_t.ap())
            ident_sb = singles.tile([P, P], BF16)
            nc.sync.dma_start(out=ident_sb[:], in_=ident_t.ap())
            w0e_sb = singles.tile([P, d0 + 2 * nh0], BF16)
            nc.sync.dma_start(out=w0e_sb[:], in_=w0e_t.ap())
            w1e_sb = singles.tile([P, kchunks, d1 + 2 * nh1], BF16)
            nc.sync.dma_start(out=w1e_sb[:], in_=w1e_t.ap())
            src16_sb = singles.tile([P, totblk * 8], I16)
            nc.sync.dma_start(out=src16_sb[:], in_=src16_t.ap())
            dstloc_sb = singles.tile([P, totblk], BF16)
            nc.sync.dma_start(out=dstloc_sb[:], in_=dstloc_t.ap())
            eri16_sb = singles.tile([P, npc_pad // 16], I16)
            nc.sync.dma_start(out=eri16_sb[:], in_=eri16_t.ap())
            ersel_sb = singles.tile([P, 128], F32)
            nc.sync.dma_start(out=ersel_sb[:], in_=ersel_t.ap())
            er0_tile = singles.tile([P, tpc, nh0], BF16)
            er1_tile = singles.tile([P, tpc, nh1], BF16)
            eps0 = singles.tile([P, nh0], F32)
            nc.vector.memset(eps0[:], 1e-30)
            eps1 = singles.tile([P, nh1], F32)
            nc.vector.memset(eps1[:], 1e-30)
            slope_sb = singles.tile([P, nh0], F32)
            nc.vector.memset(slope_sb[:], SLOPE)
            zrow = singles.tile([P, row0], BF16)
            nc.vector.memset(zrow[:], 0.0)
            # zero guard rows (row 0 and row n_nodes+1 of each table)
            nc.sync.dma_start(out=tab0.ap()[0:1], in_=zrow[:1, :row0])
            nc.sync.dma_start(out=tab0.ap()[n_nodes + 1:n_nodes + 2],
                              in_=zrow[:1, :row0])
            nc.sync.dma_start(out=tab1.ap()[0:1], in_=zrow[:1, :row1])
            nc.sync.dma_start(out=tab1.ap()[n_nodes + 1:n_nodes + 2],
                              in_=zrow[:1, :row1])

            # ---- Phase A: replicated dense layer 0 -> tab0 + er0p ----
            with (tc.tile_pool(name="pa", bufs=4) as pa,
                  tc.tile_pool(name="pa_ph", bufs=4, space="PSUM") as pa_ph):
                base = 0
                chunk_i = 0
                while base < n_nodes:
                    ch = min(CH, (n_nodes - base) // P)
                    partial = ch == 0
                    ch = max(ch, 1)
                    rows = rem if partial else ch * P
                    ftc = pa.tile([P, CH * P], BF16, tag="ftc", name="ftc")
                    nc.sync.dma_start(out=ftc[:, :rows],
                                      in_=featT_t.ap()[:, base:base + rows])
                    hstage = pa.tile([P, CH, row0], BF16, tag="hstage",
                                     name="hstage")
                    if chunk_i < 4:  # pool bufs: pad cols stay zero on reuse
                        nc.vector.memset(hstage[:, :, d0 + 2 * nh0:row0], 0.0)
                    chunk_i += 1
                    for i in range(ch):
                        m = rows - i * P if partial else P
                        hps = pa_ph.tile([P, d0 + 2 * nh0], F32, name="hps")
                        nc.tensor.matmul(hps[:m, :], lhsT=ftc[:, i * P:i * P + m],
                                         rhs=w0e_sb[:], start=True, stop=True)
                        nc.vector.tensor_copy(hstage[:m, i, 0:d0 + 2 * nh0],
                                              hps[:m, :])
                    # write chunk rows once into the single table
                    weng = nc.sync if (chunk_i % 2 == 0) else nc.scalar
                    if partial:
                        weng.dma_start(
                            out=tab0.ap()[base + 1:base + rows + 1, :],
                            in_=hstage[:rows, 0, :])
                    else:
                        weng.dma_start(
                            out=tab0.ap()[base + 1:base + rows + 1, :].rearrange(
                                "(i p) d -> p i d", p=P),
                            in_=hstage[:, :ch, :])
                    # er columns -> packed er0p (node-major bf16, 4 per node)
                    er0p_t = er0p.ap().tensor
                    if partial:
                        nc.sync.dma_start(
                            out=bass.AP(tensor=er0p_t, offset=base * 4,
                                        ap=[[4, rows], [1, 4]]),
                            in_=hstage[:rows, 0, d0 + nh0:d0 + 2 * nh0])
                    else:
                        nc.sync.dma_start(
                            out=bass.AP(tensor=er0p_t, offset=base * 4,
                                        ap=[[4, P], [4 * P, ch], [1, 4]]),
                            in_=hstage[:, :ch, d0 + nh0:d0 + 2 * nh0])
                    base += rows

            # ---- er0_tile: one gather + select-reduce ----
            with tc.tile_pool(name="ebld", bufs=1) as ebld:
                ERAW = ebld.tile([P, tpc, 128], BF16, tag="eraw", name="eraw")
                nc.gpsimd.dma_gather(
                    out_ap=ERAW[:], in_ap=er0p.ap(),
                    idxs_ap=eri16_sb[:], num_idxs=npc_pad,
                    num_idxs_reg=npc_pad, elem_size=128, elem_step=128,
                    single_packet=False, queue_num=1)
                EMUL = ebld.tile([P, tpc, 128], F32, tag="emul", name="emul")
                nc.vector.tensor_tensor(out=EMUL[:], in0=ERAW[:],
                                        in1=_bcast_mid(ersel_sb[:], tpc),
                                        op=mybir.AluOpType.mult)
                ERED = ebld.tile([P, tpc, nh0], F32, tag="ered", name="ered")
                nc.vector.tensor_reduce(
                    out=ERED[:],
                    in_=EMUL[:].rearrange("p t (s h) -> p t h s", h=nh0),
                    axis=mybir.AxisListType.X, op=mybir.AluOpType.add)
                nc.scalar.copy(out=er0_tile[:], in_=ERED[:])

            # ---- shared pools for edge phases ----
            with (tc.tile_pool(name="hg", bufs=6) as hg_pool,
                  tc.tile_pool(name="ms", bufs=4) as ms_pool,
                  tc.tile_pool(name="mk", bufs=4) as mask_pool,
                  tc.tile_pool(name="mt", bufs=6) as mt_pool,
                  tc.tile_pool(name="sm", bufs=3) as small_pool,
                  tc.tile_pool(name="eb", bufs=3) as eb_pool,
                  tc.tile_pool(name="fin", bufs=2) as fin_pool,
                  tc.tile_pool(name="ps_acc", bufs=4, space="PSUM") as psum_acc,
                  tc.tile_pool(name="ps_tp", bufs=2, space="PSUM") as psum_tp,
                  tc.tile_pool(name="ps_er", bufs=1, space="PSUM") as psum_er,
                  tc.tile_pool(name="ps_h2", bufs=1, space="PSUM") as psum_h2):

                def finalize0(t, acc):
                    rows = min(P, npc - t * P)
                    S = small_pool.tile([P, nh0], F32, tag="s0", name="s0")
                    nc.vector.tensor_tensor(out=S[:], in0=acc[:, d0:d0 + nh0],
                                            in1=eps0[:],
                                            op=mybir.AluOpType.max)
                    RC = small_pool.tile([P, nh0], F32, tag="rc0", name="rc0")
                    nc.vector.reciprocal(RC[:], S[:])
                    H1T = fin_pool.tile([P, d0], BF16, tag="h1t", name="h1t")
                    nc.vector.tensor_tensor(
                        out=H1T[:].rearrange("p (h e) -> p h e", h=nh0),
                        in0=acc[:, 0:d0].rearrange("p (h e) -> p h e", h=nh0),
                        in1=_bcast_inner(RC[:], hid0),
                        op=mybir.AluOpType.mult)
                    h2ps = psum_h2.tile([P, d1 + 2 * nh1], F32, name="h2ps")
                    for k in range(kchunks):
                        tp = psum_tp.tile([P, P], BF16, tag="tp", name="ftp")
                        nc.tensor.transpose(tp[:], H1T[:, k * P:(k + 1) * P],
                                            ident_sb[:])
                        ts = fin_pool.tile([P, P], BF16, tag="tsb", name="tsb")
                        nc.scalar.copy(out=ts[:], in_=tp[:])
                        nc.tensor.matmul(h2ps[:], lhsT=ts[:], rhs=w1e_sb[:, k, :],
                                         start=(k == 0), stop=(k == kchunks - 1))
                    h2sb = fin_pool.tile([P, d1 + 2 * nh1], BF16, tag="h2sb",
                                         name="h2sb")
                    nc.scalar.copy(out=h2sb[:], in_=h2ps[:])
                    nc.sync.dma_start(out=h2slice.ap()[t * P:t * P + rows],
                                      in_=h2sb[:rows, :])
                    nc.sync.dma_start(out=er1locT.ap()[:, t:t + 1],
                                      in_=h2sb[:, d1 + nh1:d1 + 2 * nh1])

                # chunked AllGather: after the last tile of each tile-group
                # finishes, gather that row range and repack it into tab1,
                # overlapping with the tail of the layer-0 edge phase.
                rw1 = d1 + 2 * nh1
                grp_last = {last_t: g for g, (_, _, last_t) in
                            enumerate(grp_bounds)}
                tab1_t = tab1.ap().tensor

                def tile_done0(t):
                    if t not in grp_last:
                        return
                    g = grp_last[t]
                    lo, hi, _ = grp_bounds[g]
                    nc.gpsimd.collective_compute(
                        "AllGather", mybir.AluOpType.bypass,
                        replica_groups=[list(range(NCORES))],
                        ins=[h2slice.ap()[lo:hi]],
                        outs=[h2fullg[g].ap()])
                    # tab1 row for node (c, l) is 1 + c*npc + l
                    out_ap = bass.AP(
                        tensor=tab1_t, offset=(1 + lo) * row1,
                        ap=[[npc * row1, NCORES], [row1, hi - lo], [1, rw1]])
                    nc.sync.dma_start(out=out_ap, in_=h2fullg[g].ap())

                _edge_phase(nc, tc,
                            (hg_pool, ms_pool, mask_pool, mt_pool, small_pool,
                             eb_pool, psum_acc, psum_tp, psum_er),
                            tab0.ap()[0:wrows], tab0.ap()[wb_base:wb_base + wrows],
                            er0_tile, d0, nh0, hid0,
                            row0, plan, src16_sb, dstloc_sb, iota_sb, ident_sb,
                            slope_sb, finalize0, tile_done=tile_done0)

                nc.sync.dma_start(
                    out=er1_tile[:, :, 0],
                    in_=er1locT.ap())

                def finalize1(t, acc):
                    rows = min(P, npc - t * P)
                    S = small_pool.tile([P, nh1], F32, tag="s1", name="s1")
                    nc.vector.tensor_tensor(out=S[:], in0=acc[:, d1:d1 + nh1],
                                            in1=eps1[:],
                                            op=mybir.AluOpType.max)
                    RC = small_pool.tile([P, nh1], F32, tag="rc1", name="rc1")
                    nc.vector.reciprocal(RC[:], S[:])
                    OUT = fin_pool.tile([P, d1], F32, tag="outt", name="outt")
                    nc.vector.tensor_tensor(out=OUT[:], in0=acc[:, 0:d1],
                                            in1=_bcast_inner(RC[:], d1),
                                            op=mybir.AluOpType.mult)
                    nc.sync.dma_start(out=out_t.ap()[t * P:t * P + rows],
                                      in_=OUT[:rows, :])

                _edge_phase(nc, tc,
                            (hg_pool, ms_pool, mask_pool, mt_pool, small_pool,
                             eb_pool, psum_acc, psum_tp, psum_er),
                            tab1.ap()[0:wrows], tab1.ap()[wb_base:wb_base + wrows],
                            er1_tile, d1, nh1, hid1,
                            row1, plan, src16_sb, dstloc_sb, iota_sb, ident_sb,
                            slope_sb, finalize1)

    nc.compile()
    if os.environ.get("GAT_COMPILE_ONLY", "0") == "1":
        LAST_BUILD[0] = (nc, None)
        return np.zeros((n_nodes, d1), np.float32)

    in_maps = []
    for c in range(NCORES):
        in_maps.append({
            "featT": featT,
            "w0e": np.ascontiguousarray(w0e).astype(BFNP),
            "w1e": w1p,
            "iota": np.ascontiguousarray(iota),
            "ident": np.ascontiguousarray(ident),
            "src16": _wrap16(plan.srcw[c]),
            "dstloc": np.ascontiguousarray(
                plan.dstlocv[c].reshape(totblk, P).T).astype(BFNP),
            "eri16": _wrap16(eri[c]),
            "ersel": np.ascontiguousarray(ersel[c]),
        })
    LAST_BUILD[0] = (nc, in_maps)
    if simulate:
        from concourse import bass_interp
        sim = bass_interp.MultiCoreSim(nc, NCORES, ignore_data_errors=True)
        for c in range(NCORES):
            for k, v in in_maps[c].items():
                sim.cores[c].tensor(k)[:] = v
        sim.simulate()
        LAST_SIM[0] = sim
        out = np.concatenate(
            [np.array(sim.cores[c].tensor("out")) for c in range(NCORES)], axis=0)
        return out
    res = run_bass_kernel_spmd(nc, in_maps, list(range(NCORES)), trace=trace)
    LAST_RES[0] = res
    LAST_EXEC_NS[0] = res.exec_time_ns
    out = np.concatenate([res.results[c]["out"] for c in range(NCORES)], axis=0)
    return out


def kernel(feat, src, dst, W0, al0, ar0, W1, al1, ar1):
    trace = os.environ.get("GAT_TRACE", "0") == "1"
    out = build_and_run(np.asarray(feat), np.asarray(src), np.asarray(dst),
                        np.asarray(W0), np.asarray(al0), np.asarray(ar0),
                        np.asarray(W1), np.asarray(al1), np.asarray(ar1),
                        trace=trace)
    return out.astype(np.float32)
